# revision 1
# baseline (speedup 1.0000x reference)
"""AttentionDTI forward pass on 8 Trainium2 NeuronCores (pure data parallel).

Batch of 8 peptide/MHC pairs; one batch element per core, weights
replicated. The 4D additive-attention tensor h[b,p,m,c] =
relu(p_att + m_att) is never materialized in HBM: since the following
linear layer is, well, linear, mean_m(h @ Wa) == mean_m(h) @ Wa, so the
kernel only accumulates hp[c,p] = sum_m h and hm[c,m] = sum_p h on the
fly. hp comes from the ScalarEngine's fused relu+bias+accum activation;
hm is accumulated in PSUM by streaming h tiles through the TensorEngine
against a stationary identity matrix.

Environment constraints discovered empirically (this axon terminal):
  - GPSIMD/Pool ucode ops (SWDGE DMA, gpsimd memset/iota) hang: all DMAs
    go through the sync-engine HWDGE, memsets through the VectorEngine.
  - scalar_tensor_tensor hangs: only tensor_scalar / tensor_tensor /
    tensor_reduce / activation / matmul are used.
  - walrus here allows at most ONE semaphore wait per instruction:
    _split_excess_waits() rewrites the Tile-scheduled program, moving
    excess waits onto standalone InstEventSemaphore instructions.
"""
import sys

_BASS_ROOT = '/opt/trn_rl_repo'
if _BASS_ROOT not in sys.path:
    sys.path.insert(0, _BASS_ROOT)

import numpy as np
import ml_dtypes

import concourse.bass as bass
import concourse.tile as tile
from concourse import mybir
from concourse.bass_utils import run_bass_kernel_spmd

F32 = mybir.dt.float32
BF16 = mybir.dt.bfloat16
ALU = mybir.AluOpType
AF = mybir.ActivationFunctionType
AX = mybir.AxisListType

# model dims (hardcoded from the problem spec)
B = 8
LP, LM, DIM, CONV = 100, 1000, 64, 40
C2, C4 = CONV * 2, CONV * 4          # 80, 160
K1, K2, K3 = 4, 6, 8
LP1, LP2, LP3 = 97, 92, 85           # peptide conv output lengths
LM1, LM2, LM3 = 997, 992, 985        # MHC conv output lengths
MPAD = 992                           # LM3 padded to a multiple of 32
NEG = -30000.0                       # -inf stand-in that survives bf16
NPB = 22                             # ceil(85/4) packed p-groups for the c[128:160] chunk

_ctr = [0]


def _split_excess_waits(nc, max_waits=1):
    n_split = 0
    for f in nc.m.functions:
        for b in f.blocks:
            insts = list(b.instructions)
            out = []
            changed = False
            for inst in insts:
                si = inst.sync_info
                waits = list(si.on_wait) if (si is not None and si.on_wait) else []
                if len(waits) > max_waits:
                    changed = True
                    n_split += 1
                    keep = max(1, max_waits)
                    head, tail = waits[:-keep], waits[-keep:]
                    for i in range(0, len(head), keep):
                        chunk = head[i:i + keep]
                        nop = mybir.InstEventSemaphore(
                            name=f"ant-wait-split-{_ctr[0]}", ins=[], outs=[])
                        _ctr[0] += 1
                        nop.engine = inst.engine
                        nop.sync_info = mybir.SyncInfo(on_wait=chunk, on_update=[])
                        nc.register_instruction(nop)
                        out.append(nop)
                    upd = list(si.on_update) if si.on_update else []
                    inst.sync_info = mybir.SyncInfo(on_wait=tail, on_update=upd)
                out.append(inst)
            if changed:
                b.instructions = out
    return n_split


def _conv_matmuls(nc, psum, wtile, x, k_taps, co_lo, co_hi, m_lo, m_hi, cout_stride):
    """Accumulate a valid 1-D conv as k shifted matmuls into `psum`.

    psum: [co_hi-co_lo, m_hi-m_lo]; wtile: [ci, K*cout_stride] with tap k
    at columns [k*cout_stride, (k+1)*cout_stride); x: [ci, L].
    """
    for k in range(k_taps):
        nc.tensor.matmul(
            psum,
            wtile[:, k * cout_stride + co_lo: k * cout_stride + co_hi],
            x[:, m_lo + k: m_hi + k],
            start=(k == 0), stop=(k == k_taps - 1))


def _build_program():
    nc = bass.Bass("TRN2", target_bir_lowering=False, debug=False)

    def par(name, shape, dtype=F32):
        return nc.declare_dram_parameter(name, list(shape), dtype, isOutput=False)

    # per-core activations
    poh_e = par("pep_oh", [26, LP])
    moh_e = par("mhc_oh", [26, LM])
    # tables / weights (identical on all cores)
    pemb_e = par("pep_emb", [26, DIM])
    memb_e = par("mhc_emb", [26, DIM])
    pw1_e = par("pw1t", [DIM, K1 * CONV])
    pw2_e = par("pw2t", [CONV, K2 * C2])
    pw3_e = par("pw3t", [C2, K3 * C4])
    mw1_e = par("mw1t", [DIM, K1 * CONV])
    mw2_e = par("mw2t", [CONV, K2 * C2])
    mw3_e = par("mw3t", [C2, K3 * C4])
    pb1_e = par("pb1", [CONV, 1]); pb2_e = par("pb2", [C2, 1]); pb3_e = par("pb3", [128, 2])
    mb1_e = par("mb1", [CONV, 1]); mb2_e = par("mb2", [C2, 1]); mb3_e = par("mb3", [128, 2])
    wpaa_e = par("wpa_a", [128, C4]); wpab_e = par("wpa_b", [32, C4])
    wmaa_e = par("wma_a", [128, C4]); wmab_e = par("wma_b", [32, C4])
    wcaa_e = par("wca_a", [128, C4]); wcab_e = par("wca_b", [32, C4])   # Wa/985
    wmaa2_e = par("wma2_a", [128, C4]); wmab2_e = par("wma2_b", [32, C4])  # Wa/85
    bpa_e = par("bpa", [128, 2]); bma_e = par("bma", [128, 2]); ba_e = par("ba", [128, 2])
    w1a_e = par("w1a", [128, 2 * 1024]); w1b_e = par("w1b", [32, 2 * 1024])
    w2_e = par("w2", [128, 8 * 1024])
    w3_e = par("w3", [128, 8 * 512])
    wo_e = par("wo", [128, 8])
    b1_e = par("b1", [128, 8]); b2_e = par("b2", [128, 8]); b3_e = par("b3", [128, 4])
    bo_e = par("bo", [2, 1])
    id128_e = par("ident128", [128, 128], BF16)
    idst_e = par("ident_st", [128, 32], BF16)

    out_e = nc.declare_dram_parameter("out", [2, 1], F32, isOutput=True)

    with tile.TileContext(nc) as tc:
        with tc.tile_pool(name="consts", bufs=1) as cp, \
             tc.tile_pool(name="work", bufs=1) as wp, \
             tc.tile_pool(name="hpool", bufs=6) as hpool, \
             tc.tile_pool(name="ps_hm", bufs=1, space="PSUM") as ps_hm, \
             tc.tile_pool(name="ps_work", bufs=2, space="PSUM") as ps:

            def load(ext, shape, dtype=F32, name=None):
                t = cp.tile(shape, dtype, name=name or ext.name + "_sb")
                nc.sync.dma_start(out=t, in_=ext[:])
                return t

            # ---- constant loads (small, needed early) ----
            poh = load(poh_e, [26, LP]); moh = load(moh_e, [26, LM])
            pemb = load(pemb_e, [26, DIM]); memb = load(memb_e, [26, DIM])
            pw1 = load(pw1_e, [DIM, K1 * CONV]); pw2 = load(pw2_e, [CONV, K2 * C2]); pw3 = load(pw3_e, [C2, K3 * C4])
            mw1 = load(mw1_e, [DIM, K1 * CONV]); mw2 = load(mw2_e, [CONV, K2 * C2]); mw3 = load(mw3_e, [C2, K3 * C4])
            pb1 = load(pb1_e, [CONV, 1]); pb2 = load(pb2_e, [C2, 1]); pb3 = load(pb3_e, [128, 2])
            mb1 = load(mb1_e, [CONV, 1]); mb2 = load(mb2_e, [C2, 1]); mb3 = load(mb3_e, [128, 2])
            wpaa = load(wpaa_e, [128, C4]); wpab = load(wpab_e, [32, C4])
            wmaa = load(wmaa_e, [128, C4]); wmab = load(wmab_e, [32, C4])
            wcaa = load(wcaa_e, [128, C4]); wcab = load(wcab_e, [32, C4])
            wmaa2 = load(wmaa2_e, [128, C4]); wmab2 = load(wmab2_e, [32, C4])
            bpa = load(bpa_e, [128, 2]); bma = load(bma_e, [128, 2]); ba = load(ba_e, [128, 2])
            id128 = load(id128_e, [128, 128], BF16); idst = load(idst_e, [128, 32], BF16)
            # FC weights (big, only needed at the very end)
            w1a = load(w1a_e, [128, 2048]); w1b = load(w1b_e, [32, 2048])
            w2 = load(w2_e, [128, 8192])
            w3 = load(w3_e, [128, 4096])
            wo = load(wo_e, [128, 8])
            b1 = load(b1_e, [128, 8]); b2 = load(b2_e, [128, 8]); b3 = load(b3_e, [128, 4])
            bo = load(bo_e, [2, 1])

            # ---- embeddings: emb.T @ onehot -> [DIM, L] ----
            pe_ps = ps.tile([DIM, LP], F32, name="pe_ps", tag="ps")
            nc.tensor.matmul(pe_ps, pemb, poh, start=True, stop=True)
            pe = wp.tile([DIM, LP], F32, name="pe")
            nc.scalar.copy(pe, pe_ps)

            me_ps = ps.tile([DIM, LM], F32, name="me_ps", tag="ps")
            nc.tensor.matmul(me_ps[:, 0:512], pemb if False else memb, moh[:, 0:512], start=True, stop=True)
            nc.tensor.matmul(me_ps[:, 512:LM], memb, moh[:, 512:LM], start=True, stop=True)
            me = wp.tile([DIM, LM], F32, name="me")
            nc.scalar.copy(me, me_ps)

            # ---- peptide conv stack ----
            px1_ps = ps.tile([CONV, LP1], F32, name="px1_ps", tag="ps")
            _conv_matmuls(nc, px1_ps, pw1, pe, K1, 0, CONV, 0, LP1, CONV)
            px1 = wp.tile([CONV, LP1], F32, name="px1")
            nc.scalar.activation(out=px1, in_=px1_ps, func=AF.Relu, bias=pb1[:, 0:1])

            px2_ps = ps.tile([C2, LP2], F32, name="px2_ps", tag="ps")
            _conv_matmuls(nc, px2_ps, pw2, px1, K2, 0, C2, 0, LP2, C2)
            px2 = wp.tile([C2, LP2], F32, name="px2")
            nc.scalar.activation(out=px2, in_=px2_ps, func=AF.Relu, bias=pb2[:, 0:1])

            pc0_ps = ps.tile([128, LP3], F32, name="pc0_ps", tag="ps")
            _conv_matmuls(nc, pc0_ps, pw3, px2, K3, 0, 128, 0, LP3, C4)
            pc0 = wp.tile([128, LP3], F32, name="pc0")
            nc.scalar.activation(out=pc0, in_=pc0_ps, func=AF.Relu, bias=pb3[:, 0:1])
            pc1_ps = ps.tile([32, LP3], F32, name="pc1_ps", tag="ps")
            _conv_matmuls(nc, pc1_ps, pw3, px2, K3, 128, C4, 0, LP3, C4)
            pc1 = wp.tile([32, LP3], F32, name="pc1")
            nc.scalar.activation(out=pc1, in_=pc1_ps, func=AF.Relu, bias=pb3[0:32, 1:2])

            # ---- MHC conv stack (free dim chunked to <=512) ----
            mx1_ps = ps.tile([CONV, LM1], F32, name="mx1_ps", tag="ps")
            _conv_matmuls(nc, mx1_ps[:, 0:512], mw1, me, K1, 0, CONV, 0, 512, CONV)
            _conv_matmuls(nc, mx1_ps[:, 512:LM1], mw1, me, K1, 0, CONV, 512, LM1, CONV)
            mx1 = wp.tile([CONV, LM1], F32, name="mx1")
            nc.scalar.activation(out=mx1, in_=mx1_ps, func=AF.Relu, bias=mb1[:, 0:1])

            mx2_ps = ps.tile([C2, LM2], F32, name="mx2_ps", tag="ps")
            _conv_matmuls(nc, mx2_ps[:, 0:512], mw2, mx1, K2, 0, C2, 0, 512, C2)
            _conv_matmuls(nc, mx2_ps[:, 512:LM2], mw2, mx1, K2, 0, C2, 512, LM2, C2)
            mx2 = wp.tile([C2, LM2], F32, name="mx2")
            nc.scalar.activation(out=mx2, in_=mx2_ps, func=AF.Relu, bias=mb2[:, 0:1])

            mc0_ps = ps.tile([128, LM3], F32, name="mc0_ps", tag="ps")
            _conv_matmuls(nc, mc0_ps[:, 0:512], mw3, mx2, K3, 0, 128, 0, 512, C4)
            _conv_matmuls(nc, mc0_ps[:, 512:LM3], mw3, mx2, K3, 0, 128, 512, LM3, C4)
            mc0 = wp.tile([128, LM3], F32, name="mc0")
            nc.scalar.activation(out=mc0, in_=mc0_ps, func=AF.Relu, bias=mb3[:, 0:1])
            mc1_ps = ps.tile([32, LM3], F32, name="mc1_ps", tag="ps")
            _conv_matmuls(nc, mc1_ps[:, 0:512], mw3, mx2, K3, 128, C4, 0, 512, C4)
            _conv_matmuls(nc, mc1_ps[:, 512:LM3], mw3, mx2, K3, 128, C4, 512, LM3, C4)
            mc1 = wp.tile([32, LM3], F32, name="mc1")
            nc.scalar.activation(out=mc1, in_=mc1_ps, func=AF.Relu, bias=mb3[0:32, 1:2])

            # ---- attention projections ----
            # pa[c,p] = sum_c' pc[c',p] * Wpa[c',c] + bpa[c]
            pa0_ps = ps.tile([128, LP3], F32, name="pa0_ps", tag="ps")
            nc.tensor.matmul(pa0_ps, wpaa[:, 0:128], pc0, start=True, stop=False)
            nc.tensor.matmul(pa0_ps, wpab[:, 0:128], pc1, start=False, stop=True)
            pa0 = wp.tile([128, LP3], F32, name="pa0")
            nc.scalar.add(pa0, pa0_ps, bpa[:, 0:1])

            pa1_ps = ps.tile([32, LP3], F32, name="pa1_ps", tag="ps")
            nc.tensor.matmul(pa1_ps, wpaa[:, 128:C4], pc0, start=True, stop=False)
            nc.tensor.matmul(pa1_ps, wpab[:, 128:C4], pc1, start=False, stop=True)
            pa1 = wp.tile([32, 4 * NPB], F32, name="pa1")
            nc.vector.memset(pa1, NEG)
            nc.scalar.add(pa1[:, 0:LP3], pa1_ps, bpa[0:32, 1:2])
            # pack 4 p-positions per 32-row block: pa1p[32j+d, g] = pa1[d, 4g+j]
            pa1p = wp.tile([128, NPB], F32, name="pa1p")
            pa1_g = pa1.rearrange("d (g f) -> d g f", f=4)
            for j in range(4):
                nc.sync.dma_start(
                    out=pa1p[32 * j:32 * (j + 1), 0:NPB],
                    in_=pa1_g[:, :, j])

            # ma[c,m] = sum_c' mc[c',m] * Wma[c',c] + bma[c]  (bf16, m padded with NEG)
            ma0_ps = ps.tile([128, LM3], F32, name="ma0_ps", tag="ps")
            nc.tensor.matmul(ma0_ps[:, 0:512], wmaa[:, 0:128], mc0[:, 0:512], start=True, stop=False)
            nc.tensor.matmul(ma0_ps[:, 0:512], wmab[:, 0:128], mc1[:, 0:512], start=False, stop=True)
            nc.tensor.matmul(ma0_ps[:, 512:LM3], wmaa[:, 0:128], mc0[:, 512:LM3], start=True, stop=False)
            nc.tensor.matmul(ma0_ps[:, 512:LM3], wmab[:, 0:128], mc1[:, 512:LM3], start=False, stop=True)
            ma0 = wp.tile([128, MPAD], BF16, name="ma0")
            nc.vector.memset(ma0, NEG)
            nc.scalar.add(ma0[:, 0:LM3], ma0_ps, bma[:, 0:1])

            ma1_ps = ps.tile([32, LM3], F32, name="ma1_ps", tag="ps")
            nc.tensor.matmul(ma1_ps[:, 0:512], wmaa[:, 128:C4], mc0[:, 0:512], start=True, stop=False)
            nc.tensor.matmul(ma1_ps[:, 0:512], wmab[:, 128:C4], mc1[:, 0:512], start=False, stop=True)
            nc.tensor.matmul(ma1_ps[:, 512:LM3], wmaa[:, 128:C4], mc0[:, 512:LM3], start=True, stop=False)
            nc.tensor.matmul(ma1_ps[:, 512:LM3], wmab[:, 128:C4], mc1[:, 512:LM3], start=False, stop=True)
            ma1 = wp.tile([32, MPAD], BF16, name="ma1")
            nc.vector.memset(ma1, NEG)
            nc.scalar.add(ma1[:, 0:LM3], ma1_ps, bma[0:32, 1:2])
            # replicate 4x vertically for the packed c[128:160] loop
            ma1p = wp.tile([128, MPAD], BF16, name="ma1p")
            for j in range(4):
                nc.sync.dma_start(out=ma1p[32 * j:32 * (j + 1), :], in_=ma1[:, :])

            # ---- 4D attention reductions ----
            hp0 = wp.tile([128, 88], F32, name="hp0")
            hp1p = wp.tile([128, NPB], F32, name="hp1p")
            hm0_ps = ps_hm.tile([128, MPAD], F32, name="hm0_ps")
            hm1_ps = ps_hm.tile([32, MPAD], F32, name="hm1_ps")

            for p in range(LP3):
                h = hpool.tile([128, MPAD], BF16, tag="h", name="h")
                nc.scalar.activation(out=h, in_=ma0, func=AF.Relu,
                                     bias=pa0[:, p:p + 1], accum_out=hp0[:, p:p + 1])
                nc.tensor.matmul(hm0_ps[:, 0:512], id128, h[:, 0:512],
                                 start=(p == 0), stop=(p == LP3 - 1))
                nc.tensor.matmul(hm0_ps[:, 512:MPAD], id128, h[:, 512:MPAD],
                                 start=(p == 0), stop=(p == LP3 - 1))
            for g in range(NPB):
                h = hpool.tile([128, MPAD], BF16, tag="h", name="h")
                nc.scalar.activation(out=h, in_=ma1p, func=AF.Relu,
                                     bias=pa1p[:, g:g + 1], accum_out=hp1p[:, g:g + 1])
                nc.tensor.matmul(hm1_ps[:, 0:512], idst, h[:, 0:512],
                                 start=(g == 0), stop=(g == NPB - 1))
                nc.tensor.matmul(hm1_ps[:, 512:MPAD], idst, h[:, 512:MPAD],
                                 start=(g == 0), stop=(g == NPB - 1))

            # unpack hp1p -> hp1[d, 4g+j]
            hp1 = wp.tile([32, 88], F32, name="hp1")
            hp1_g = hp1.rearrange("d (g f) -> d g f", f=4)
            for j in range(4):
                nc.sync.dma_start(
                    out=hp1_g[:, :, j],
                    in_=hp1p[32 * j:32 * j + 32, 0:NPB])

            # ---- peptide attention gate ----
            # catt[d,p] = sigmoid(sum_c hp[c,p]/LM3 * Wa[c,d] + ba[d]); 1/LM3 folded into wca
            cl0_ps = ps.tile([128, LP3], F32, name="cl0_ps", tag="ps")
            nc.tensor.matmul(cl0_ps, wcaa[:, 0:128], hp0[:, 0:LP3], start=True, stop=False)
            nc.tensor.matmul(cl0_ps, wcab[:, 0:128], hp1[:, 0:LP3], start=False, stop=True)
            catt0 = wp.tile([128, LP3], F32, name="catt0")
            nc.scalar.activation(out=catt0, in_=cl0_ps, func=AF.Sigmoid, bias=ba[:, 0:1])
            cl1_ps = ps.tile([32, LP3], F32, name="cl1_ps", tag="ps")
            nc.tensor.matmul(cl1_ps, wcaa[:, 128:C4], hp0[:, 0:LP3], start=True, stop=False)
            nc.tensor.matmul(cl1_ps, wcab[:, 128:C4], hp1[:, 0:LP3], start=False, stop=True)
            catt1 = wp.tile([32, LP3], F32, name="catt1")
            nc.scalar.activation(out=catt1, in_=cl1_ps, func=AF.Sigmoid, bias=ba[0:32, 1:2])

            pg0 = wp.tile([128, LP3], F32, name="pg0")
            nc.vector.tensor_scalar(out=catt0, in0=catt0, scalar1=0.5, scalar2=None, op0=ALU.add)
            nc.vector.tensor_tensor(out=pg0, in0=catt0, in1=pc0, op=ALU.mult)
            pv0 = wp.tile([128, 1], F32, name="pv0")
            nc.vector.tensor_reduce(out=pv0, in_=pg0, op=ALU.max, axis=AX.X)
            pg1 = wp.tile([32, LP3], F32, name="pg1")
            nc.vector.tensor_scalar(out=catt1, in0=catt1, scalar1=0.5, scalar2=None, op0=ALU.add)
            nc.vector.tensor_tensor(out=pg1, in0=catt1, in1=pc1, op=ALU.mult)
            pv1 = wp.tile([32, 1], F32, name="pv1")
            nc.vector.tensor_reduce(out=pv1, in_=pg1, op=ALU.max, axis=AX.X)

            # ---- MHC attention gate ----
            hm0 = wp.tile([128, LM3], F32, name="hm0")
            nc.scalar.copy(hm0, hm0_ps[:, 0:LM3])
            hm1 = wp.tile([32, LM3], F32, name="hm1")
            nc.scalar.copy(hm1, hm1_ps[:, 0:LM3])

            matt0 = wp.tile([128, LM3], F32, name="matt0")
            ml0_ps = ps.tile([128, LM3], F32, name="ml0_ps", tag="ps")
            for lo, hi in ((0, 512), (512, LM3)):
                nc.tensor.matmul(ml0_ps[:, lo:hi], wmaa2[:, 0:128], hm0[:, lo:hi], start=True, stop=False)
                nc.tensor.matmul(ml0_ps[:, lo:hi], wmab2[:, 0:128], hm1[:, lo:hi], start=False, stop=True)
            nc.scalar.activation(out=matt0, in_=ml0_ps, func=AF.Sigmoid, bias=ba[:, 0:1])
            matt1 = wp.tile([32, LM3], F32, name="matt1")
            ml1_ps = ps.tile([32, LM3], F32, name="ml1_ps", tag="ps")
            for lo, hi in ((0, 512), (512, LM3)):
                nc.tensor.matmul(ml1_ps[:, lo:hi], wmaa2[:, 128:C4], hm0[:, lo:hi], start=True, stop=False)
                nc.tensor.matmul(ml1_ps[:, lo:hi], wmab2[:, 128:C4], hm1[:, lo:hi], start=False, stop=True)
            nc.scalar.activation(out=matt1, in_=ml1_ps, func=AF.Sigmoid, bias=ba[0:32, 1:2])

            mg0 = wp.tile([128, LM3], F32, name="mg0")
            nc.vector.tensor_scalar(out=matt0, in0=matt0, scalar1=0.5, scalar2=None, op0=ALU.add)
            nc.vector.tensor_tensor(out=mg0, in0=matt0, in1=mc0, op=ALU.mult)
            mv0 = wp.tile([128, 1], F32, name="mv0")
            nc.vector.tensor_reduce(out=mv0, in_=mg0, op=ALU.max, axis=AX.X)
            mg1 = wp.tile([32, LM3], F32, name="mg1")
            nc.vector.tensor_scalar(out=matt1, in0=matt1, scalar1=0.5, scalar2=None, op0=ALU.add)
            nc.vector.tensor_tensor(out=mg1, in0=matt1, in1=mc1, op=ALU.mult)
            mv1 = wp.tile([32, 1], F32, name="mv1")
            nc.vector.tensor_reduce(out=mv1, in_=mg1, op=ALU.max, axis=AX.X)

            # ---- FC head: outputs laid out [128 partitions, block] ----
            def fc_layer(name, w_tiles_rhs, nblk, blk_w, bias, nout_cols):
                """w_tiles_rhs: list of (wtile, col_base, rhs [K,1]) accumulated per block."""
                f_ps = ps.tile([128, nout_cols], F32, name=name + "_ps", tag="ps")
                for a in range(nblk):
                    n = len(w_tiles_rhs)
                    for i, (wt, base, rhs) in enumerate(w_tiles_rhs):
                        nc.tensor.matmul(
                            f_ps[:, a:a + 1],
                            wt[:, base + a * 128: base + a * 128 + 128],
                            rhs,
                            start=(i == 0), stop=(i == n - 1))
                fb = wp.tile([128, nout_cols], F32, name=name + "_b")
                nc.vector.tensor_tensor(out=fb, in0=f_ps, in1=bias, op=ALU.add)
                fs = wp.tile([128, nout_cols], F32, name=name + "_s")
                nc.vector.tensor_scalar(out=fs, in0=fb, scalar1=0.01, scalar2=None, op0=ALU.mult)
                fo = wp.tile([128, nout_cols], F32, name=name)
                nc.vector.tensor_tensor(out=fo, in0=fb, in1=fs, op=ALU.max)
                return fo

            f1 = fc_layer("f1", [(w1a, 0, pv0), (w1b, 0, pv1), (w1a, 1024, mv0), (w1b, 1024, mv1)],
                          8, 128, b1, 8)
            f2 = fc_layer("f2", [(w2, jb * 1024, f1[:, jb:jb + 1]) for jb in range(8)], 8, 128, b2, 8)
            f3 = fc_layer("f3", [(w3, jb * 512, f2[:, jb:jb + 1]) for jb in range(8)], 4, 128, b3, 4)

            o_ps = ps.tile([2, 1], F32, name="o_ps", tag="ps")
            for c in range(4):
                nc.tensor.matmul(o_ps, wo[:, 2 * c:2 * c + 2], f3[:, c:c + 1],
                                 start=(c == 0), stop=(c == 3))
            o_sb = wp.tile([2, 1], F32, name="o_sb")
            nc.vector.tensor_tensor(out=o_sb, in0=o_ps, in1=bo, op=ALU.add)
            nc.sync.dma_start(out=out_e[:], in_=o_sb)

    _split_excess_waits(nc, max_waits=1)
    return nc


_PROGRAM = None


def _get_program():
    global _PROGRAM
    if _PROGRAM is None:
        _PROGRAM = _build_program()
    return _PROGRAM


def _prep_weights(inp):
    """Host-side layout prep shared by all cores. All f32 contiguous."""
    f = lambda x: np.ascontiguousarray(np.asarray(x, dtype=np.float32))
    bf = lambda x: np.ascontiguousarray(np.asarray(x).astype(ml_dtypes.bfloat16))

    def convw(w):  # [co, ci, k] -> [ci, k*co]
        w = np.asarray(w, dtype=np.float32)
        ci = w.shape[1]
        return np.ascontiguousarray(w.transpose(1, 2, 0).reshape(ci, -1))

    def bias2(b):  # [160] -> [128, 2] (col 0 = [0:128], col 1 rows 0:32 = [128:160])
        b = np.asarray(b, dtype=np.float32)
        out = np.zeros((128, 2), np.float32)
        out[:, 0] = b[0:128]
        out[0:32, 1] = b[128:160]
        return out

    def fcw(w, nblk):  # [I, J] with I = nblk*128 -> [128, nblk*J]
        w = np.asarray(w, dtype=np.float32)
        i, j = w.shape
        return np.ascontiguousarray(w.reshape(nblk, 128, j).transpose(1, 0, 2).reshape(128, nblk * j))

    def fcb(b, nblk):  # [nblk*128] -> [128, nblk]
        b = np.asarray(b, dtype=np.float32)
        return np.ascontiguousarray(b.reshape(nblk, 128).T)

    wa985 = np.asarray(inp['Wa'], np.float32) / float(LM3)
    wa85 = np.asarray(inp['Wa'], np.float32) / float(LP3)
    w1 = np.asarray(inp['W1'], np.float32)
    d = {
        'pep_emb': f(inp['pep_emb']), 'mhc_emb': f(inp['mhc_emb']),
        'pw1t': convw(inp['pw1']), 'pw2t': convw(inp['pw2']), 'pw3t': convw(inp['pw3']),
        'mw1t': convw(inp['mw1']), 'mw2t': convw(inp['mw2']), 'mw3t': convw(inp['mw3']),
        'pb1': f(inp['pb1']).reshape(CONV, 1), 'pb2': f(inp['pb2']).reshape(C2, 1), 'pb3': bias2(inp['pb3']),
        'mb1': f(inp['mb1']).reshape(CONV, 1), 'mb2': f(inp['mb2']).reshape(C2, 1), 'mb3': bias2(inp['mb3']),
        'wpa_a': f(inp['Wpa'][0:128]), 'wpa_b': f(inp['Wpa'][128:160]),
        'wma_a': f(inp['Wma'][0:128]), 'wma_b': f(inp['Wma'][128:160]),
        'wca_a': f(wa985[0:128]), 'wca_b': f(wa985[128:160]),
        'wma2_a': f(wa85[0:128]), 'wma2_b': f(wa85[128:160]),
        'bpa': bias2(inp['bpa']), 'bma': bias2(inp['bma']), 'ba': bias2(inp['ba']),
        'w1a': np.ascontiguousarray(np.concatenate([w1[0:128], w1[160:288]], axis=1)),
        'w1b': np.ascontiguousarray(np.concatenate([w1[128:160], w1[288:320]], axis=1)),
        'w2': fcw(inp['W2'], 8), 'w3': fcw(inp['W3'], 8),
        'wo': fcw(inp['Wo'], 4),
        'b1': fcb(inp['b1'], 8), 'b2': fcb(inp['b2'], 8),
        'b3': np.ascontiguousarray(np.asarray(inp['b3'], np.float32).reshape(4, 128).T),
        'bo': f(inp['bo']).reshape(2, 1),
        'ident128': np.eye(128, dtype=ml_dtypes.bfloat16),
        'ident_st': np.ascontiguousarray(np.tile(np.eye(32, dtype=ml_dtypes.bfloat16), (4, 1))),
    }
    return d


def _onehot(idx, length):
    idx = np.asarray(idx).astype(np.int64)
    oh = np.zeros((26, length), np.float32)
    oh[idx, np.arange(length)] = 1.0
    return oh


def kernel(**inputs):
    nc = _get_program()
    shared = _prep_weights(inputs)
    peptide = np.asarray(inputs['peptide'])
    mhc = np.asarray(inputs['MHC'])
    in_maps = []
    for b in range(B):
        m = dict(shared)
        m['pep_oh'] = _onehot(peptide[b], LP)
        m['mhc_oh'] = _onehot(mhc[b], LM)
        in_maps.append(m)
    res = run_bass_kernel_spmd(nc, in_maps, core_ids=list(range(B)))
    return np.stack([np.asarray(res.results[i]['out']).reshape(2) for i in range(B)]).astype(np.float32)



# revision 7
# speedup vs baseline: 2.1226x; 2.1226x over previous
"""AttentionDTI forward pass on 8 Trainium2 NeuronCores (pure data parallel).

One batch element per core, weights replicated. All matmul operands are
fp16 (f32 PSUM accumulation): the PE runs 1 cycle/col instead of f32's 4,
and the DVE gets its 2-byte fast modes. Embedding lookup is done host-side
(the kernel receives gathered [64, 1100] per-core embeddings), and all
weights arrive in four packed DMAs (f32 smalls, conv pack, attention pack,
FC pack) to keep the SP sequencer's per-DMA cost off the critical path.

The 4D additive-attention tensor h[b,p,m,c] = relu(pa + ma) is never
materialized: mean_m(h @ Wa) == mean_m(h) @ Wa, so only hp[c,p] = sum_m h
and hm[c,m] = sum_p h are accumulated on the fly. hm accumulates in PSUM
by streaming h tiles through the PE against a stationary fp16 identity;
hp comes from per-tile accumulator outputs. h tiles are produced by BOTH
the Scalar engine (fused relu+bias+accum activation) and the Vector
engine, interleaved, so neither producer is the bottleneck.

c-channels [128:160] are handled packed: ma rows replicated 4x vertically
(via replicated stationary columns, free) so each tile covers 4 peptide
positions; a 4-stacked [128,32] identity reduces them into hm1.

Environment constraints discovered empirically (this axon terminal):
  - GPSIMD/Pool ucode ops and SWDGE DMA hang: DMAs go through the
    sync-engine HWDGE only.
  - scalar_tensor_tensor fails to compile.
  - walrus allows at most ONE semaphore wait per instruction:
    _split_excess_waits() rewrites the scheduled program.
"""
import sys

_BASS_ROOT = '/opt/trn_rl_repo'
if _BASS_ROOT not in sys.path:
    sys.path.insert(0, _BASS_ROOT)

import numpy as np

import concourse.bass as bass
import concourse.tile as tile
from concourse import mybir
from concourse.bass_utils import run_bass_kernel_spmd

F32 = mybir.dt.float32
F16 = mybir.dt.float16
ALU = mybir.AluOpType
AF = mybir.ActivationFunctionType
AX = mybir.AxisListType

B = 8
LP, LM, DIM, CONV = 100, 1000, 64, 40
C2, C4 = CONV * 2, CONV * 4          # 80, 160
K1, K2, K3 = 4, 6, 8
LP1, LP2, LP3 = 97, 92, 85           # peptide conv output lengths
LM1, LM2, LM3 = 997, 992, 985        # MHC conv output lengths
NP4 = 22                             # ceil(85/4) packed p-groups
NEG = -30000.0

# h-loop producers: Scalar activation (fused relu+bias+accum, ~1199
# ns/tile) interleaved 1:1 with Vector scalar_tensor_tensor (relu via
# max-with-zeros, fused f32 sum accum, ~1250 ns/tile). Any DVE op with
# an accum output runs at the 1x element rate (measured), so the fused
# single-pass form is optimal on both engines.
SPLIT_MOD = 2                   # i % SPLIT_MOD == 0 -> scalar tile

# ---- wconv column map (fp16 [128, 3840]) ----
PW1, PW2, PW3 = 0, 160, 640
MW1, MW2, MW3 = 1920, 2080, 2560
NCONV = 3840
# ---- wattn column map (fp16 [128, 1632]) ----
WPA_A, WPA_B = 0, 160        # [128,160], [32,160]
WMA_A, WMA_B = 320, 576      # [128,256], [32,256] (cols 128:256 = rep4 of Wma[:,128:160])
WCA_A, WCA_B = 832, 992      # Wa/LM3: [128,160], [32,160]
WM2_A, WM2_B = 1152, 1312    # Wa/LP3
ID128, IDST = 1472, 1600     # [128,128], [128,32]
NATTN = 1632
# ---- wfc column map (fp16 [128, 16392]) ----
W1A, W1B = 0, 2048           # [128, 2048], [32, 2048]
W2C, W3C, WOC = 4096, 12288, 16384
NFC = 16392
# ---- wsmall column map (f32 [128, 35]) ----
SB_PB1, SB_PB2, SB_PB3A, SB_PB3B = 0, 1, 2, 3
SB_MB1, SB_MB2, SB_MB3A, SB_MB3B = 4, 5, 6, 7
SB_BPA_A, SB_BPA_B = 8, 9
SB_BMA_A, SB_BMA_R4 = 10, 11
SB_BA_A, SB_BA_B = 12, 13
SB_B1, SB_B2, SB_B3, SB_BO = 14, 22, 30, 34
NSMALL = 35

_ctr = [0]


def _split_excess_waits(nc, max_waits=1):
    n_split = 0
    for f in nc.m.functions:
        for b in f.blocks:
            insts = list(b.instructions)
            out = []
            changed = False
            for inst in insts:
                si = inst.sync_info
                waits = list(si.on_wait) if (si is not None and si.on_wait) else []
                if len(waits) > max_waits:
                    changed = True
                    n_split += 1
                    keep = max(1, max_waits)
                    head, tail = waits[:-keep], waits[-keep:]
                    for i in range(0, len(head), keep):
                        chunk = head[i:i + keep]
                        nop = mybir.InstEventSemaphore(
                            name=f"ant-wait-split-{_ctr[0]}", ins=[], outs=[])
                        _ctr[0] += 1
                        nop.engine = inst.engine
                        nop.sync_info = mybir.SyncInfo(on_wait=chunk, on_update=[])
                        nc.register_instruction(nop)
                        out.append(nop)
                    upd = list(si.on_update) if si.on_update else []
                    inst.sync_info = mybir.SyncInfo(on_wait=tail, on_update=upd)
                out.append(inst)
            if changed:
                b.instructions = out
    return n_split


def _conv_matmuls(nc, psum, wtile, x, k_taps, co_lo, co_hi, m_lo, m_hi, cout_stride):
    """Valid 1-D conv as k shifted matmuls accumulated into `psum`."""
    for k in range(k_taps):
        nc.tensor.matmul(
            psum,
            wtile[:, k * cout_stride + co_lo: k * cout_stride + co_hi],
            x[:, m_lo + k: m_hi + k],
            start=(k == 0), stop=(k == k_taps - 1))


def _build_program():
    nc = bass.Bass("TRN2", target_bir_lowering=False, debug=False)

    emb_e = nc.declare_dram_parameter("emb", [DIM, LP + LM], F16, isOutput=False)
    wsmall_e = nc.declare_dram_parameter("wsmall", [128, NSMALL], F32, isOutput=False)
    wconv_e = nc.declare_dram_parameter("wconv", [128, NCONV], F16, isOutput=False)
    wattn_e = nc.declare_dram_parameter("wattn", [128, NATTN], F16, isOutput=False)
    wfc_e = nc.declare_dram_parameter("wfc", [128, NFC], F16, isOutput=False)
    out_e = nc.declare_dram_parameter("out", [2, 1], F32, isOutput=True)

    with tile.TileContext(nc) as tc:
        with tc.tile_pool(name="consts", bufs=1) as cp, \
             tc.tile_pool(name="work", bufs=1) as wp, \
             tc.tile_pool(name="hpool", bufs=8) as hpool, \
             tc.tile_pool(name="ps_hm", bufs=1, space="PSUM") as ps_hm, \
             tc.tile_pool(name="ps_work", bufs=2, space="PSUM") as ps:

            wsmall = cp.tile([128, NSMALL], F32, name="wsmall")
            nc.sync.dma_start(out=wsmall, in_=wsmall_e[:])
            emb = cp.tile([DIM, LP + LM], F16, name="emb")
            nc.sync.dma_start(out=emb, in_=emb_e[:])
            wconv = cp.tile([128, NCONV], F16, name="wconv")
            nc.sync.dma_start(out=wconv, in_=wconv_e[:])
            wattn = cp.tile([128, NATTN], F16, name="wattn")
            nc.sync.dma_start(out=wattn, in_=wattn_e[:])
            wfc = cp.tile([128, NFC], F16, name="wfc")
            nc.sync.dma_start(out=wfc, in_=wfc_e[:])

            pe = emb[:, 0:LP]
            me = emb[:, LP:LP + LM]
            bias = lambda col, rows=128: wsmall[0:rows, col:col + 1]

            # ================= conv stacks (fp16, f32 psum) =================
            # MHC conv1: [64,1000] -> [40,997]
            mx1_ps = ps.tile([CONV, LM1], F32, name="mx1_ps", tag="ps")
            _conv_matmuls(nc, mx1_ps[:, 0:512], wconv[0:DIM, MW1:MW1 + K1 * CONV], me, K1, 0, CONV, 0, 512, CONV)
            _conv_matmuls(nc, mx1_ps[:, 512:LM1], wconv[0:DIM, MW1:MW1 + K1 * CONV], me, K1, 0, CONV, 512, LM1, CONV)
            mx1 = wp.tile([CONV, LM1], F16, name="mx1")
            nc.scalar.activation(out=mx1, in_=mx1_ps, func=AF.Relu, bias=bias(SB_MB1, CONV))

            # peptide conv1: [64,100] -> [40,97]
            px1_ps = ps.tile([CONV, LP1], F32, name="px1_ps", tag="ps")
            _conv_matmuls(nc, px1_ps, wconv[0:DIM, PW1:PW1 + K1 * CONV], pe, K1, 0, CONV, 0, LP1, CONV)
            px1 = wp.tile([CONV, LP1], F16, name="px1")
            nc.scalar.activation(out=px1, in_=px1_ps, func=AF.Relu, bias=bias(SB_PB1, CONV))

            # MHC conv2: -> [80, 992]
            mx2_ps = ps.tile([C2, LM2], F32, name="mx2_ps", tag="ps")
            _conv_matmuls(nc, mx2_ps[:, 0:512], wconv[0:CONV, MW2:MW2 + K2 * C2], mx1, K2, 0, C2, 0, 512, C2)
            _conv_matmuls(nc, mx2_ps[:, 512:LM2], wconv[0:CONV, MW2:MW2 + K2 * C2], mx1, K2, 0, C2, 512, LM2, C2)
            mx2 = wp.tile([C2, LM2], F16, name="mx2")
            nc.scalar.activation(out=mx2, in_=mx2_ps, func=AF.Relu, bias=bias(SB_MB2, C2))

            px2_ps = ps.tile([C2, LP2], F32, name="px2_ps", tag="ps")
            _conv_matmuls(nc, px2_ps, wconv[0:CONV, PW2:PW2 + K2 * C2], px1, K2, 0, C2, 0, LP2, C2)
            px2 = wp.tile([C2, LP2], F16, name="px2")
            nc.scalar.activation(out=px2, in_=px2_ps, func=AF.Relu, bias=bias(SB_PB2, C2))

            # MHC conv3: -> [160, 985] as [128,985]+[32,985]
            mc0_ps = ps.tile([128, LM3], F32, name="mc0_ps", tag="ps")
            _conv_matmuls(nc, mc0_ps[:, 0:512], wconv[0:C2, MW3:MW3 + K3 * C4], mx2, K3, 0, 128, 0, 512, C4)
            _conv_matmuls(nc, mc0_ps[:, 512:LM3], wconv[0:C2, MW3:MW3 + K3 * C4], mx2, K3, 0, 128, 512, LM3, C4)
            mc0 = wp.tile([128, LM3], F16, name="mc0")
            nc.scalar.activation(out=mc0, in_=mc0_ps, func=AF.Relu, bias=bias(SB_MB3A))
            mc1_ps = ps.tile([32, LM3], F32, name="mc1_ps", tag="ps")
            _conv_matmuls(nc, mc1_ps[:, 0:512], wconv[0:C2, MW3:MW3 + K3 * C4], mx2, K3, 128, C4, 0, 512, C4)
            _conv_matmuls(nc, mc1_ps[:, 512:LM3], wconv[0:C2, MW3:MW3 + K3 * C4], mx2, K3, 128, C4, 512, LM3, C4)
            mc1 = wp.tile([32, LM3], F16, name="mc1")
            nc.scalar.activation(out=mc1, in_=mc1_ps, func=AF.Relu, bias=bias(SB_MB3B, 32))

            # peptide conv3 (tiles padded to 88 cols for the 4-strided views)
            pc0_ps = ps.tile([128, LP3], F32, name="pc0_ps", tag="ps")
            _conv_matmuls(nc, pc0_ps, wconv[0:C2, PW3:PW3 + K3 * C4], px2, K3, 0, 128, 0, LP3, C4)
            pc0 = wp.tile([128, 88], F16, name="pc0")
            nc.scalar.activation(out=pc0[:, 0:LP3], in_=pc0_ps, func=AF.Relu, bias=bias(SB_PB3A))
            pc1_ps = ps.tile([32, LP3], F32, name="pc1_ps", tag="ps")
            _conv_matmuls(nc, pc1_ps, wconv[0:C2, PW3:PW3 + K3 * C4], px2, K3, 128, C4, 0, LP3, C4)
            pc1 = wp.tile([32, 88], F16, name="pc1")
            nc.scalar.activation(out=pc1[:, 0:LP3], in_=pc1_ps, func=AF.Relu, bias=bias(SB_PB3B, 32))

            # ================= attention projections =================
            # ma0[c,m] c in 0:128
            ma0_ps = ps.tile([128, LM3], F32, name="ma0_ps", tag="ps")
            for lo, hi in ((0, 512), (512, LM3)):
                nc.tensor.matmul(ma0_ps[:, lo:hi], wattn[0:128, WMA_A:WMA_A + 128], mc0[:, lo:hi], start=True, stop=False)
                nc.tensor.matmul(ma0_ps[:, lo:hi], wattn[0:32, WMA_B:WMA_B + 128], mc1[:, lo:hi], start=False, stop=True)
            ma0 = wp.tile([128, LM3], F16, name="ma0")
            nc.scalar.activation(out=ma0, in_=ma0_ps, func=AF.Identity, bias=bias(SB_BMA_A))

            # ma1p: c in 128:160 replicated 4x vertically (stationary pre-replicated)
            ma1p_ps = ps.tile([128, LM3], F32, name="ma1p_ps", tag="ps")
            for lo, hi in ((0, 512), (512, LM3)):
                nc.tensor.matmul(ma1p_ps[:, lo:hi], wattn[0:128, WMA_A + 128:WMA_A + 256], mc0[:, lo:hi], start=True, stop=False)
                nc.tensor.matmul(ma1p_ps[:, lo:hi], wattn[0:32, WMA_B + 128:WMA_B + 256], mc1[:, lo:hi], start=False, stop=True)
            ma1p = wp.tile([128, LM3], F16, name="ma1p")
            nc.vector.tensor_scalar(out=ma1p, in0=ma1p_ps, scalar1=bias(SB_BMA_R4), scalar2=None, op0=ALU.add)

            # pa0[c,p] c in 0:128 (f32, used as per-partition bias)
            pa0_ps = ps.tile([128, LP3], F32, name="pa0_ps", tag="ps")
            nc.tensor.matmul(pa0_ps, wattn[0:128, WPA_A:WPA_A + 128], pc0[:, 0:LP3], start=True, stop=False)
            nc.tensor.matmul(pa0_ps, wattn[0:32, WPA_B:WPA_B + 128], pc1[:, 0:LP3], start=False, stop=True)
            pa0 = wp.tile([128, LP3], F32, name="pa0")
            nc.scalar.add(pa0, pa0_ps, bias(SB_BPA_A))

            # pa1p[32j+d, g] = pa[128+d, 4g+j]: 4 partition-offset matmuls with
            # 4-strided moving views of pc
            pa1p_ps = ps.tile([128, NP4], F32, name="pa1p_ps", tag="ps")
            pc0_g = pc0.rearrange("c (g f) -> c g f", f=4)
            pc1_g = pc1.rearrange("c (g f) -> c g f", f=4)
            pa1p = wp.tile([128, NP4], F32, name="pa1p")
            nc.vector.memset(pa1p, NEG)
            for j in range(4):
                ncol = NP4 if j == 0 else NP4 - 1
                nc.tensor.matmul(pa1p_ps[32 * j:32 * j + 32, 0:ncol],
                                 wattn[0:128, WPA_A + 128:WPA_A + 160],
                                 pc0_g[:, 0:ncol, j],
                                 start=True, stop=False, skip_group_check=True,
                                 tile_position=(0, 32 * j))
                nc.tensor.matmul(pa1p_ps[32 * j:32 * j + 32, 0:ncol],
                                 wattn[0:32, WPA_B + 128:WPA_B + 160],
                                 pc1_g[:, 0:ncol, j],
                                 start=False, stop=True, skip_group_check=True,
                                 tile_position=(0, 32 * j))
                nc.scalar.add(pa1p[32 * j:32 * j + 32, 0:ncol],
                              pa1p_ps[32 * j:32 * j + 32, 0:ncol], bias(SB_BPA_B, 32))

            # ================= 4D attention reductions =================
            hp0 = wp.tile([128, 88], F32, name="hp0")
            hp1p = wp.tile([128, NP4], F32, name="hp1p")
            hm0_ps = ps_hm.tile([128, LM3], F32, name="hm0_ps")
            hm1_ps = ps_hm.tile([32, LM3], F32, name="hm1_ps")
            zt = wp.tile([128, LM3], F16, name="zt")
            nc.vector.memset(zt, 0.0)

            id128 = wattn[0:128, ID128:ID128 + 128]
            idst = wattn[0:128, IDST:IDST + 32]

            jobs = [('p', p) for p in range(LP3)] + [('g', g) for g in range(NP4)]
            np_seen = ng_seen = 0
            for i, (kind, idx) in enumerate(jobs):
                if kind == 'p':
                    src, bias_ap, acc = ma0, pa0[:, idx:idx + 1], hp0[:, idx:idx + 1]
                else:
                    src, bias_ap, acc = ma1p, pa1p[:, idx:idx + 1], hp1p[:, idx:idx + 1]
                h = hpool.tile([128, LM3], F16, tag="h", name="h")
                if (i % SPLIT_MOD) == 0:
                    nc.scalar.activation(out=h, in_=src, func=AF.Relu,
                                         bias=bias_ap, accum_out=acc)
                else:
                    nc.vector.scalar_tensor_tensor(out=h, in0=src, scalar=bias_ap,
                                                   in1=zt, op0=ALU.add, op1=ALU.max,
                                                   accum_out=acc)
                if kind == 'p':
                    nc.tensor.matmul(hm0_ps[:, 0:512], id128, h[:, 0:512],
                                     start=(np_seen == 0), stop=(np_seen == LP3 - 1))
                    nc.tensor.matmul(hm0_ps[:, 512:LM3], id128, h[:, 512:LM3],
                                     start=(np_seen == 0), stop=(np_seen == LP3 - 1))
                    np_seen += 1
                else:
                    nc.tensor.matmul(hm1_ps[:, 0:512], idst, h[:, 0:512],
                                     start=(ng_seen == 0), stop=(ng_seen == NP4 - 1))
                    nc.tensor.matmul(hm1_ps[:, 512:LM3], idst, h[:, 512:LM3],
                                     start=(ng_seen == 0), stop=(ng_seen == NP4 - 1))
                    ng_seen += 1

            # unpack hp1p -> hp1[d, 4g+j] (DMA partition remap)
            hp1 = wp.tile([32, 88], F32, name="hp1")
            hp1_g = hp1.rearrange("d (g f) -> d g f", f=4)
            for j in range(4):
                ncol = NP4 if j == 0 else NP4 - 1
                nc.sync.dma_start(out=hp1_g[:, 0:ncol, j],
                                  in_=hp1p[32 * j:32 * j + 32, 0:ncol])
            hp0f = wp.tile([128, LP3], F16, name="hp0f")
            nc.scalar.copy(hp0f, hp0[:, 0:LP3])
            hp1f = wp.tile([32, LP3], F16, name="hp1f")
            nc.scalar.copy(hp1f, hp1[:, 0:LP3])

            # ================= peptide gate =================
            cl0_ps = ps.tile([128, LP3], F32, name="cl0_ps", tag="ps")
            nc.tensor.matmul(cl0_ps, wattn[0:128, WCA_A:WCA_A + 128], hp0f, start=True, stop=False)
            nc.tensor.matmul(cl0_ps, wattn[0:32, WCA_B:WCA_B + 128], hp1f, start=False, stop=True)
            catt0 = wp.tile([128, LP3], F16, name="catt0")
            nc.scalar.activation(out=catt0, in_=cl0_ps, func=AF.Sigmoid, bias=bias(SB_BA_A))
            cl1_ps = ps.tile([32, LP3], F32, name="cl1_ps", tag="ps")
            nc.tensor.matmul(cl1_ps, wattn[0:128, WCA_A + 128:WCA_A + 160], hp0f, start=True, stop=False)
            nc.tensor.matmul(cl1_ps, wattn[0:32, WCA_B + 128:WCA_B + 160], hp1f, start=False, stop=True)
            catt1 = wp.tile([32, LP3], F16, name="catt1")
            nc.scalar.activation(out=catt1, in_=cl1_ps, func=AF.Sigmoid, bias=bias(SB_BA_B, 32))

            pvf = wp.tile([128, 4], F16, name="pvf")   # cols: pv0, pv1, mv0, mv1
            pg0 = wp.tile([128, LP3], F16, name="pg0")
            nc.vector.scalar_tensor_tensor(out=pg0, in0=catt0, scalar=0.5, in1=pc0[:, 0:LP3],
                                           op0=ALU.add, op1=ALU.mult)
            pg1 = wp.tile([32, LP3], F16, name="pg1")
            nc.vector.scalar_tensor_tensor(out=pg1, in0=catt1, scalar=0.5, in1=pc1[0:32, 0:LP3],
                                           op0=ALU.add, op1=ALU.mult)
            with nc.allow_low_precision(reason="fp16 max-pool rounds values only"):
                nc.vector.tensor_reduce(out=pvf[:, 0:1], in_=pg0, op=ALU.max, axis=AX.X)
                nc.vector.tensor_reduce(out=pvf[0:32, 1:2], in_=pg1, op=ALU.max, axis=AX.X)

            # ================= MHC gate =================
            hm0f = wp.tile([128, LM3], F16, name="hm0f")
            nc.scalar.copy(hm0f, hm0_ps)
            hm1f = wp.tile([32, LM3], F16, name="hm1f")
            nc.vector.tensor_scalar(out=hm1f, in0=hm1_ps, scalar1=0.0, scalar2=None, op0=ALU.add)

            ml0_ps = ps.tile([128, LM3], F32, name="ml0_ps", tag="ps")
            for lo, hi in ((0, 512), (512, LM3)):
                nc.tensor.matmul(ml0_ps[:, lo:hi], wattn[0:128, WM2_A:WM2_A + 128], hm0f[:, lo:hi], start=True, stop=False)
                nc.tensor.matmul(ml0_ps[:, lo:hi], wattn[0:32, WM2_B:WM2_B + 128], hm1f[:, lo:hi], start=False, stop=True)
            matt0 = wp.tile([128, LM3], F16, name="matt0")
            nc.scalar.activation(out=matt0, in_=ml0_ps, func=AF.Sigmoid, bias=bias(SB_BA_A))
            ml1_ps = ps.tile([32, LM3], F32, name="ml1_ps", tag="ps")
            for lo, hi in ((0, 512), (512, LM3)):
                nc.tensor.matmul(ml1_ps[:, lo:hi], wattn[0:128, WM2_A + 128:WM2_A + 160], hm0f[:, lo:hi], start=True, stop=False)
                nc.tensor.matmul(ml1_ps[:, lo:hi], wattn[0:32, WM2_B + 128:WM2_B + 160], hm1f[:, lo:hi], start=False, stop=True)
            matt1 = wp.tile([32, LM3], F16, name="matt1")
            nc.scalar.activation(out=matt1, in_=ml1_ps, func=AF.Sigmoid, bias=bias(SB_BA_B, 32))

            mg0 = wp.tile([128, LM3], F16, name="mg0")
            nc.vector.scalar_tensor_tensor(out=mg0, in0=matt0, scalar=0.5, in1=mc0,
                                           op0=ALU.add, op1=ALU.mult)
            mg1 = wp.tile([32, LM3], F16, name="mg1")
            nc.vector.scalar_tensor_tensor(out=mg1, in0=matt1, scalar=0.5, in1=mc1,
                                           op0=ALU.add, op1=ALU.mult)
            with nc.allow_low_precision(reason="fp16 max-pool rounds values only"):
                nc.vector.tensor_reduce(out=pvf[:, 2:3], in_=mg0, op=ALU.max, axis=AX.X)
                nc.vector.tensor_reduce(out=pvf[0:32, 3:4], in_=mg1, op=ALU.max, axis=AX.X)

            # ================= FC head =================
            def lrelu(name, f_ps, bias_lo, ncols, out_dtype=F16):
                fb = wp.tile([128, ncols], F32, name=name + "_b")
                nc.vector.tensor_tensor(out=fb, in0=f_ps, in1=wsmall[:, bias_lo:bias_lo + ncols], op=ALU.add)
                fs = wp.tile([128, ncols], F32, name=name + "_s")
                nc.vector.tensor_scalar(out=fs, in0=fb, scalar1=0.01, scalar2=None, op0=ALU.mult)
                fo = wp.tile([128, ncols], out_dtype, name=name)
                nc.vector.tensor_tensor(out=fo, in0=fb, in1=fs, op=ALU.max)
                return fo

            f1_ps = ps.tile([128, 8], F32, name="f1_ps", tag="ps")
            for a in range(8):
                nc.tensor.matmul(f1_ps[:, a:a + 1], wfc[0:128, W1A + a * 128:W1A + a * 128 + 128],
                                 pvf[:, 0:1], start=True, stop=False)
                nc.tensor.matmul(f1_ps[:, a:a + 1], wfc[0:32, W1B + a * 128:W1B + a * 128 + 128],
                                 pvf[0:32, 1:2], start=False, stop=False)
                nc.tensor.matmul(f1_ps[:, a:a + 1], wfc[0:128, W1A + 1024 + a * 128:W1A + 1024 + a * 128 + 128],
                                 pvf[:, 2:3], start=False, stop=False)
                nc.tensor.matmul(f1_ps[:, a:a + 1], wfc[0:32, W1B + 1024 + a * 128:W1B + 1024 + a * 128 + 128],
                                 pvf[0:32, 3:4], start=False, stop=True)
            f1 = lrelu("f1", f1_ps, SB_B1, 8)

            f2_ps = ps.tile([128, 8], F32, name="f2_ps", tag="ps")
            for a in range(8):
                for jb in range(8):
                    nc.tensor.matmul(f2_ps[:, a:a + 1],
                                     wfc[0:128, W2C + jb * 1024 + a * 128:W2C + jb * 1024 + a * 128 + 128],
                                     f1[:, jb:jb + 1], start=(jb == 0), stop=(jb == 7))
            f2 = lrelu("f2", f2_ps, SB_B2, 8)

            f3_ps = ps.tile([128, 4], F32, name="f3_ps", tag="ps")
            for a in range(4):
                for jb in range(8):
                    nc.tensor.matmul(f3_ps[:, a:a + 1],
                                     wfc[0:128, W3C + jb * 512 + a * 128:W3C + jb * 512 + a * 128 + 128],
                                     f2[:, jb:jb + 1], start=(jb == 0), stop=(jb == 7))
            f3 = lrelu("f3", f3_ps, SB_B3, 4)

            o_ps = ps.tile([2, 1], F32, name="o_ps", tag="ps")
            for c in range(4):
                nc.tensor.matmul(o_ps, wfc[0:128, WOC + 2 * c:WOC + 2 * c + 2], f3[:, c:c + 1],
                                 start=(c == 0), stop=(c == 3))
            o_sb = wp.tile([2, 1], F32, name="o_sb")
            nc.vector.tensor_tensor(out=o_sb, in0=o_ps, in1=wsmall[0:2, SB_BO:SB_BO + 1], op=ALU.add)
            nc.sync.dma_start(out=out_e[:], in_=o_sb)

    _split_excess_waits(nc, max_waits=1)
    return nc


_PROGRAM = None


def _get_program():
    global _PROGRAM
    if _PROGRAM is None:
        _PROGRAM = _build_program()
    return _PROGRAM


def _prep_weights(inp):
    """Host-side packing shared by all cores."""
    f16 = np.float16
    f32 = lambda x: np.asarray(x, dtype=np.float32)

    def convw(w):  # [co, ci, k] -> [ci, k*co] fp16
        w = np.asarray(w, dtype=np.float32)
        ci = w.shape[1]
        return w.transpose(1, 2, 0).reshape(ci, -1).astype(f16)

    wconv = np.zeros((128, NCONV), f16)
    wconv[0:DIM, PW1:PW1 + K1 * CONV] = convw(inp['pw1'])
    wconv[0:CONV, PW2:PW2 + K2 * C2] = convw(inp['pw2'])
    wconv[0:C2, PW3:PW3 + K3 * C4] = convw(inp['pw3'])
    wconv[0:DIM, MW1:MW1 + K1 * CONV] = convw(inp['mw1'])
    wconv[0:CONV, MW2:MW2 + K2 * C2] = convw(inp['mw2'])
    wconv[0:C2, MW3:MW3 + K3 * C4] = convw(inp['mw3'])

    wpa, wma = f32(inp['Wpa']), f32(inp['Wma'])
    wca = f32(inp['Wa']) / float(LM3)
    wm2 = f32(inp['Wa']) / float(LP3)
    wattn = np.zeros((128, NATTN), f16)
    wattn[0:128, WPA_A:WPA_A + 160] = wpa[0:128].astype(f16)
    wattn[0:32, WPA_B:WPA_B + 160] = wpa[128:160].astype(f16)
    wattn[0:128, WMA_A:WMA_A + 128] = wma[0:128, 0:128].astype(f16)
    wattn[0:128, WMA_A + 128:WMA_A + 256] = np.tile(wma[0:128, 128:160], (1, 4)).astype(f16)
    wattn[0:32, WMA_B:WMA_B + 128] = wma[128:160, 0:128].astype(f16)
    wattn[0:32, WMA_B + 128:WMA_B + 256] = np.tile(wma[128:160, 128:160], (1, 4)).astype(f16)
    wattn[0:128, WCA_A:WCA_A + 160] = wca[0:128].astype(f16)
    wattn[0:32, WCA_B:WCA_B + 160] = wca[128:160].astype(f16)
    wattn[0:128, WM2_A:WM2_A + 160] = wm2[0:128].astype(f16)
    wattn[0:32, WM2_B:WM2_B + 160] = wm2[128:160].astype(f16)
    wattn[0:128, ID128:ID128 + 128] = np.eye(128, dtype=f16)
    wattn[0:128, IDST:IDST + 32] = np.tile(np.eye(32, dtype=f16), (4, 1))

    w1 = f32(inp['W1'])
    wfc = np.zeros((128, NFC), f16)
    wfc[0:128, W1A:W1A + 2048] = np.concatenate([w1[0:128], w1[160:288]], axis=1).astype(f16)
    wfc[0:32, W1B:W1B + 2048] = np.concatenate([w1[128:160], w1[288:320]], axis=1).astype(f16)

    def fcw(w, nblk):  # [I, J], I = nblk*128 -> [128, nblk*J]
        w = np.asarray(w, dtype=np.float32)
        i, j = w.shape
        return w.reshape(nblk, 128, j).transpose(1, 0, 2).reshape(128, nblk * j).astype(f16)

    wfc[0:128, W2C:W2C + 8192] = fcw(inp['W2'], 8)
    wfc[0:128, W3C:W3C + 4096] = fcw(inp['W3'], 8)
    wfc[0:128, WOC:WOC + 8] = fcw(inp['Wo'], 4)

    wsmall = np.zeros((128, NSMALL), np.float32)
    def bias2(col_a, col_b, b):
        b = f32(b)
        wsmall[0:128, col_a] = b[0:128]
        wsmall[0:32, col_b] = b[128:160]
    wsmall[0:CONV, SB_PB1] = f32(inp['pb1'])
    wsmall[0:C2, SB_PB2] = f32(inp['pb2'])
    bias2(SB_PB3A, SB_PB3B, inp['pb3'])
    wsmall[0:CONV, SB_MB1] = f32(inp['mb1'])
    wsmall[0:C2, SB_MB2] = f32(inp['mb2'])
    bias2(SB_MB3A, SB_MB3B, inp['mb3'])
    bias2(SB_BPA_A, SB_BPA_B, inp['bpa'])
    wsmall[0:128, SB_BMA_A] = f32(inp['bma'])[0:128]
    wsmall[0:128, SB_BMA_R4] = np.tile(f32(inp['bma'])[128:160], 4)
    bias2(SB_BA_A, SB_BA_B, inp['ba'])
    wsmall[0:128, SB_B1:SB_B1 + 8] = f32(inp['b1']).reshape(8, 128).T
    wsmall[0:128, SB_B2:SB_B2 + 8] = f32(inp['b2']).reshape(8, 128).T
    wsmall[0:128, SB_B3:SB_B3 + 4] = f32(inp['b3']).reshape(4, 128).T
    wsmall[0:2, SB_BO] = f32(inp['bo'])

    return {'wconv': wconv, 'wattn': wattn, 'wfc': wfc, 'wsmall': wsmall}


def _prep_core(inp, b):
    """Per-core embedding gather: [64, 1100] fp16."""
    pep = np.asarray(inp['peptide'])[b]
    mhc = np.asarray(inp['MHC'])[b]
    pe = np.asarray(inp['pep_emb'], np.float32)[pep].T   # [64, 100]
    me = np.asarray(inp['mhc_emb'], np.float32)[mhc].T   # [64, 1000]
    return np.concatenate([pe, me], axis=1).astype(np.float16)


def kernel(**inputs):
    nc = _get_program()
    shared = _prep_weights(inputs)
    in_maps = []
    for b in range(B):
        m = dict(shared)
        m['emb'] = _prep_core(inputs, b)
        in_maps.append(m)
    res = run_bass_kernel_spmd(nc, in_maps, core_ids=list(range(B)))
    return np.stack([np.asarray(res.results[i]['out']).reshape(2) for i in range(B)]).astype(np.float32)


# revision 14
# speedup vs baseline: 2.5763x; 1.2137x over previous
"""AttentionDTI forward pass on 8 Trainium2 NeuronCores (pure data parallel).

One batch element per core, weights replicated. All matmul operands are
16-bit (f32 PSUM accumulation): fp16 for conv/FC weights and activations,
bf16 for the attention tiles (the Activation engine runs ~1.3x slower on
fp16 than bf16, and the attention path tolerates bf16). Embedding lookup
is done host-side; weights arrive in packed DMAs issued from both the SP
and Activation HWDGE queues so transfers overlap the NEFF prologue.

The 4D additive-attention tensor h[b,p,m,c] = relu(pa + ma) is never
materialized: mean_m(h @ Wa) == mean_m(h) @ Wa, so only hp[c,p] = sum_m h
and hm[c,m] = sum_p h are accumulated on the fly. hm accumulates in PSUM
by streaming h tiles through the PE against a stationary identity; hp
comes from fused per-tile accumulators: the Scalar engine's
relu+bias+accum activation alternates 1:1 with the Vector engine's
scalar_tensor_tensor (relu via max-with-zeros + sum accum) — any DVE op
with an accum output runs at the 1x element rate, so the fused form is
optimal on both engines.

c-channels [128:160] run packed: ma rows replicated 4x vertically (via
host-replicated stationary columns, free) so each tile covers 4 peptide
positions; a 4-stacked [128,32] identity reduces them into hm1. The
peptide gate keeps its column axis in "j-major" order (jm(p) =
(p%4)*22 + p//4) end-to-end — max-pool over p is order-invariant — which
makes the packed hp1p contributions contiguous matmuls and avoids any
unpack DMAs.

Environment constraints discovered empirically (this axon terminal):
  - GPSIMD/Pool compute ops fail codegen; SWDGE DMA hangs: DMAs go
    through SP/Activation HWDGE only.
  - tensor_tensor_reduce fails walrus codegen ("ISA wrong length").
  - tensor_scalar's accum_out hijacks op1 as the reduce op (no fused
    two-op elementwise + sum) — scalar_tensor_tensor does fuse it.
  - walrus allows at most ONE semaphore wait per instruction:
    _split_excess_waits() rewrites the scheduled program.
"""
import sys

_BASS_ROOT = '/opt/trn_rl_repo'
if _BASS_ROOT not in sys.path:
    sys.path.insert(0, _BASS_ROOT)

import numpy as np

import concourse.bass as bass
import concourse.tile as tile
from concourse import mybir
from concourse.bass_utils import run_bass_kernel_spmd

F32 = mybir.dt.float32
F16 = mybir.dt.float16
BF16 = mybir.dt.bfloat16
ALU = mybir.AluOpType
AF = mybir.ActivationFunctionType
AX = mybir.AxisListType

B = 8
LP, LM, DIM, CONV = 100, 1000, 64, 40
C2, C4 = CONV * 2, CONV * 4          # 80, 160
K1, K2, K3 = 4, 6, 8
LP1, LP2, LP3 = 97, 92, 85           # peptide conv output lengths
LM1, LM2, LM3 = 997, 992, 985        # MHC conv output lengths
NP4 = 22                             # ceil(85/4) packed p-groups
JM = 4 * NP4                         # 88 j-major gate columns
MP = 992                             # LM3 padded for 4x-eligible DVE gate ops
NEG = -30000.0
SPLIT_MOD = 2                        # i % SPLIT_MOD == 0 -> scalar h tile

# ---- wboot column map (fp16 [128, 1280]): conv1+conv2 weights ----
PW1, MW1, PW2, MW2 = 0, 160, 320, 800
NBOOT = 1280
# ---- wc3 column map (fp16 [128, 2560]): conv3 weights ----
PW3, MW3 = 0, 1280
NC3 = 2560
# ---- wattn column map (fp16 [128, 1632]) ----
WPA_A, WPA_B = 0, 160        # [128,160], [32,160]
WMA_A, WMA_B = 320, 576      # [128,256], [32,256] (cols 128:256 = rep4 of Wma[:,128:160])
WCA_A, WCA_B = 832, 992      # Wa/LM3: [128,160], [32,160]
WM2_A, WM2_B = 1152, 1312    # Wa/LP3
ID128, IDST = 1472, 1600     # bf16 identities: [128,128], [128,32]
WCB0, WCB1 = 1632, 2144      # j-lifted Wa[128:160]/LM3: 4x[128,128], 4x[128,32]
NATTN = 2272
# ---- wfc column map (fp16 [128, 16392]) ----
W1A, W1B = 0, 2048           # [128, 2048], [32, 2048]
W2C, W3C, WOC = 4096, 12288, 16384
NFC = 16392
# ---- wsmall column map (f32 [128, 35]) ----
SB_PB1, SB_PB2, SB_PB3A, SB_PB3B = 0, 1, 2, 3
SB_MB1, SB_MB2, SB_MB3A, SB_MB3B = 4, 5, 6, 7
SB_BPA_A, SB_BPA_B = 8, 9
SB_BMA_A, SB_BMA_R4 = 10, 11
SB_BA_A, SB_BA_B = 12, 13
SB_B1, SB_B2, SB_B3, SB_BO = 14, 22, 30, 34
NSMALL = 35


def _jm(p):
    return (p % 4) * NP4 + (p // 4)


_ctr = [0]


def _split_excess_waits(nc, max_waits=1):
    n_split = 0
    for f in nc.m.functions:
        for b in f.blocks:
            insts = list(b.instructions)
            out = []
            changed = False
            for inst in insts:
                si = inst.sync_info
                waits = list(si.on_wait) if (si is not None and si.on_wait) else []
                if len(waits) > max_waits:
                    changed = True
                    n_split += 1
                    keep = max(1, max_waits)
                    head, tail = waits[:-keep], waits[-keep:]
                    for i in range(0, len(head), keep):
                        chunk = head[i:i + keep]
                        nop = mybir.InstEventSemaphore(
                            name=f"ant-wait-split-{_ctr[0]}", ins=[], outs=[])
                        _ctr[0] += 1
                        nop.engine = inst.engine
                        nop.sync_info = mybir.SyncInfo(on_wait=chunk, on_update=[])
                        nc.register_instruction(nop)
                        out.append(nop)
                    upd = list(si.on_update) if si.on_update else []
                    inst.sync_info = mybir.SyncInfo(on_wait=tail, on_update=upd)
                out.append(inst)
            if changed:
                b.instructions = out
    return n_split


def _conv_matmuls(nc, psum, wtile, x, k_taps, co_lo, co_hi, m_lo, m_hi, cout_stride):
    """Valid 1-D conv as k shifted matmuls accumulated into `psum`."""
    for k in range(k_taps):
        nc.tensor.matmul(
            psum,
            wtile[:, k * cout_stride + co_lo: k * cout_stride + co_hi],
            x[:, m_lo + k: m_hi + k],
            start=(k == 0), stop=(k == k_taps - 1))


def _build_program():
    nc = bass.Bass("TRN2", target_bir_lowering=False, debug=False)

    emb_e = nc.declare_dram_parameter("emb", [DIM, LP + LM], F16, isOutput=False)
    wsmall_e = nc.declare_dram_parameter("wsmall", [128, NSMALL], F32, isOutput=False)
    wboot_e = nc.declare_dram_parameter("wboot", [128, NBOOT], F16, isOutput=False)
    wc3_e = nc.declare_dram_parameter("wc3", [128, NC3], F16, isOutput=False)
    wattn_e = nc.declare_dram_parameter("wattn", [128, NATTN], F16, isOutput=False)
    wfc_e = nc.declare_dram_parameter("wfc", [128, NFC], F16, isOutput=False)
    out_e = nc.declare_dram_parameter("out", [2, 1], F32, isOutput=True)

    with tile.TileContext(nc) as tc:
        with tc.tile_pool(name="consts", bufs=1) as cp, \
             tc.tile_pool(name="work", bufs=1) as wp, \
             tc.tile_pool(name="hpool", bufs=8) as hpool, \
             tc.tile_pool(name="ps_hm", bufs=1, space="PSUM") as ps_hm, \
             tc.tile_pool(name="ps_work", bufs=2, space="PSUM") as ps:

            # loads: small/boot/emb from SP; conv3+attn from Activation HWDGE;
            # the big FC pack last on SP.
            wsmall = cp.tile([128, NSMALL], F32, name="wsmall")
            nc.sync.dma_start(out=wsmall, in_=wsmall_e[:])
            wboot = cp.tile([128, NBOOT], F16, name="wboot")
            nc.sync.dma_start(out=wboot, in_=wboot_e[:])
            emb = cp.tile([DIM, LP + LM], F16, name="emb")
            nc.sync.dma_start(out=emb, in_=emb_e[:])
            wc3 = cp.tile([128, NC3], F16, name="wc3")
            nc.scalar.dma_start(out=wc3, in_=wc3_e[:])
            wattn = cp.tile([128, NATTN], F16, name="wattn")
            nc.scalar.dma_start(out=wattn, in_=wattn_e[:])
            wfc = cp.tile([128, NFC], F16, name="wfc")
            nc.sync.dma_start(out=wfc, in_=wfc_e[:])

            pe = emb[:, 0:LP]
            me = emb[:, LP:LP + LM]
            bias = lambda col, rows=128: wsmall[0:rows, col:col + 1]

            # early zero/NEG fills on the (idle) Vector engine
            zt = wp.tile([128, LM3], BF16, name="zt")
            nc.vector.memset(zt, 0.0)
            hp0 = wp.tile([128, JM], F32, name="hp0")
            nc.vector.memset(hp0, 0.0)
            pa1p = wp.tile([128, NP4], F32, name="pa1p")
            nc.vector.memset(pa1p, NEG)
            pc0jm = wp.tile([128, JM], F16, name="pc0jm")
            nc.vector.memset(pc0jm, 0.0)
            pc1jm = wp.tile([32, JM], F16, name="pc1jm")
            nc.vector.memset(pc1jm, 0.0)
            hm0f = wp.tile([128, MP], F16, name="hm0f")
            nc.vector.memset(hm0f[:, LM3:MP], 0.0)
            hm1f = wp.tile([32, MP], F16, name="hm1f")
            nc.vector.memset(hm1f[:, LM3:MP], 0.0)
            mc0 = wp.tile([128, MP], F16, name="mc0")
            nc.vector.memset(mc0[:, LM3:MP], 0.0)
            mc1 = wp.tile([32, MP], F16, name="mc1")
            nc.vector.memset(mc1[:, LM3:MP], 0.0)

            # ================= conv stacks (fp16, f32 psum) =================
            # MHC conv1: [64,1000] -> [40,997]; relu chunked so conv2 starts early
            mx1_ps = ps.tile([CONV, LM1], F32, name="mx1_ps", tag="ps")
            _conv_matmuls(nc, mx1_ps[:, 0:512], wboot[0:DIM, MW1:MW1 + K1 * CONV], me, K1, 0, CONV, 0, 512, CONV)
            _conv_matmuls(nc, mx1_ps[:, 512:LM1], wboot[0:DIM, MW1:MW1 + K1 * CONV], me, K1, 0, CONV, 512, LM1, CONV)
            # peptide conv1 fills the PE while relu1 runs
            px1_ps = ps.tile([CONV, LP1], F32, name="px1_ps", tag="ps")
            _conv_matmuls(nc, px1_ps, wboot[0:DIM, PW1:PW1 + K1 * CONV], pe, K1, 0, CONV, 0, LP1, CONV)
            mx1 = wp.tile([CONV, LM1], F16, name="mx1")
            nc.scalar.activation(out=mx1[:, 0:520], in_=mx1_ps[:, 0:520], func=AF.Relu, bias=bias(SB_MB1, CONV))
            nc.scalar.activation(out=mx1[:, 520:LM1], in_=mx1_ps[:, 520:LM1], func=AF.Relu, bias=bias(SB_MB1, CONV))
            px1 = wp.tile([CONV, LP1], F16, name="px1")
            nc.scalar.activation(out=px1, in_=px1_ps, func=AF.Relu, bias=bias(SB_PB1, CONV))

            # MHC conv2 -> [80, 992]
            mx2_ps = ps.tile([C2, LM2], F32, name="mx2_ps", tag="ps")
            _conv_matmuls(nc, mx2_ps[:, 0:512], wboot[0:CONV, MW2:MW2 + K2 * C2], mx1, K2, 0, C2, 0, 512, C2)
            _conv_matmuls(nc, mx2_ps[:, 512:LM2], wboot[0:CONV, MW2:MW2 + K2 * C2], mx1, K2, 0, C2, 512, LM2, C2)
            px2_ps = ps.tile([C2, LP2], F32, name="px2_ps", tag="ps")
            _conv_matmuls(nc, px2_ps, wboot[0:CONV, PW2:PW2 + K2 * C2], px1, K2, 0, C2, 0, LP2, C2)
            mx2 = wp.tile([C2, LM2], F16, name="mx2")
            nc.scalar.activation(out=mx2[:, 0:520], in_=mx2_ps[:, 0:520], func=AF.Relu, bias=bias(SB_MB2, C2))
            nc.scalar.activation(out=mx2[:, 520:LM2], in_=mx2_ps[:, 520:LM2], func=AF.Relu, bias=bias(SB_MB2, C2))
            px2 = wp.tile([C2, LP2], F16, name="px2")
            nc.scalar.activation(out=px2, in_=px2_ps, func=AF.Relu, bias=bias(SB_PB2, C2))

            # MHC conv3 -> [160, 985] as [128,985]+[32,985] (into MP-padded tiles)
            mc0_ps = ps.tile([128, LM3], F32, name="mc0_ps", tag="ps")
            _conv_matmuls(nc, mc0_ps[:, 0:512], wc3[0:C2, MW3:MW3 + K3 * C4], mx2, K3, 0, 128, 0, 512, C4)
            _conv_matmuls(nc, mc0_ps[:, 512:LM3], wc3[0:C2, MW3:MW3 + K3 * C4], mx2, K3, 0, 128, 512, LM3, C4)
            nc.scalar.activation(out=mc0[:, 0:512], in_=mc0_ps[:, 0:512], func=AF.Relu, bias=bias(SB_MB3A))
            nc.scalar.activation(out=mc0[:, 512:LM3], in_=mc0_ps[:, 512:LM3], func=AF.Relu, bias=bias(SB_MB3A))
            mc1_ps = ps.tile([32, LM3], F32, name="mc1_ps", tag="ps")
            _conv_matmuls(nc, mc1_ps[:, 0:512], wc3[0:C2, MW3:MW3 + K3 * C4], mx2, K3, 128, C4, 0, 512, C4)
            _conv_matmuls(nc, mc1_ps[:, 512:LM3], wc3[0:C2, MW3:MW3 + K3 * C4], mx2, K3, 128, C4, 512, LM3, C4)
            nc.scalar.activation(out=mc1[:, 0:512], in_=mc1_ps[:, 0:512], func=AF.Relu, bias=bias(SB_MB3B, 32))
            nc.scalar.activation(out=mc1[:, 512:LM3], in_=mc1_ps[:, 512:LM3], func=AF.Relu, bias=bias(SB_MB3B, 32))

            # peptide conv3 (tiles padded to 88 cols for the 4-strided views)
            pc0_ps = ps.tile([128, LP3], F32, name="pc0_ps", tag="ps")
            _conv_matmuls(nc, pc0_ps, wc3[0:C2, PW3:PW3 + K3 * C4], px2, K3, 0, 128, 0, LP3, C4)
            pc0 = wp.tile([128, 88], F16, name="pc0")
            nc.scalar.activation(out=pc0[:, 0:LP3], in_=pc0_ps, func=AF.Relu, bias=bias(SB_PB3A))
            pc1_ps = ps.tile([32, LP3], F32, name="pc1_ps", tag="ps")
            _conv_matmuls(nc, pc1_ps, wc3[0:C2, PW3:PW3 + K3 * C4], px2, K3, 128, C4, 0, LP3, C4)
            pc1 = wp.tile([32, 88], F16, name="pc1")
            nc.scalar.activation(out=pc1[:, 0:LP3], in_=pc1_ps, func=AF.Relu, bias=bias(SB_PB3B, 32))

            # j-major copies of pc for the gate (vector, strided reads)
            pc0_g = pc0.rearrange("c (g f) -> c g f", f=4)
            pc1_g = pc1.rearrange("c (g f) -> c g f", f=4)
            for j in range(4):
                ncol = NP4 if j == 0 else NP4 - 1
                nc.vector.tensor_scalar(out=pc0jm[:, j * NP4:j * NP4 + ncol],
                                        in0=pc0_g[:, 0:ncol, j], scalar1=0.0,
                                        scalar2=None, op0=ALU.add)
                nc.vector.tensor_scalar(out=pc1jm[:, j * NP4:j * NP4 + ncol],
                                        in0=pc1_g[:, 0:ncol, j], scalar1=0.0,
                                        scalar2=None, op0=ALU.add)

            # ================= attention projections =================
            # ma0[c,m] c in 0:128 (bf16 for the Activation-engine h producer)
            ma0_ps = ps.tile([128, LM3], F32, name="ma0_ps", tag="ps")
            for lo, hi in ((0, 512), (512, LM3)):
                nc.tensor.matmul(ma0_ps[:, lo:hi], wattn[0:128, WMA_A:WMA_A + 128], mc0[:, lo:hi], start=True, stop=False)
                nc.tensor.matmul(ma0_ps[:, lo:hi], wattn[0:32, WMA_B:WMA_B + 128], mc1[:, lo:hi], start=False, stop=True)
            ma0 = wp.tile([128, LM3], BF16, name="ma0")
            nc.scalar.activation(out=ma0, in_=ma0_ps, func=AF.Identity, bias=bias(SB_BMA_A))

            # ma1p: c in 128:160 replicated 4x vertically (stationary pre-replicated)
            ma1p_ps = ps.tile([128, LM3], F32, name="ma1p_ps", tag="ps")
            for lo, hi in ((0, 512), (512, LM3)):
                nc.tensor.matmul(ma1p_ps[:, lo:hi], wattn[0:128, WMA_A + 128:WMA_A + 256], mc0[:, lo:hi], start=True, stop=False)
                nc.tensor.matmul(ma1p_ps[:, lo:hi], wattn[0:32, WMA_B + 128:WMA_B + 256], mc1[:, lo:hi], start=False, stop=True)
            ma1p = wp.tile([128, LM3], BF16, name="ma1p")
            nc.vector.tensor_scalar(out=ma1p, in0=ma1p_ps, scalar1=bias(SB_BMA_R4), scalar2=None, op0=ALU.add)

            # pa0[c,p] c in 0:128 (f32, used as per-partition bias)
            pa0_ps = ps.tile([128, LP3], F32, name="pa0_ps", tag="ps")
            nc.tensor.matmul(pa0_ps, wattn[0:128, WPA_A:WPA_A + 128], pc0[:, 0:LP3], start=True, stop=False)
            nc.tensor.matmul(pa0_ps, wattn[0:32, WPA_B:WPA_B + 128], pc1[0:32, 0:LP3], start=False, stop=True)
            pa0 = wp.tile([128, LP3], F32, name="pa0")
            nc.scalar.add(pa0, pa0_ps, bias(SB_BPA_A))

            # pa1p[32j+d, g] = pa[128+d, 4g+j]: partition-offset matmuls over
            # 4-strided moving views of pc
            pa1p_ps = ps.tile([128, NP4], F32, name="pa1p_ps", tag="ps")
            for j in range(4):
                ncol = NP4 if j == 0 else NP4 - 1
                nc.tensor.matmul(pa1p_ps[32 * j:32 * j + 32, 0:ncol],
                                 wattn[0:128, WPA_A + 128:WPA_A + 160],
                                 pc0_g[:, 0:ncol, j],
                                 start=True, stop=False, skip_group_check=True,
                                 tile_position=(0, 32 * j))
                nc.tensor.matmul(pa1p_ps[32 * j:32 * j + 32, 0:ncol],
                                 wattn[0:32, WPA_B + 128:WPA_B + 160],
                                 pc1_g[:, 0:ncol, j],
                                 start=False, stop=True, skip_group_check=True,
                                 tile_position=(0, 32 * j))
                nc.scalar.add(pa1p[32 * j:32 * j + 32, 0:ncol],
                              pa1p_ps[32 * j:32 * j + 32, 0:ncol], bias(SB_BPA_B, 32))

            # ================= 4D attention reductions =================
            # hp0 columns are written in j-major order (gate is order-free)
            hp1p = wp.tile([128, NP4], F32, name="hp1p")
            hm0_ps = ps_hm.tile([128, LM3], F32, name="hm0_ps")
            hm1_ps = ps_hm.tile([32, LM3], F32, name="hm1_ps")

            # identities stored as bf16 bit patterns in the fp16 pack; bitcast
            # the views so they pair with the bf16 h tiles
            id128 = wattn[0:128, ID128:ID128 + 128].bitcast(BF16)
            idst = wattn[0:128, IDST:IDST + 32].bitcast(BF16)

            def h_tile(i, src, bias_ap, acc):
                h = hpool.tile([128, LM3], BF16, tag="h", name="h")
                if (i % SPLIT_MOD) == 0:
                    nc.scalar.activation(out=h, in_=src, func=AF.Relu,
                                         bias=bias_ap, accum_out=acc)
                else:
                    nc.vector.scalar_tensor_tensor(out=h, in0=src, scalar=bias_ap,
                                                   in1=zt, op0=ALU.add, op1=ALU.max,
                                                   accum_out=acc)
                return h

            for p in range(LP3):
                h = h_tile(p, ma0, pa0[:, p:p + 1], hp0[:, _jm(p):_jm(p) + 1])
                nc.tensor.matmul(hm0_ps[:, 0:512], id128, h[:, 0:512],
                                 start=(p == 0), stop=(p == LP3 - 1))
                nc.tensor.matmul(hm0_ps[:, 512:LM3], id128, h[:, 512:LM3],
                                 start=(p == 0), stop=(p == LP3 - 1))

            # p-side gate head start: hp0 is complete before the packed loop
            hp0f = wp.tile([128, JM], F16, name="hp0f")
            nc.scalar.copy(hp0f, hp0)
            cl0_ps = ps.tile([128, JM], F32, name="cl0_ps", tag="ps")
            nc.tensor.matmul(cl0_ps, wattn[0:128, WCA_A:WCA_A + 128], hp0f,
                             start=True, stop=False, skip_group_check=True)
            cl1_ps = ps.tile([32, JM], F32, name="cl1_ps", tag="ps")
            nc.tensor.matmul(cl1_ps, wattn[0:128, WCA_A + 128:WCA_A + 160], hp0f,
                             start=True, stop=False, skip_group_check=True)

            for g in range(NP4):
                h = h_tile(LP3 + g, ma1p, pa1p[:, g:g + 1], hp1p[:, g:g + 1])
                nc.tensor.matmul(hm1_ps[:, 0:512], idst, h[:, 0:512],
                                 start=(g == 0), stop=(g == NP4 - 1))
                nc.tensor.matmul(hm1_ps[:, 512:LM3], idst, h[:, 512:LM3],
                                 start=(g == 0), stop=(g == NP4 - 1))

            # ================= peptide gate (j-major) =================
            hp1pf = wp.tile([128, NP4], F16, name="hp1pf")
            nc.scalar.copy(hp1pf, hp1p)
            for j in range(4):
                nc.tensor.matmul(cl0_ps[:, j * NP4:(j + 1) * NP4],
                                 wattn[0:128, WCB0 + j * 128:WCB0 + (j + 1) * 128],
                                 hp1pf, start=False, stop=(j == 3), skip_group_check=True)
                nc.tensor.matmul(cl1_ps[:, j * NP4:(j + 1) * NP4],
                                 wattn[0:128, WCB1 + j * 32:WCB1 + (j + 1) * 32],
                                 hp1pf, start=False, stop=(j == 3), skip_group_check=True)
            catt0 = wp.tile([128, JM], F16, name="catt0")
            nc.scalar.activation(out=catt0, in_=cl0_ps, func=AF.Sigmoid, bias=bias(SB_BA_A))
            catt1 = wp.tile([32, JM], F16, name="catt1")
            nc.scalar.activation(out=catt1, in_=cl1_ps, func=AF.Sigmoid, bias=bias(SB_BA_B, 32))

            pvf = wp.tile([128, 4], F16, name="pvf")   # cols: pv0, pv1, mv0, mv1
            pg0 = wp.tile([128, JM], F16, name="pg0")
            nc.vector.scalar_tensor_tensor(out=pg0, in0=catt0, scalar=0.5, in1=pc0jm,
                                           op0=ALU.add, op1=ALU.mult)
            pg1 = wp.tile([32, JM], F16, name="pg1")
            nc.vector.scalar_tensor_tensor(out=pg1, in0=catt1, scalar=0.5, in1=pc1jm,
                                           op0=ALU.add, op1=ALU.mult)
            with nc.allow_low_precision(reason="fp16 max-pool rounds values only"):
                nc.vector.tensor_reduce(out=pvf[:, 0:1], in_=pg0, op=ALU.max, axis=AX.X)
                nc.vector.tensor_reduce(out=pvf[0:32, 1:2], in_=pg1, op=ALU.max, axis=AX.X)

            # ================= MHC gate (chunk-pipelined) =================
            for lo, hi in ((0, 512), (512, LM3)):
                nc.scalar.activation(out=hm0f[:, lo:hi], in_=hm0_ps[:, lo:hi], func=AF.Copy)
                nc.vector.tensor_scalar(out=hm1f[:, lo:hi], in0=hm1_ps[:, lo:hi],
                                        scalar1=0.0, scalar2=None, op0=ALU.add)
            ml0_ps = ps.tile([128, MP], F32, name="ml0_ps", tag="ps")
            ml1_ps = ps.tile([32, MP], F32, name="ml1_ps", tag="ps")
            matt0 = wp.tile([128, MP], F16, name="matt0")
            matt1 = wp.tile([32, MP], F16, name="matt1")
            for lo, hi in ((0, 512), (512, MP)):
                nc.tensor.matmul(ml0_ps[:, lo:hi], wattn[0:128, WM2_A:WM2_A + 128], hm0f[:, lo:hi], start=True, stop=False)
                nc.tensor.matmul(ml0_ps[:, lo:hi], wattn[0:32, WM2_B:WM2_B + 128], hm1f[:, lo:hi], start=False, stop=True)
                nc.scalar.activation(out=matt0[:, lo:hi], in_=ml0_ps[:, lo:hi], func=AF.Sigmoid, bias=bias(SB_BA_A))
                nc.tensor.matmul(ml1_ps[:, lo:hi], wattn[0:128, WM2_A + 128:WM2_A + 160], hm0f[:, lo:hi], start=True, stop=False)
                nc.tensor.matmul(ml1_ps[:, lo:hi], wattn[0:32, WM2_B + 128:WM2_B + 160], hm1f[:, lo:hi], start=False, stop=True)
                nc.scalar.activation(out=matt1[:, lo:hi], in_=ml1_ps[:, lo:hi], func=AF.Sigmoid, bias=bias(SB_BA_B, 32))

            mg0 = wp.tile([128, MP], F16, name="mg0")
            nc.vector.scalar_tensor_tensor(out=mg0, in0=matt0, scalar=0.5, in1=mc0,
                                           op0=ALU.add, op1=ALU.mult)
            mg1 = wp.tile([32, MP], F16, name="mg1")
            nc.vector.scalar_tensor_tensor(out=mg1, in0=matt1, scalar=0.5, in1=mc1,
                                           op0=ALU.add, op1=ALU.mult)
            with nc.allow_low_precision(reason="fp16 max-pool rounds values only"):
                nc.vector.tensor_reduce(out=pvf[:, 2:3], in_=mg0, op=ALU.max, axis=AX.X)
                nc.vector.tensor_reduce(out=pvf[0:32, 3:4], in_=mg1, op=ALU.max, axis=AX.X)

            # ================= FC head =================
            def lrelu(name, f_ps, bias_lo, ncols):
                fb = wp.tile([128, ncols], F32, name=name + "_b")
                nc.vector.tensor_tensor(out=fb, in0=f_ps, in1=wsmall[:, bias_lo:bias_lo + ncols], op=ALU.add)
                fs = wp.tile([128, ncols], F32, name=name + "_s")
                nc.vector.tensor_scalar(out=fs, in0=fb, scalar1=0.01, scalar2=None, op0=ALU.mult)
                fo = wp.tile([128, ncols], F16, name=name)
                nc.vector.tensor_tensor(out=fo, in0=fb, in1=fs, op=ALU.max)
                return fo

            # f1: per-column accumulation groups (one 2KB region holds all
            # columns, so groups must not interleave); within a column the
            # two 128-row stationaries go first to reduce PE config flips
            f1_ps = ps.tile([128, 8], F32, name="f1_ps", tag="ps")
            for a in range(8):
                nc.tensor.matmul(f1_ps[:, a:a + 1], wfc[0:128, W1A + a * 128:W1A + a * 128 + 128],
                                 pvf[:, 0:1], start=True, stop=False)
                nc.tensor.matmul(f1_ps[:, a:a + 1], wfc[0:128, W1A + 1024 + a * 128:W1A + 1024 + a * 128 + 128],
                                 pvf[:, 2:3], start=False, stop=False)
                nc.tensor.matmul(f1_ps[:, a:a + 1], wfc[0:32, W1B + a * 128:W1B + a * 128 + 128],
                                 pvf[0:32, 1:2], start=False, stop=False)
                nc.tensor.matmul(f1_ps[:, a:a + 1], wfc[0:32, W1B + 1024 + a * 128:W1B + 1024 + a * 128 + 128],
                                 pvf[0:32, 3:4], start=False, stop=True)
            f1 = lrelu("f1", f1_ps, SB_B1, 8)

            f2_ps = ps.tile([128, 8], F32, name="f2_ps", tag="ps")
            for a in range(8):
                for jb in range(8):
                    nc.tensor.matmul(f2_ps[:, a:a + 1],
                                     wfc[0:128, W2C + jb * 1024 + a * 128:W2C + jb * 1024 + a * 128 + 128],
                                     f1[:, jb:jb + 1], start=(jb == 0), stop=(jb == 7))
            f2 = lrelu("f2", f2_ps, SB_B2, 8)

            f3_ps = ps.tile([128, 4], F32, name="f3_ps", tag="ps")
            for a in range(4):
                for jb in range(8):
                    nc.tensor.matmul(f3_ps[:, a:a + 1],
                                     wfc[0:128, W3C + jb * 512 + a * 128:W3C + jb * 512 + a * 128 + 128],
                                     f2[:, jb:jb + 1], start=(jb == 0), stop=(jb == 7))
            f3 = lrelu("f3", f3_ps, SB_B3, 4)

            o_ps = ps.tile([2, 1], F32, name="o_ps", tag="ps")
            for c in range(4):
                nc.tensor.matmul(o_ps, wfc[0:128, WOC + 2 * c:WOC + 2 * c + 2], f3[:, c:c + 1],
                                 start=(c == 0), stop=(c == 3))
            o_sb = wp.tile([2, 1], F32, name="o_sb")
            nc.vector.tensor_tensor(out=o_sb, in0=o_ps, in1=wsmall[0:2, SB_BO:SB_BO + 1], op=ALU.add)
            nc.sync.dma_start(out=out_e[:], in_=o_sb)

    _split_excess_waits(nc, max_waits=1)
    return nc


_PROGRAM = None


def _get_program():
    global _PROGRAM
    if _PROGRAM is None:
        _PROGRAM = _build_program()
    return _PROGRAM


def _prep_weights(inp):
    """Host-side packing shared by all cores."""
    f16 = np.float16
    f32 = lambda x: np.asarray(x, dtype=np.float32)

    def convw(w):  # [co, ci, k] -> [ci, k*co] fp16
        w = np.asarray(w, dtype=np.float32)
        ci = w.shape[1]
        return w.transpose(1, 2, 0).reshape(ci, -1).astype(f16)

    wboot = np.zeros((128, NBOOT), f16)
    wboot[0:DIM, PW1:PW1 + K1 * CONV] = convw(inp['pw1'])
    wboot[0:DIM, MW1:MW1 + K1 * CONV] = convw(inp['mw1'])
    wboot[0:CONV, PW2:PW2 + K2 * C2] = convw(inp['pw2'])
    wboot[0:CONV, MW2:MW2 + K2 * C2] = convw(inp['mw2'])

    wc3 = np.zeros((128, NC3), f16)
    wc3[0:C2, PW3:PW3 + K3 * C4] = convw(inp['pw3'])
    wc3[0:C2, MW3:MW3 + K3 * C4] = convw(inp['mw3'])

    wpa, wma = f32(inp['Wpa']), f32(inp['Wma'])
    wca = f32(inp['Wa']) / float(LM3)
    wm2 = f32(inp['Wa']) / float(LP3)
    wattn = np.zeros((128, NATTN), f16)
    wattn[0:128, WPA_A:WPA_A + 160] = wpa[0:128].astype(f16)
    wattn[0:32, WPA_B:WPA_B + 160] = wpa[128:160].astype(f16)
    wattn[0:128, WMA_A:WMA_A + 128] = wma[0:128, 0:128].astype(f16)
    wattn[0:128, WMA_A + 128:WMA_A + 256] = np.tile(wma[0:128, 128:160], (1, 4)).astype(f16)
    wattn[0:32, WMA_B:WMA_B + 128] = wma[128:160, 0:128].astype(f16)
    wattn[0:32, WMA_B + 128:WMA_B + 256] = np.tile(wma[128:160, 128:160], (1, 4)).astype(f16)
    wattn[0:128, WCA_A:WCA_A + 160] = wca[0:128].astype(f16)
    wattn[0:32, WCA_B:WCA_B + 160] = wca[128:160].astype(f16)
    wattn[0:128, WM2_A:WM2_A + 160] = wm2[0:128].astype(f16)
    wattn[0:32, WM2_B:WM2_B + 160] = wm2[128:160].astype(f16)
    bf = np.asarray(np.eye(128), dtype=np.float32)
    # identities stored as bf16 bit patterns inside the fp16 pack: keep them
    # as separate dtype via bitcast-compatible fill
    import ml_dtypes
    id128 = np.eye(128, dtype=ml_dtypes.bfloat16)
    idst = np.tile(np.eye(32, dtype=ml_dtypes.bfloat16), (4, 1))
    wattn[0:128, ID128:ID128 + 128] = id128.view(np.uint16).view(f16)
    wattn[0:128, IDST:IDST + 32] = idst.view(np.uint16).view(f16)
    for j in range(4):
        wattn[32 * j:32 * j + 32, WCB0 + j * 128:WCB0 + (j + 1) * 128] = wca[128:160, 0:128].astype(f16)
        wattn[32 * j:32 * j + 32, WCB1 + j * 32:WCB1 + (j + 1) * 32] = wca[128:160, 128:160].astype(f16)

    w1 = f32(inp['W1'])
    wfc = np.zeros((128, NFC), f16)
    wfc[0:128, W1A:W1A + 2048] = np.concatenate([w1[0:128], w1[160:288]], axis=1).astype(f16)
    wfc[0:32, W1B:W1B + 2048] = np.concatenate([w1[128:160], w1[288:320]], axis=1).astype(f16)

    def fcw(w, nblk):  # [I, J], I = nblk*128 -> [128, nblk*J]
        w = np.asarray(w, dtype=np.float32)
        i, j = w.shape
        return w.reshape(nblk, 128, j).transpose(1, 0, 2).reshape(128, nblk * j).astype(f16)

    wfc[0:128, W2C:W2C + 8192] = fcw(inp['W2'], 8)
    wfc[0:128, W3C:W3C + 4096] = fcw(inp['W3'], 8)
    wfc[0:128, WOC:WOC + 8] = fcw(inp['Wo'], 4)

    wsmall = np.zeros((128, NSMALL), np.float32)
    def bias2(col_a, col_b, b):
        b = f32(b)
        wsmall[0:128, col_a] = b[0:128]
        wsmall[0:32, col_b] = b[128:160]
    wsmall[0:CONV, SB_PB1] = f32(inp['pb1'])
    wsmall[0:C2, SB_PB2] = f32(inp['pb2'])
    bias2(SB_PB3A, SB_PB3B, inp['pb3'])
    wsmall[0:CONV, SB_MB1] = f32(inp['mb1'])
    wsmall[0:C2, SB_MB2] = f32(inp['mb2'])
    bias2(SB_MB3A, SB_MB3B, inp['mb3'])
    bias2(SB_BPA_A, SB_BPA_B, inp['bpa'])
    wsmall[0:128, SB_BMA_A] = f32(inp['bma'])[0:128]
    wsmall[0:128, SB_BMA_R4] = np.tile(f32(inp['bma'])[128:160], 4)
    bias2(SB_BA_A, SB_BA_B, inp['ba'])
    wsmall[0:128, SB_B1:SB_B1 + 8] = f32(inp['b1']).reshape(8, 128).T
    wsmall[0:128, SB_B2:SB_B2 + 8] = f32(inp['b2']).reshape(8, 128).T
    wsmall[0:128, SB_B3:SB_B3 + 4] = f32(inp['b3']).reshape(4, 128).T
    wsmall[0:2, SB_BO] = f32(inp['bo'])

    return {'wboot': wboot, 'wc3': wc3, 'wattn': wattn, 'wfc': wfc, 'wsmall': wsmall}


def _prep_core(inp, b):
    """Per-core embedding gather: [64, 1100] fp16."""
    pep = np.asarray(inp['peptide'])[b]
    mhc = np.asarray(inp['MHC'])[b]
    pe = np.asarray(inp['pep_emb'], np.float32)[pep].T   # [64, 100]
    me = np.asarray(inp['mhc_emb'], np.float32)[mhc].T   # [64, 1000]
    return np.concatenate([pe, me], axis=1).astype(np.float16)


def kernel(**inputs):
    nc = _get_program()
    shared = _prep_weights(inputs)
    in_maps = []
    for b in range(B):
        m = dict(shared)
        m['emb'] = _prep_core(inputs, b)
        in_maps.append(m)
    res = run_bass_kernel_spmd(nc, in_maps, core_ids=list(range(B)))
    return np.stack([np.asarray(res.results[i]['out']).reshape(2) for i in range(B)]).astype(np.float32)


# revision 17
# speedup vs baseline: 2.5870x; 1.0042x over previous
"""AttentionDTI forward pass on 8 Trainium2 NeuronCores (pure data parallel).

One batch element per core, weights replicated. All matmul operands are
16-bit (f32 PSUM accumulation): fp16 for conv/FC weights and activations,
bf16 for the attention tiles (the Activation engine runs ~1.3x slower on
fp16 than bf16, and the attention path tolerates bf16). Embedding lookup
is done host-side; weights arrive in packed DMAs issued from both the SP
and Activation HWDGE queues so transfers overlap the NEFF prologue.

The 4D additive-attention tensor h[b,p,m,c] = relu(pa + ma) is never
materialized: mean_m(h @ Wa) == mean_m(h) @ Wa, so only hp[c,p] = sum_m h
and hm[c,m] = sum_p h are accumulated on the fly. hm accumulates in PSUM
by streaming h tiles through the PE against a stationary identity; hp
comes from fused per-tile accumulators: the Scalar engine's
relu+bias+accum activation alternates 1:1 with the Vector engine's
scalar_tensor_tensor (relu via max-with-zeros + sum accum) — any DVE op
with an accum output runs at the 1x element rate, so the fused form is
optimal on both engines.

c-channels [128:160] run packed: ma rows replicated 4x vertically (via
host-replicated stationary columns, free) so each tile covers 4 peptide
positions; a 4-stacked [128,32] identity reduces them into hm1. The
peptide gate keeps its column axis in "j-major" order (jm(p) =
(p%4)*22 + p//4) end-to-end — max-pool over p is order-invariant — which
makes the packed hp1p contributions contiguous matmuls and avoids any
unpack DMAs.

Environment constraints discovered empirically (this axon terminal):
  - GPSIMD/Pool compute ops fail codegen; SWDGE DMA hangs: DMAs go
    through SP/Activation HWDGE only.
  - tensor_tensor_reduce fails walrus codegen ("ISA wrong length").
  - tensor_scalar's accum_out hijacks op1 as the reduce op (no fused
    two-op elementwise + sum) — scalar_tensor_tensor does fuse it.
  - walrus allows at most ONE semaphore wait per instruction:
    _split_excess_waits() rewrites the scheduled program.
"""
import sys

_BASS_ROOT = '/opt/trn_rl_repo'
if _BASS_ROOT not in sys.path:
    sys.path.insert(0, _BASS_ROOT)

import numpy as np

import concourse.bass as bass
import concourse.tile as tile
from concourse import mybir
from concourse.bass_utils import run_bass_kernel_spmd

F32 = mybir.dt.float32
F16 = mybir.dt.float16
BF16 = mybir.dt.bfloat16
ALU = mybir.AluOpType
AF = mybir.ActivationFunctionType
AX = mybir.AxisListType

B = 8
LP, LM, DIM, CONV = 100, 1000, 64, 40
C2, C4 = CONV * 2, CONV * 4          # 80, 160
K1, K2, K3 = 4, 6, 8
LP1, LP2, LP3 = 97, 92, 85           # peptide conv output lengths
LM1, LM2, LM3 = 997, 992, 985        # MHC conv output lengths
NP4 = 22                             # ceil(85/4) packed p-groups
JM = 4 * NP4                         # 88 j-major gate columns
MP = 992                             # LM3 padded for 4x-eligible DVE gate ops
NEG = -30000.0
SPLIT_MOD = 2                        # i % SPLIT_MOD == 0 -> scalar h tile

# ---- wboot column map (fp16 [128, 1280]): conv1+conv2 weights ----
PW1, MW1, PW2, MW2 = 0, 160, 320, 800
NBOOT = 1280
# ---- wc3 column map (fp16 [128, 2560]): conv3 weights ----
PW3, MW3 = 0, 1280
NC3 = 2560
# ---- wattn column map (fp16 [128, 1632]) ----
WPA_A, WPA_B = 0, 160        # [128,160], [32,160]
WMA_A, WMA_B = 320, 576      # [128,256], [32,256] (cols 128:256 = rep4 of Wma[:,128:160])
WCA_A, WCA_B = 832, 992      # Wa/LM3: [128,160], [32,160]
WM2_A, WM2_B = 1152, 1312    # Wa/LP3
ID128, IDST = 1472, 1600     # bf16 identities: [128,128], [128,32]
WCB0, WCB1 = 1632, 2144      # j-lifted Wa[128:160]/LM3: 4x[128,128], 4x[128,32]
NATTN = 2272
# ---- wfc column map (fp16 [128, 16392]) ----
W1A, W1B = 0, 2048           # [128, 2048], [32, 2048]
W2C, W3C, WOC = 4096, 12288, 16384
NFC = 16392
# ---- wsmall column map (f32 [128, 35]) ----
SB_PB1, SB_PB2, SB_PB3A, SB_PB3B = 0, 1, 2, 3
SB_MB1, SB_MB2, SB_MB3A, SB_MB3B = 4, 5, 6, 7
SB_BPA_A, SB_BPA_B = 8, 9
SB_BMA_A, SB_BMA_R4 = 10, 11
SB_BA_A, SB_BA_B = 12, 13
SB_B1, SB_B2, SB_B3, SB_BO = 14, 22, 30, 34
NSMALL = 35


def _jm(p):
    return (p % 4) * NP4 + (p // 4)


_ctr = [0]


def _split_excess_waits(nc, max_waits=1):
    n_split = 0
    for f in nc.m.functions:
        for b in f.blocks:
            insts = list(b.instructions)
            out = []
            changed = False
            for inst in insts:
                si = inst.sync_info
                waits = list(si.on_wait) if (si is not None and si.on_wait) else []
                if len(waits) > max_waits:
                    changed = True
                    n_split += 1
                    keep = max(1, max_waits)
                    head, tail = waits[:-keep], waits[-keep:]
                    for i in range(0, len(head), keep):
                        chunk = head[i:i + keep]
                        nop = mybir.InstEventSemaphore(
                            name=f"ant-wait-split-{_ctr[0]}", ins=[], outs=[])
                        _ctr[0] += 1
                        nop.engine = inst.engine
                        nop.sync_info = mybir.SyncInfo(on_wait=chunk, on_update=[])
                        nc.register_instruction(nop)
                        out.append(nop)
                    upd = list(si.on_update) if si.on_update else []
                    inst.sync_info = mybir.SyncInfo(on_wait=tail, on_update=upd)
                out.append(inst)
            if changed:
                b.instructions = out
    return n_split


def _conv_matmuls(nc, psum, wtile, x, k_taps, co_lo, co_hi, m_lo, m_hi, cout_stride):
    """Valid 1-D conv as k shifted matmuls accumulated into `psum`."""
    for k in range(k_taps):
        nc.tensor.matmul(
            psum,
            wtile[:, k * cout_stride + co_lo: k * cout_stride + co_hi],
            x[:, m_lo + k: m_hi + k],
            start=(k == 0), stop=(k == k_taps - 1))


def _build_program():
    nc = bass.Bass("TRN2", target_bir_lowering=False, debug=False)

    emb_e = nc.declare_dram_parameter("emb", [DIM, LP + LM], BF16, isOutput=False)
    wsmall_e = nc.declare_dram_parameter("wsmall", [128, NSMALL], F32, isOutput=False)
    wboot_e = nc.declare_dram_parameter("wboot", [128, NBOOT], BF16, isOutput=False)
    wc3_e = nc.declare_dram_parameter("wc3", [128, NC3], BF16, isOutput=False)
    wattn_e = nc.declare_dram_parameter("wattn", [128, NATTN], F16, isOutput=False)
    wfc_e = nc.declare_dram_parameter("wfc", [128, NFC], F16, isOutput=False)
    out_e = nc.declare_dram_parameter("out", [2, 1], F32, isOutput=True)

    with tile.TileContext(nc) as tc:
        with tc.tile_pool(name="consts", bufs=1) as cp, \
             tc.tile_pool(name="work", bufs=1) as wp, \
             tc.tile_pool(name="hpool", bufs=8) as hpool, \
             tc.tile_pool(name="ps_hm", bufs=1, space="PSUM") as ps_hm, \
             tc.tile_pool(name="ps_work", bufs=2, space="PSUM") as ps:

            # loads: small/boot/emb from SP; conv3+attn from Activation HWDGE;
            # the big FC pack last on SP.
            emb = cp.tile([DIM, LP + LM], BF16, name="emb")
            nc.sync.dma_start(out=emb, in_=emb_e[:])
            wboot = cp.tile([128, NBOOT], BF16, name="wboot")
            nc.sync.dma_start(out=wboot, in_=wboot_e[:])
            wsmall = cp.tile([128, NSMALL], F32, name="wsmall")
            nc.sync.dma_start(out=wsmall, in_=wsmall_e[:])
            wc3 = cp.tile([128, NC3], BF16, name="wc3")
            nc.scalar.dma_start(out=wc3, in_=wc3_e[:])
            wattn = cp.tile([128, NATTN], F16, name="wattn")
            nc.scalar.dma_start(out=wattn, in_=wattn_e[:])
            wfc = cp.tile([128, NFC], F16, name="wfc")
            nc.sync.dma_start(out=wfc, in_=wfc_e[:])

            pe = emb[:, 0:LP]
            me = emb[:, LP:LP + LM]
            bias = lambda col, rows=128: wsmall[0:rows, col:col + 1]

            # early zero/NEG fills on the (idle) Vector engine
            zt = wp.tile([128, LM3], BF16, name="zt")
            nc.vector.memset(zt, 0.0)
            hp0 = wp.tile([128, JM], F32, name="hp0")
            nc.vector.memset(hp0, 0.0)
            pa1p = wp.tile([128, NP4], F32, name="pa1p")
            nc.vector.memset(pa1p, NEG)
            pc0jm = wp.tile([128, JM], F16, name="pc0jm")
            nc.vector.memset(pc0jm, 0.0)
            pc1jm = wp.tile([32, JM], F16, name="pc1jm")
            nc.vector.memset(pc1jm, 0.0)
            hm0f = wp.tile([128, MP], BF16, name="hm0f")
            nc.vector.memset(hm0f[:, LM3:MP], 0.0)
            hm1f = wp.tile([32, MP], BF16, name="hm1f")
            nc.vector.memset(hm1f[:, LM3:MP], 0.0)
            mc0 = wp.tile([128, MP], F16, name="mc0")
            nc.vector.memset(mc0[:, LM3:MP], 0.0)
            mc1 = wp.tile([32, MP], F16, name="mc1")
            nc.vector.memset(mc1[:, LM3:MP], 0.0)

            # ================= conv stacks (fp16, f32 psum) =================
            # MHC conv1: [64,1000] -> [40,997]; relu chunked so conv2 starts early
            mx1_ps = ps.tile([CONV, LM1], F32, name="mx1_ps", tag="ps")
            _conv_matmuls(nc, mx1_ps[:, 0:512], wboot[0:DIM, MW1:MW1 + K1 * CONV], me, K1, 0, CONV, 0, 512, CONV)
            _conv_matmuls(nc, mx1_ps[:, 512:LM1], wboot[0:DIM, MW1:MW1 + K1 * CONV], me, K1, 0, CONV, 512, LM1, CONV)
            # peptide conv1 fills the PE while relu1 runs
            px1_ps = ps.tile([CONV, LP1], F32, name="px1_ps", tag="ps")
            _conv_matmuls(nc, px1_ps, wboot[0:DIM, PW1:PW1 + K1 * CONV], pe, K1, 0, CONV, 0, LP1, CONV)
            mx1 = wp.tile([CONV, LM1], BF16, name="mx1")
            nc.scalar.activation(out=mx1[:, 0:520], in_=mx1_ps[:, 0:520], func=AF.Relu, bias=bias(SB_MB1, CONV))
            nc.vector.tensor_scalar(out=mx1[:, 520:LM1], in0=mx1_ps[:, 520:LM1], scalar1=bias(SB_MB1, CONV),
                                    scalar2=0.0, op0=ALU.add, op1=ALU.max)
            px1 = wp.tile([CONV, LP1], BF16, name="px1")
            nc.scalar.activation(out=px1, in_=px1_ps, func=AF.Relu, bias=bias(SB_PB1, CONV))

            # MHC conv2 -> [80, 992]
            mx2_ps = ps.tile([C2, LM2], F32, name="mx2_ps", tag="ps")
            _conv_matmuls(nc, mx2_ps[:, 0:512], wboot[0:CONV, MW2:MW2 + K2 * C2], mx1, K2, 0, C2, 0, 512, C2)
            _conv_matmuls(nc, mx2_ps[:, 512:LM2], wboot[0:CONV, MW2:MW2 + K2 * C2], mx1, K2, 0, C2, 512, LM2, C2)
            px2_ps = ps.tile([C2, LP2], F32, name="px2_ps", tag="ps")
            _conv_matmuls(nc, px2_ps, wboot[0:CONV, PW2:PW2 + K2 * C2], px1, K2, 0, C2, 0, LP2, C2)
            mx2 = wp.tile([C2, LM2], BF16, name="mx2")
            nc.scalar.activation(out=mx2[:, 0:520], in_=mx2_ps[:, 0:520], func=AF.Relu, bias=bias(SB_MB2, C2))
            nc.vector.tensor_scalar(out=mx2[:, 520:LM2], in0=mx2_ps[:, 520:LM2], scalar1=bias(SB_MB2, C2),
                                    scalar2=0.0, op0=ALU.add, op1=ALU.max)
            px2 = wp.tile([C2, LP2], BF16, name="px2")
            nc.scalar.activation(out=px2, in_=px2_ps, func=AF.Relu, bias=bias(SB_PB2, C2))

            # MHC conv3 -> [160, 985] as [128,985]+[32,985] (into MP-padded tiles)
            mc0_ps = ps.tile([128, LM3], F32, name="mc0_ps", tag="ps")
            _conv_matmuls(nc, mc0_ps[:, 0:512], wc3[0:C2, MW3:MW3 + K3 * C4], mx2, K3, 0, 128, 0, 512, C4)
            _conv_matmuls(nc, mc0_ps[:, 512:LM3], wc3[0:C2, MW3:MW3 + K3 * C4], mx2, K3, 0, 128, 512, LM3, C4)
            nc.scalar.activation(out=mc0[:, 0:512], in_=mc0_ps[:, 0:512], func=AF.Relu, bias=bias(SB_MB3A))
            nc.vector.tensor_scalar(out=mc0[:, 512:LM3], in0=mc0_ps[:, 512:LM3], scalar1=bias(SB_MB3A),
                                    scalar2=0.0, op0=ALU.add, op1=ALU.max)
            mc1_ps = ps.tile([32, LM3], F32, name="mc1_ps", tag="ps")
            _conv_matmuls(nc, mc1_ps[:, 0:512], wc3[0:C2, MW3:MW3 + K3 * C4], mx2, K3, 128, C4, 0, 512, C4)
            _conv_matmuls(nc, mc1_ps[:, 512:LM3], wc3[0:C2, MW3:MW3 + K3 * C4], mx2, K3, 128, C4, 512, LM3, C4)
            nc.scalar.activation(out=mc1[:, 0:512], in_=mc1_ps[:, 0:512], func=AF.Relu, bias=bias(SB_MB3B, 32))
            nc.vector.tensor_scalar(out=mc1[:, 512:LM3], in0=mc1_ps[:, 512:LM3], scalar1=bias(SB_MB3B, 32),
                                    scalar2=0.0, op0=ALU.add, op1=ALU.max)

            # peptide conv3 (tiles padded to 88 cols for the 4-strided views)
            pc0_ps = ps.tile([128, LP3], F32, name="pc0_ps", tag="ps")
            _conv_matmuls(nc, pc0_ps, wc3[0:C2, PW3:PW3 + K3 * C4], px2, K3, 0, 128, 0, LP3, C4)
            pc0 = wp.tile([128, 88], F16, name="pc0")
            nc.scalar.activation(out=pc0[:, 0:LP3], in_=pc0_ps, func=AF.Relu, bias=bias(SB_PB3A))
            pc1_ps = ps.tile([32, LP3], F32, name="pc1_ps", tag="ps")
            _conv_matmuls(nc, pc1_ps, wc3[0:C2, PW3:PW3 + K3 * C4], px2, K3, 128, C4, 0, LP3, C4)
            pc1 = wp.tile([32, 88], F16, name="pc1")
            nc.scalar.activation(out=pc1[:, 0:LP3], in_=pc1_ps, func=AF.Relu, bias=bias(SB_PB3B, 32))

            # j-major copies of pc for the gate (vector, strided reads)
            pc0_g = pc0.rearrange("c (g f) -> c g f", f=4)
            pc1_g = pc1.rearrange("c (g f) -> c g f", f=4)
            for j in range(4):
                ncol = NP4 if j == 0 else NP4 - 1
                nc.vector.tensor_scalar(out=pc0jm[:, j * NP4:j * NP4 + ncol],
                                        in0=pc0_g[:, 0:ncol, j], scalar1=0.0,
                                        scalar2=None, op0=ALU.add)
                nc.vector.tensor_scalar(out=pc1jm[:, j * NP4:j * NP4 + ncol],
                                        in0=pc1_g[:, 0:ncol, j], scalar1=0.0,
                                        scalar2=None, op0=ALU.add)

            # ================= attention projections =================
            # ma0[c,m] c in 0:128 (bf16 for the Activation-engine h producer)
            ma0_ps = ps.tile([128, LM3], F32, name="ma0_ps", tag="ps")
            for lo, hi in ((0, 512), (512, LM3)):
                nc.tensor.matmul(ma0_ps[:, lo:hi], wattn[0:128, WMA_A:WMA_A + 128], mc0[:, lo:hi], start=True, stop=False)
                nc.tensor.matmul(ma0_ps[:, lo:hi], wattn[0:32, WMA_B:WMA_B + 128], mc1[:, lo:hi], start=False, stop=True)
            ma0 = wp.tile([128, LM3], BF16, name="ma0")
            nc.scalar.activation(out=ma0, in_=ma0_ps, func=AF.Identity, bias=bias(SB_BMA_A))

            # ma1p: c in 128:160 replicated 4x vertically (stationary pre-replicated)
            ma1p_ps = ps.tile([128, LM3], F32, name="ma1p_ps", tag="ps")
            for lo, hi in ((0, 512), (512, LM3)):
                nc.tensor.matmul(ma1p_ps[:, lo:hi], wattn[0:128, WMA_A + 128:WMA_A + 256], mc0[:, lo:hi], start=True, stop=False)
                nc.tensor.matmul(ma1p_ps[:, lo:hi], wattn[0:32, WMA_B + 128:WMA_B + 256], mc1[:, lo:hi], start=False, stop=True)
            ma1p = wp.tile([128, LM3], BF16, name="ma1p")
            nc.vector.tensor_scalar(out=ma1p, in0=ma1p_ps, scalar1=bias(SB_BMA_R4), scalar2=None, op0=ALU.add)

            # pa0[c,p] c in 0:128 (f32, used as per-partition bias)
            pa0_ps = ps.tile([128, LP3], F32, name="pa0_ps", tag="ps")
            nc.tensor.matmul(pa0_ps, wattn[0:128, WPA_A:WPA_A + 128], pc0[:, 0:LP3], start=True, stop=False)
            nc.tensor.matmul(pa0_ps, wattn[0:32, WPA_B:WPA_B + 128], pc1[0:32, 0:LP3], start=False, stop=True)
            pa0 = wp.tile([128, LP3], F32, name="pa0")
            nc.scalar.add(pa0, pa0_ps, bias(SB_BPA_A))

            # pa1p[32j+d, g] = pa[128+d, 4g+j]: partition-offset matmuls over
            # 4-strided moving views of pc
            pa1p_ps = ps.tile([128, NP4], F32, name="pa1p_ps", tag="ps")
            for j in range(4):
                ncol = NP4 if j == 0 else NP4 - 1
                nc.tensor.matmul(pa1p_ps[32 * j:32 * j + 32, 0:ncol],
                                 wattn[0:128, WPA_A + 128:WPA_A + 160],
                                 pc0_g[:, 0:ncol, j],
                                 start=True, stop=False, skip_group_check=True,
                                 tile_position=(0, 32 * j))
                nc.tensor.matmul(pa1p_ps[32 * j:32 * j + 32, 0:ncol],
                                 wattn[0:32, WPA_B + 128:WPA_B + 160],
                                 pc1_g[:, 0:ncol, j],
                                 start=False, stop=True, skip_group_check=True,
                                 tile_position=(0, 32 * j))
                nc.scalar.add(pa1p[32 * j:32 * j + 32, 0:ncol],
                              pa1p_ps[32 * j:32 * j + 32, 0:ncol], bias(SB_BPA_B, 32))

            # ================= 4D attention reductions =================
            # hp0 columns are written in j-major order (gate is order-free)
            hp1p = wp.tile([128, NP4], F32, name="hp1p")
            hm0_ps = ps_hm.tile([128, LM3], F32, name="hm0_ps")
            hm1_ps = ps_hm.tile([32, LM3], F32, name="hm1_ps")

            # identities and gate weights stored as bf16 bit patterns in the
            # fp16 pack; bitcast views pair them with bf16 moving operands
            id128 = wattn[0:128, ID128:ID128 + 128].bitcast(BF16)
            idst = wattn[0:128, IDST:IDST + 32].bitcast(BF16)

            # greedy producer balance: assign each tile to whichever engine
            # would finish it first (measured fused costs: ACT 1199, DVE 1263)
            prod_t = {'sc': 0.0, 've': 0.0}

            def h_tile(i, src, bias_ap, acc):
                h = hpool.tile([128, LM3], BF16, tag="h", name="h")
                if prod_t['sc'] + 1199.0 <= prod_t['ve'] + 1263.0:
                    prod_t['sc'] += 1199.0
                    nc.scalar.activation(out=h, in_=src, func=AF.Relu,
                                         bias=bias_ap, accum_out=acc)
                else:
                    prod_t['ve'] += 1263.0
                    nc.vector.scalar_tensor_tensor(out=h, in0=src, scalar=bias_ap,
                                                   in1=zt, op0=ALU.add, op1=ALU.max,
                                                   accum_out=acc)
                return h

            for p in range(LP3):
                h = h_tile(p, ma0, pa0[:, p:p + 1], hp0[:, _jm(p):_jm(p) + 1])
                nc.tensor.matmul(hm0_ps[:, 0:512], id128, h[:, 0:512],
                                 start=(p == 0), stop=(p == LP3 - 1))
                nc.tensor.matmul(hm0_ps[:, 512:LM3], id128, h[:, 512:LM3],
                                 start=(p == 0), stop=(p == LP3 - 1))

            # p-side gate head start: hp0 is complete before the packed loop
            hp0f = wp.tile([128, JM], BF16, name="hp0f")
            nc.scalar.copy(hp0f, hp0)
            cl0_ps = ps.tile([128, JM], F32, name="cl0_ps", tag="ps")
            nc.tensor.matmul(cl0_ps, wattn[0:128, WCA_A:WCA_A + 128].bitcast(BF16), hp0f,
                             start=True, stop=False, skip_group_check=True)
            cl1_ps = ps.tile([32, JM], F32, name="cl1_ps", tag="ps")
            nc.tensor.matmul(cl1_ps, wattn[0:128, WCA_A + 128:WCA_A + 160].bitcast(BF16), hp0f,
                             start=True, stop=False, skip_group_check=True)

            for g in range(NP4):
                h = h_tile(LP3 + g, ma1p, pa1p[:, g:g + 1], hp1p[:, g:g + 1])
                nc.tensor.matmul(hm1_ps[:, 0:512], idst, h[:, 0:512],
                                 start=(g == 0), stop=(g == NP4 - 1))
                nc.tensor.matmul(hm1_ps[:, 512:LM3], idst, h[:, 512:LM3],
                                 start=(g == 0), stop=(g == NP4 - 1))

            # ================= peptide gate (j-major) =================
            hp1pf = wp.tile([128, NP4], BF16, name="hp1pf")
            nc.scalar.copy(hp1pf, hp1p)
            for j in range(4):
                nc.tensor.matmul(cl0_ps[:, j * NP4:(j + 1) * NP4],
                                 wattn[0:128, WCB0 + j * 128:WCB0 + (j + 1) * 128].bitcast(BF16),
                                 hp1pf, start=False, stop=(j == 3), skip_group_check=True)
                nc.tensor.matmul(cl1_ps[:, j * NP4:(j + 1) * NP4],
                                 wattn[0:128, WCB1 + j * 32:WCB1 + (j + 1) * 32].bitcast(BF16),
                                 hp1pf, start=False, stop=(j == 3), skip_group_check=True)
            catt0 = wp.tile([128, JM], F16, name="catt0")
            nc.scalar.activation(out=catt0, in_=cl0_ps, func=AF.Sigmoid, bias=bias(SB_BA_A))
            catt1 = wp.tile([32, JM], F16, name="catt1")
            nc.scalar.activation(out=catt1, in_=cl1_ps, func=AF.Sigmoid, bias=bias(SB_BA_B, 32))

            pvf = wp.tile([128, 4], F16, name="pvf")   # cols: pv0, pv1, mv0, mv1
            pg0 = wp.tile([128, JM], F16, name="pg0")
            nc.vector.scalar_tensor_tensor(out=pg0, in0=catt0, scalar=0.5, in1=pc0jm,
                                           op0=ALU.add, op1=ALU.mult)
            pg1 = wp.tile([32, JM], F16, name="pg1")
            nc.vector.scalar_tensor_tensor(out=pg1, in0=catt1, scalar=0.5, in1=pc1jm,
                                           op0=ALU.add, op1=ALU.mult)
            with nc.allow_low_precision(reason="fp16 max-pool rounds values only"):
                nc.vector.tensor_reduce(out=pvf[:, 0:1], in_=pg0, op=ALU.max, axis=AX.X)
                nc.vector.tensor_reduce(out=pvf[0:32, 1:2], in_=pg1, op=ALU.max, axis=AX.X)

            # ================= MHC gate (chunk-pipelined) =================
            for lo, hi in ((0, 512), (512, LM3)):
                nc.scalar.activation(out=hm0f[:, lo:hi], in_=hm0_ps[:, lo:hi], func=AF.Copy)
                nc.vector.tensor_scalar(out=hm1f[:, lo:hi], in0=hm1_ps[:, lo:hi],
                                        scalar1=0.0, scalar2=None, op0=ALU.add)
            ml0_ps = ps.tile([128, MP], F32, name="ml0_ps", tag="ps")
            ml1_ps = ps.tile([32, MP], F32, name="ml1_ps", tag="ps")
            matt0 = wp.tile([128, MP], F16, name="matt0")
            matt1 = wp.tile([32, MP], F16, name="matt1")
            for lo, hi in ((0, 512), (512, MP)):
                nc.tensor.matmul(ml0_ps[:, lo:hi], wattn[0:128, WM2_A:WM2_A + 128].bitcast(BF16), hm0f[:, lo:hi], start=True, stop=False)
                nc.tensor.matmul(ml0_ps[:, lo:hi], wattn[0:32, WM2_B:WM2_B + 128].bitcast(BF16), hm1f[:, lo:hi], start=False, stop=True)
                nc.scalar.activation(out=matt0[:, lo:hi], in_=ml0_ps[:, lo:hi], func=AF.Sigmoid, bias=bias(SB_BA_A))
                nc.tensor.matmul(ml1_ps[:, lo:hi], wattn[0:128, WM2_A + 128:WM2_A + 160].bitcast(BF16), hm0f[:, lo:hi], start=True, stop=False)
                nc.tensor.matmul(ml1_ps[:, lo:hi], wattn[0:32, WM2_B + 128:WM2_B + 160].bitcast(BF16), hm1f[:, lo:hi], start=False, stop=True)
                nc.scalar.activation(out=matt1[:, lo:hi], in_=ml1_ps[:, lo:hi], func=AF.Sigmoid, bias=bias(SB_BA_B, 32))

            mg0 = wp.tile([128, MP], F16, name="mg0")
            nc.vector.scalar_tensor_tensor(out=mg0, in0=matt0, scalar=0.5, in1=mc0,
                                           op0=ALU.add, op1=ALU.mult)
            mg1 = wp.tile([32, MP], F16, name="mg1")
            nc.vector.scalar_tensor_tensor(out=mg1, in0=matt1, scalar=0.5, in1=mc1,
                                           op0=ALU.add, op1=ALU.mult)
            with nc.allow_low_precision(reason="fp16 max-pool rounds values only"):
                nc.vector.tensor_reduce(out=pvf[:, 2:3], in_=mg0, op=ALU.max, axis=AX.X)
                nc.vector.tensor_reduce(out=pvf[0:32, 3:4], in_=mg1, op=ALU.max, axis=AX.X)

            # ================= FC head =================
            def lrelu(name, f_ps, bias_lo, ncols):
                fb = wp.tile([128, ncols], F32, name=name + "_b")
                nc.vector.tensor_tensor(out=fb, in0=f_ps, in1=wsmall[:, bias_lo:bias_lo + ncols], op=ALU.add)
                fs = wp.tile([128, ncols], F32, name=name + "_s")
                nc.vector.tensor_scalar(out=fs, in0=fb, scalar1=0.01, scalar2=None, op0=ALU.mult)
                fo = wp.tile([128, ncols], F16, name=name)
                nc.vector.tensor_tensor(out=fo, in0=fb, in1=fs, op=ALU.max)
                return fo

            # f1: per-column accumulation groups (one 2KB region holds all
            # columns, so groups must not interleave); within a column the
            # two 128-row stationaries go first to reduce PE config flips
            f1_ps = ps.tile([128, 8], F32, name="f1_ps", tag="ps")
            for a in range(8):
                nc.tensor.matmul(f1_ps[:, a:a + 1], wfc[0:128, W1A + a * 128:W1A + a * 128 + 128],
                                 pvf[:, 0:1], start=True, stop=False)
                nc.tensor.matmul(f1_ps[:, a:a + 1], wfc[0:128, W1A + 1024 + a * 128:W1A + 1024 + a * 128 + 128],
                                 pvf[:, 2:3], start=False, stop=False)
                nc.tensor.matmul(f1_ps[:, a:a + 1], wfc[0:32, W1B + a * 128:W1B + a * 128 + 128],
                                 pvf[0:32, 1:2], start=False, stop=False)
                nc.tensor.matmul(f1_ps[:, a:a + 1], wfc[0:32, W1B + 1024 + a * 128:W1B + 1024 + a * 128 + 128],
                                 pvf[0:32, 3:4], start=False, stop=True)
            f1 = lrelu("f1", f1_ps, SB_B1, 8)

            f2_ps = ps.tile([128, 8], F32, name="f2_ps", tag="ps")
            for a in range(8):
                for jb in range(8):
                    nc.tensor.matmul(f2_ps[:, a:a + 1],
                                     wfc[0:128, W2C + jb * 1024 + a * 128:W2C + jb * 1024 + a * 128 + 128],
                                     f1[:, jb:jb + 1], start=(jb == 0), stop=(jb == 7))
            f2 = lrelu("f2", f2_ps, SB_B2, 8)

            f3_ps = ps.tile([128, 4], F32, name="f3_ps", tag="ps")
            for a in range(4):
                for jb in range(8):
                    nc.tensor.matmul(f3_ps[:, a:a + 1],
                                     wfc[0:128, W3C + jb * 512 + a * 128:W3C + jb * 512 + a * 128 + 128],
                                     f2[:, jb:jb + 1], start=(jb == 0), stop=(jb == 7))
            f3 = lrelu("f3", f3_ps, SB_B3, 4)

            o_ps = ps.tile([2, 1], F32, name="o_ps", tag="ps")
            for c in range(4):
                nc.tensor.matmul(o_ps, wfc[0:128, WOC + 2 * c:WOC + 2 * c + 2], f3[:, c:c + 1],
                                 start=(c == 0), stop=(c == 3))
            o_sb = wp.tile([2, 1], F32, name="o_sb")
            nc.vector.tensor_tensor(out=o_sb, in0=o_ps, in1=wsmall[0:2, SB_BO:SB_BO + 1], op=ALU.add)
            nc.sync.dma_start(out=out_e[:], in_=o_sb)

    _split_excess_waits(nc, max_waits=1)
    return nc


_PROGRAM = None


def _get_program():
    global _PROGRAM
    if _PROGRAM is None:
        _PROGRAM = _build_program()
    return _PROGRAM


def _prep_weights(inp):
    """Host-side packing shared by all cores."""
    import ml_dtypes
    f16 = np.float16
    bf16 = ml_dtypes.bfloat16
    f32 = lambda x: np.asarray(x, dtype=np.float32)
    as_f16bits = lambda a: np.ascontiguousarray(a).view(np.uint16).view(f16)

    def convw(w):  # [co, ci, k] -> [ci, k*co] bf16
        w = np.asarray(w, dtype=np.float32)
        ci = w.shape[1]
        return w.transpose(1, 2, 0).reshape(ci, -1).astype(bf16)

    wboot = np.zeros((128, NBOOT), bf16)
    wboot[0:DIM, PW1:PW1 + K1 * CONV] = convw(inp['pw1'])
    wboot[0:DIM, MW1:MW1 + K1 * CONV] = convw(inp['mw1'])
    wboot[0:CONV, PW2:PW2 + K2 * C2] = convw(inp['pw2'])
    wboot[0:CONV, MW2:MW2 + K2 * C2] = convw(inp['mw2'])

    wc3 = np.zeros((128, NC3), bf16)
    wc3[0:C2, PW3:PW3 + K3 * C4] = convw(inp['pw3'])
    wc3[0:C2, MW3:MW3 + K3 * C4] = convw(inp['mw3'])

    wpa, wma = f32(inp['Wpa']), f32(inp['Wma'])
    wca = f32(inp['Wa']) / float(LM3)
    wm2 = f32(inp['Wa']) / float(LP3)
    wattn = np.zeros((128, NATTN), f16)
    wattn[0:128, WPA_A:WPA_A + 160] = wpa[0:128].astype(f16)
    wattn[0:32, WPA_B:WPA_B + 160] = wpa[128:160].astype(f16)
    wattn[0:128, WMA_A:WMA_A + 128] = wma[0:128, 0:128].astype(f16)
    wattn[0:128, WMA_A + 128:WMA_A + 256] = np.tile(wma[0:128, 128:160], (1, 4)).astype(f16)
    wattn[0:32, WMA_B:WMA_B + 128] = wma[128:160, 0:128].astype(f16)
    wattn[0:32, WMA_B + 128:WMA_B + 256] = np.tile(wma[128:160, 128:160], (1, 4)).astype(f16)
    wattn[0:128, WCA_A:WCA_A + 160] = as_f16bits(wca[0:128].astype(bf16))
    wattn[0:32, WCA_B:WCA_B + 160] = as_f16bits(wca[128:160].astype(bf16))
    wattn[0:128, WM2_A:WM2_A + 160] = as_f16bits(wm2[0:128].astype(bf16))
    wattn[0:32, WM2_B:WM2_B + 160] = as_f16bits(wm2[128:160].astype(bf16))
    id128 = np.eye(128, dtype=bf16)
    idst = np.tile(np.eye(32, dtype=bf16), (4, 1))
    wattn[0:128, ID128:ID128 + 128] = as_f16bits(id128)
    wattn[0:128, IDST:IDST + 32] = as_f16bits(idst)
    for j in range(4):
        wattn[32 * j:32 * j + 32, WCB0 + j * 128:WCB0 + (j + 1) * 128] = as_f16bits(wca[128:160, 0:128].astype(bf16))
        wattn[32 * j:32 * j + 32, WCB1 + j * 32:WCB1 + (j + 1) * 32] = as_f16bits(wca[128:160, 128:160].astype(bf16))

    w1 = f32(inp['W1'])
    wfc = np.zeros((128, NFC), f16)
    wfc[0:128, W1A:W1A + 2048] = np.concatenate([w1[0:128], w1[160:288]], axis=1).astype(f16)
    wfc[0:32, W1B:W1B + 2048] = np.concatenate([w1[128:160], w1[288:320]], axis=1).astype(f16)

    def fcw(w, nblk):  # [I, J], I = nblk*128 -> [128, nblk*J]
        w = np.asarray(w, dtype=np.float32)
        i, j = w.shape
        return w.reshape(nblk, 128, j).transpose(1, 0, 2).reshape(128, nblk * j).astype(f16)

    wfc[0:128, W2C:W2C + 8192] = fcw(inp['W2'], 8)
    wfc[0:128, W3C:W3C + 4096] = fcw(inp['W3'], 8)
    wfc[0:128, WOC:WOC + 8] = fcw(inp['Wo'], 4)

    wsmall = np.zeros((128, NSMALL), np.float32)
    def bias2(col_a, col_b, b):
        b = f32(b)
        wsmall[0:128, col_a] = b[0:128]
        wsmall[0:32, col_b] = b[128:160]
    wsmall[0:CONV, SB_PB1] = f32(inp['pb1'])
    wsmall[0:C2, SB_PB2] = f32(inp['pb2'])
    bias2(SB_PB3A, SB_PB3B, inp['pb3'])
    wsmall[0:CONV, SB_MB1] = f32(inp['mb1'])
    wsmall[0:C2, SB_MB2] = f32(inp['mb2'])
    bias2(SB_MB3A, SB_MB3B, inp['mb3'])
    bias2(SB_BPA_A, SB_BPA_B, inp['bpa'])
    wsmall[0:128, SB_BMA_A] = f32(inp['bma'])[0:128]
    wsmall[0:128, SB_BMA_R4] = np.tile(f32(inp['bma'])[128:160], 4)
    bias2(SB_BA_A, SB_BA_B, inp['ba'])
    wsmall[0:128, SB_B1:SB_B1 + 8] = f32(inp['b1']).reshape(8, 128).T
    wsmall[0:128, SB_B2:SB_B2 + 8] = f32(inp['b2']).reshape(8, 128).T
    wsmall[0:128, SB_B3:SB_B3 + 4] = f32(inp['b3']).reshape(4, 128).T
    wsmall[0:2, SB_BO] = f32(inp['bo'])

    return {'wboot': wboot, 'wc3': wc3, 'wattn': wattn, 'wfc': wfc, 'wsmall': wsmall}


def _prep_core(inp, b):
    """Per-core embedding gather: [64, 1100] fp16."""
    pep = np.asarray(inp['peptide'])[b]
    mhc = np.asarray(inp['MHC'])[b]
    import ml_dtypes
    pe = np.asarray(inp['pep_emb'], np.float32)[pep].T   # [64, 100]
    me = np.asarray(inp['mhc_emb'], np.float32)[mhc].T   # [64, 1000]
    return np.concatenate([pe, me], axis=1).astype(ml_dtypes.bfloat16)


def kernel(**inputs):
    nc = _get_program()
    shared = _prep_weights(inputs)
    in_maps = []
    for b in range(B):
        m = dict(shared)
        m['emb'] = _prep_core(inputs, b)
        in_maps.append(m)
    res = run_bass_kernel_spmd(nc, in_maps, core_ids=list(range(B)))
    return np.stack([np.asarray(res.results[i]['out']).reshape(2) for i in range(B)]).astype(np.float32)


# revision 18
# speedup vs baseline: 2.6052x; 1.0070x over previous
"""AttentionDTI forward pass on 8 Trainium2 NeuronCores (pure data parallel).

One batch element per core, weights replicated. All matmul operands are
16-bit (f32 PSUM accumulation): fp16 for conv/FC weights and activations,
bf16 for the attention tiles (the Activation engine runs ~1.3x slower on
fp16 than bf16, and the attention path tolerates bf16). Embedding lookup
is done host-side; weights arrive in packed DMAs issued from both the SP
and Activation HWDGE queues so transfers overlap the NEFF prologue.

The 4D additive-attention tensor h[b,p,m,c] = relu(pa + ma) is never
materialized: mean_m(h @ Wa) == mean_m(h) @ Wa, so only hp[c,p] = sum_m h
and hm[c,m] = sum_p h are accumulated on the fly. hm accumulates in PSUM
by streaming h tiles through the PE against a stationary identity; hp
comes from fused per-tile accumulators: the Scalar engine's
relu+bias+accum activation alternates 1:1 with the Vector engine's
scalar_tensor_tensor (relu via max-with-zeros + sum accum) — any DVE op
with an accum output runs at the 1x element rate, so the fused form is
optimal on both engines.

c-channels [128:160] run packed: ma rows replicated 4x vertically (via
host-replicated stationary columns, free) so each tile covers 4 peptide
positions; a 4-stacked [128,32] identity reduces them into hm1. The
peptide gate keeps its column axis in "j-major" order (jm(p) =
(p%4)*22 + p//4) end-to-end — max-pool over p is order-invariant — which
makes the packed hp1p contributions contiguous matmuls and avoids any
unpack DMAs.

Environment constraints discovered empirically (this axon terminal):
  - GPSIMD/Pool compute ops fail codegen; SWDGE DMA hangs: DMAs go
    through SP/Activation HWDGE only.
  - tensor_tensor_reduce fails walrus codegen ("ISA wrong length").
  - tensor_scalar's accum_out hijacks op1 as the reduce op (no fused
    two-op elementwise + sum) — scalar_tensor_tensor does fuse it.
  - walrus allows at most ONE semaphore wait per instruction:
    _split_excess_waits() rewrites the scheduled program.
"""
import sys

_BASS_ROOT = '/opt/trn_rl_repo'
if _BASS_ROOT not in sys.path:
    sys.path.insert(0, _BASS_ROOT)

import numpy as np

import concourse.bass as bass
import concourse.tile as tile
from concourse import mybir
from concourse.bass_utils import run_bass_kernel_spmd

F32 = mybir.dt.float32
F16 = mybir.dt.float16
BF16 = mybir.dt.bfloat16
ALU = mybir.AluOpType
AF = mybir.ActivationFunctionType
AX = mybir.AxisListType

B = 8
LP, LM, DIM, CONV = 100, 1000, 64, 40
C2, C4 = CONV * 2, CONV * 4          # 80, 160
K1, K2, K3 = 4, 6, 8
LP1, LP2, LP3 = 97, 92, 85           # peptide conv output lengths
LM1, LM2, LM3 = 997, 992, 985        # MHC conv output lengths
NP4 = 22                             # ceil(85/4) packed p-groups
JM = 4 * NP4                         # 88 j-major gate columns
MP = 992                             # LM3 padded for 4x-eligible DVE gate ops
NEG = -30000.0
SPLIT_MOD = 2                        # i % SPLIT_MOD == 0 -> scalar h tile

# ---- wboot column map (fp16 [128, 1280]): conv1+conv2 weights ----
PW1, MW1, PW2, MW2 = 0, 160, 320, 800
NBOOT = 1280
# ---- wc3 column map (fp16 [128, 2560]): conv3 weights ----
PW3, MW3 = 0, 1280
NC3 = 2560
# ---- wattn column map (fp16 [128, 1632]) ----
WPA_A, WPA_B = 0, 160        # [128,160], [32,160]
WMA_A, WMA_B = 320, 576      # [128,256], [32,256] (cols 128:256 = rep4 of Wma[:,128:160])
WCA_A, WCA_B = 832, 992      # Wa/LM3: [128,160], [32,160]
WM2_A, WM2_B = 1152, 1312    # Wa/LP3
ID128, IDST = 1472, 1600     # bf16 identities: [128,128], [128,32]
WCB0, WCB1 = 1632, 2144      # j-lifted Wa[128:160]/LM3: 4x[128,128], 4x[128,32]
NATTN = 2272
# ---- wfc column map (fp16 [128, 16392]) ----
W1A, W1B = 0, 2048           # [128, 2048], [32, 2048]
W2C, W3C, WOC = 4096, 12288, 16384
NFC = 16392
# ---- wsmall column map (f32 [128, 35]) ----
SB_PB1, SB_PB2, SB_PB3A, SB_PB3B = 0, 1, 2, 3
SB_MB1, SB_MB2, SB_MB3A, SB_MB3B = 4, 5, 6, 7
SB_BPA_A, SB_BPA_B = 8, 9
SB_BMA_A, SB_BMA_R4 = 10, 11
SB_BA_A, SB_BA_B = 12, 13
SB_B1, SB_B2, SB_B3, SB_BO = 14, 22, 30, 34
NSMALL = 35


def _jm(p):
    return (p % 4) * NP4 + (p // 4)


_ctr = [0]


def _split_excess_waits(nc, max_waits=1):
    n_split = 0
    for f in nc.m.functions:
        for b in f.blocks:
            insts = list(b.instructions)
            out = []
            changed = False
            for inst in insts:
                si = inst.sync_info
                waits = list(si.on_wait) if (si is not None and si.on_wait) else []
                if len(waits) > max_waits:
                    changed = True
                    n_split += 1
                    keep = max(1, max_waits)
                    head, tail = waits[:-keep], waits[-keep:]
                    for i in range(0, len(head), keep):
                        chunk = head[i:i + keep]
                        nop = mybir.InstEventSemaphore(
                            name=f"ant-wait-split-{_ctr[0]}", ins=[], outs=[])
                        _ctr[0] += 1
                        nop.engine = inst.engine
                        nop.sync_info = mybir.SyncInfo(on_wait=chunk, on_update=[])
                        nc.register_instruction(nop)
                        out.append(nop)
                    upd = list(si.on_update) if si.on_update else []
                    inst.sync_info = mybir.SyncInfo(on_wait=tail, on_update=upd)
                out.append(inst)
            if changed:
                b.instructions = out
    return n_split


def _conv_matmuls(nc, psum, wtile, x, k_taps, co_lo, co_hi, m_lo, m_hi, cout_stride):
    """Valid 1-D conv as k shifted matmuls accumulated into `psum`."""
    for k in range(k_taps):
        nc.tensor.matmul(
            psum,
            wtile[:, k * cout_stride + co_lo: k * cout_stride + co_hi],
            x[:, m_lo + k: m_hi + k],
            start=(k == 0), stop=(k == k_taps - 1))


def _build_program():
    nc = bass.Bass("TRN2", target_bir_lowering=False, debug=False)

    emb_e = nc.declare_dram_parameter("emb", [DIM, LP + LM], F16, isOutput=False)
    wsmall_e = nc.declare_dram_parameter("wsmall", [128, NSMALL], F32, isOutput=False)
    wboot_e = nc.declare_dram_parameter("wboot", [128, NBOOT], F16, isOutput=False)
    wc3_e = nc.declare_dram_parameter("wc3", [128, NC3], F16, isOutput=False)
    wattn_e = nc.declare_dram_parameter("wattn", [128, NATTN], F16, isOutput=False)
    wfc_e = nc.declare_dram_parameter("wfc", [128, NFC], F16, isOutput=False)
    out_e = nc.declare_dram_parameter("out", [2, 1], F32, isOutput=True)

    with tile.TileContext(nc) as tc:
        with tc.tile_pool(name="consts", bufs=1) as cp, \
             tc.tile_pool(name="work", bufs=1) as wp, \
             tc.tile_pool(name="hpool", bufs=8) as hpool, \
             tc.tile_pool(name="ps_hm", bufs=1, space="PSUM") as ps_hm, \
             tc.tile_pool(name="ps_work", bufs=2, space="PSUM") as ps:

            # loads: small/boot/emb from SP; conv3+attn from Activation HWDGE;
            # the big FC pack last on SP.
            emb = cp.tile([DIM, LP + LM], F16, name="emb")
            nc.sync.dma_start(out=emb, in_=emb_e[:])
            wboot = cp.tile([128, NBOOT], F16, name="wboot")
            nc.sync.dma_start(out=wboot, in_=wboot_e[:])
            wsmall = cp.tile([128, NSMALL], F32, name="wsmall")
            nc.sync.dma_start(out=wsmall, in_=wsmall_e[:])
            wc3 = cp.tile([128, NC3], F16, name="wc3")
            nc.scalar.dma_start(out=wc3, in_=wc3_e[:])
            wattn = cp.tile([128, NATTN], F16, name="wattn")
            nc.scalar.dma_start(out=wattn, in_=wattn_e[:])
            wfc = cp.tile([128, NFC], F16, name="wfc")
            nc.sync.dma_start(out=wfc, in_=wfc_e[:])

            pe = emb[:, 0:LP]
            me = emb[:, LP:LP + LM]
            bias = lambda col, rows=128: wsmall[0:rows, col:col + 1]

            # early zero/NEG fills on the (idle) Vector engine
            zt = wp.tile([128, LM3], BF16, name="zt")
            nc.vector.memset(zt, 0.0)
            hp0 = wp.tile([128, JM], F32, name="hp0")
            nc.vector.memset(hp0, 0.0)
            pa1p = wp.tile([128, NP4], F32, name="pa1p")
            nc.vector.memset(pa1p, NEG)
            pc0jm = wp.tile([128, JM], F16, name="pc0jm")
            nc.vector.memset(pc0jm, 0.0)
            pc1jm = wp.tile([32, JM], F16, name="pc1jm")
            nc.vector.memset(pc1jm, 0.0)
            hm0f = wp.tile([128, MP], BF16, name="hm0f")
            nc.vector.memset(hm0f[:, LM3:MP], 0.0)
            hm1f = wp.tile([32, MP], BF16, name="hm1f")
            nc.vector.memset(hm1f[:, LM3:MP], 0.0)
            mc0 = wp.tile([128, MP], F16, name="mc0")
            nc.vector.memset(mc0[:, LM3:MP], 0.0)
            mc1 = wp.tile([32, MP], F16, name="mc1")
            nc.vector.memset(mc1[:, LM3:MP], 0.0)

            # ================= conv stacks (fp16, f32 psum) =================
            # MHC conv1: [64,1000] -> [40,997]; relu chunked so conv2 starts early
            mx1_ps = ps.tile([CONV, LM1], F32, name="mx1_ps", tag="ps")
            _conv_matmuls(nc, mx1_ps[:, 0:512], wboot[0:DIM, MW1:MW1 + K1 * CONV], me, K1, 0, CONV, 0, 512, CONV)
            _conv_matmuls(nc, mx1_ps[:, 512:LM1], wboot[0:DIM, MW1:MW1 + K1 * CONV], me, K1, 0, CONV, 512, LM1, CONV)
            # peptide conv1 fills the PE while relu1 runs
            px1_ps = ps.tile([CONV, LP1], F32, name="px1_ps", tag="ps")
            _conv_matmuls(nc, px1_ps, wboot[0:DIM, PW1:PW1 + K1 * CONV], pe, K1, 0, CONV, 0, LP1, CONV)
            mx1 = wp.tile([CONV, LM1], F16, name="mx1")
            nc.scalar.activation(out=mx1[:, 0:520], in_=mx1_ps[:, 0:520], func=AF.Relu, bias=bias(SB_MB1, CONV))
            nc.vector.tensor_scalar(out=mx1[:, 520:LM1], in0=mx1_ps[:, 520:LM1], scalar1=bias(SB_MB1, CONV),
                                    scalar2=0.0, op0=ALU.add, op1=ALU.max)
            px1 = wp.tile([CONV, LP1], F16, name="px1")
            nc.scalar.activation(out=px1, in_=px1_ps, func=AF.Relu, bias=bias(SB_PB1, CONV))

            # MHC conv2 -> [80, 992]
            mx2_ps = ps.tile([C2, LM2], F32, name="mx2_ps", tag="ps")
            _conv_matmuls(nc, mx2_ps[:, 0:512], wboot[0:CONV, MW2:MW2 + K2 * C2], mx1, K2, 0, C2, 0, 512, C2)
            _conv_matmuls(nc, mx2_ps[:, 512:LM2], wboot[0:CONV, MW2:MW2 + K2 * C2], mx1, K2, 0, C2, 512, LM2, C2)
            px2_ps = ps.tile([C2, LP2], F32, name="px2_ps", tag="ps")
            _conv_matmuls(nc, px2_ps, wboot[0:CONV, PW2:PW2 + K2 * C2], px1, K2, 0, C2, 0, LP2, C2)
            mx2 = wp.tile([C2, LM2], F16, name="mx2")
            nc.scalar.activation(out=mx2[:, 0:520], in_=mx2_ps[:, 0:520], func=AF.Relu, bias=bias(SB_MB2, C2))
            nc.vector.tensor_scalar(out=mx2[:, 520:LM2], in0=mx2_ps[:, 520:LM2], scalar1=bias(SB_MB2, C2),
                                    scalar2=0.0, op0=ALU.add, op1=ALU.max)
            px2 = wp.tile([C2, LP2], F16, name="px2")
            nc.scalar.activation(out=px2, in_=px2_ps, func=AF.Relu, bias=bias(SB_PB2, C2))

            # MHC conv3 -> [160, 985] as [128,985]+[32,985] (into MP-padded tiles)
            mc0_ps = ps.tile([128, LM3], F32, name="mc0_ps", tag="ps")
            _conv_matmuls(nc, mc0_ps[:, 0:512], wc3[0:C2, MW3:MW3 + K3 * C4], mx2, K3, 0, 128, 0, 512, C4)
            _conv_matmuls(nc, mc0_ps[:, 512:LM3], wc3[0:C2, MW3:MW3 + K3 * C4], mx2, K3, 0, 128, 512, LM3, C4)
            nc.scalar.activation(out=mc0[:, 0:512], in_=mc0_ps[:, 0:512], func=AF.Relu, bias=bias(SB_MB3A))
            nc.vector.tensor_scalar(out=mc0[:, 512:LM3], in0=mc0_ps[:, 512:LM3], scalar1=bias(SB_MB3A),
                                    scalar2=0.0, op0=ALU.add, op1=ALU.max)
            mc1_ps = ps.tile([32, LM3], F32, name="mc1_ps", tag="ps")
            _conv_matmuls(nc, mc1_ps[:, 0:512], wc3[0:C2, MW3:MW3 + K3 * C4], mx2, K3, 128, C4, 0, 512, C4)
            _conv_matmuls(nc, mc1_ps[:, 512:LM3], wc3[0:C2, MW3:MW3 + K3 * C4], mx2, K3, 128, C4, 512, LM3, C4)
            nc.scalar.activation(out=mc1[:, 0:512], in_=mc1_ps[:, 0:512], func=AF.Relu, bias=bias(SB_MB3B, 32))
            nc.vector.tensor_scalar(out=mc1[:, 512:LM3], in0=mc1_ps[:, 512:LM3], scalar1=bias(SB_MB3B, 32),
                                    scalar2=0.0, op0=ALU.add, op1=ALU.max)

            # peptide conv3 (tiles padded to 88 cols for the 4-strided views)
            pc0_ps = ps.tile([128, LP3], F32, name="pc0_ps", tag="ps")
            _conv_matmuls(nc, pc0_ps, wc3[0:C2, PW3:PW3 + K3 * C4], px2, K3, 0, 128, 0, LP3, C4)
            pc0 = wp.tile([128, 88], F16, name="pc0")
            nc.scalar.activation(out=pc0[:, 0:LP3], in_=pc0_ps, func=AF.Relu, bias=bias(SB_PB3A))
            pc1_ps = ps.tile([32, LP3], F32, name="pc1_ps", tag="ps")
            _conv_matmuls(nc, pc1_ps, wc3[0:C2, PW3:PW3 + K3 * C4], px2, K3, 128, C4, 0, LP3, C4)
            pc1 = wp.tile([32, 88], F16, name="pc1")
            nc.scalar.activation(out=pc1[:, 0:LP3], in_=pc1_ps, func=AF.Relu, bias=bias(SB_PB3B, 32))

            # j-major copies of pc for the gate (vector, strided reads)
            pc0_g = pc0.rearrange("c (g f) -> c g f", f=4)
            pc1_g = pc1.rearrange("c (g f) -> c g f", f=4)
            for j in range(4):
                ncol = NP4 if j == 0 else NP4 - 1
                nc.vector.tensor_scalar(out=pc0jm[:, j * NP4:j * NP4 + ncol],
                                        in0=pc0_g[:, 0:ncol, j], scalar1=0.0,
                                        scalar2=None, op0=ALU.add)
                nc.vector.tensor_scalar(out=pc1jm[:, j * NP4:j * NP4 + ncol],
                                        in0=pc1_g[:, 0:ncol, j], scalar1=0.0,
                                        scalar2=None, op0=ALU.add)

            # ================= attention projections =================
            # ma0[c,m] c in 0:128 (bf16 for the Activation-engine h producer)
            ma0_ps = ps.tile([128, LM3], F32, name="ma0_ps", tag="ps")
            for lo, hi in ((0, 512), (512, LM3)):
                nc.tensor.matmul(ma0_ps[:, lo:hi], wattn[0:128, WMA_A:WMA_A + 128], mc0[:, lo:hi], start=True, stop=False)
                nc.tensor.matmul(ma0_ps[:, lo:hi], wattn[0:32, WMA_B:WMA_B + 128], mc1[:, lo:hi], start=False, stop=True)
            ma0 = wp.tile([128, LM3], BF16, name="ma0")
            nc.scalar.activation(out=ma0, in_=ma0_ps, func=AF.Identity, bias=bias(SB_BMA_A))

            # ma1p: c in 128:160 replicated 4x vertically (stationary pre-replicated)
            ma1p_ps = ps.tile([128, LM3], F32, name="ma1p_ps", tag="ps")
            for lo, hi in ((0, 512), (512, LM3)):
                nc.tensor.matmul(ma1p_ps[:, lo:hi], wattn[0:128, WMA_A + 128:WMA_A + 256], mc0[:, lo:hi], start=True, stop=False)
                nc.tensor.matmul(ma1p_ps[:, lo:hi], wattn[0:32, WMA_B + 128:WMA_B + 256], mc1[:, lo:hi], start=False, stop=True)
            ma1p = wp.tile([128, LM3], BF16, name="ma1p")
            nc.vector.tensor_scalar(out=ma1p, in0=ma1p_ps, scalar1=bias(SB_BMA_R4), scalar2=None, op0=ALU.add)

            # pa0[c,p] c in 0:128 (f32, used as per-partition bias)
            pa0_ps = ps.tile([128, LP3], F32, name="pa0_ps", tag="ps")
            nc.tensor.matmul(pa0_ps, wattn[0:128, WPA_A:WPA_A + 128], pc0[:, 0:LP3], start=True, stop=False)
            nc.tensor.matmul(pa0_ps, wattn[0:32, WPA_B:WPA_B + 128], pc1[0:32, 0:LP3], start=False, stop=True)
            pa0 = wp.tile([128, LP3], F32, name="pa0")
            nc.scalar.add(pa0, pa0_ps, bias(SB_BPA_A))

            # pa1p[32j+d, g] = pa[128+d, 4g+j]: partition-offset matmuls over
            # 4-strided moving views of pc
            pa1p_ps = ps.tile([128, NP4], F32, name="pa1p_ps", tag="ps")
            for j in range(4):
                ncol = NP4 if j == 0 else NP4 - 1
                nc.tensor.matmul(pa1p_ps[32 * j:32 * j + 32, 0:ncol],
                                 wattn[0:128, WPA_A + 128:WPA_A + 160],
                                 pc0_g[:, 0:ncol, j],
                                 start=True, stop=False, skip_group_check=True,
                                 tile_position=(0, 32 * j))
                nc.tensor.matmul(pa1p_ps[32 * j:32 * j + 32, 0:ncol],
                                 wattn[0:32, WPA_B + 128:WPA_B + 160],
                                 pc1_g[:, 0:ncol, j],
                                 start=False, stop=True, skip_group_check=True,
                                 tile_position=(0, 32 * j))
                nc.scalar.add(pa1p[32 * j:32 * j + 32, 0:ncol],
                              pa1p_ps[32 * j:32 * j + 32, 0:ncol], bias(SB_BPA_B, 32))

            # ================= 4D attention reductions =================
            # hp0 columns are written in j-major order (gate is order-free)
            hp1p = wp.tile([128, NP4], F32, name="hp1p")
            hm0_ps = ps_hm.tile([128, LM3], F32, name="hm0_ps")
            hm1_ps = ps_hm.tile([32, LM3], F32, name="hm1_ps")

            # identities and gate weights stored as bf16 bit patterns in the
            # fp16 pack; bitcast views pair them with bf16 moving operands
            id128 = wattn[0:128, ID128:ID128 + 128].bitcast(BF16)
            idst = wattn[0:128, IDST:IDST + 32].bitcast(BF16)

            # greedy producer balance: assign each tile to whichever engine
            # would finish it first (measured fused costs: ACT 1199, DVE 1263)
            prod_t = {'sc': 0.0, 've': 0.0}

            def h_tile(i, src, bias_ap, acc):
                h = hpool.tile([128, LM3], BF16, tag="h", name="h")
                if prod_t['sc'] + 1199.0 <= prod_t['ve'] + 1263.0:
                    prod_t['sc'] += 1199.0
                    nc.scalar.activation(out=h, in_=src, func=AF.Relu,
                                         bias=bias_ap, accum_out=acc)
                else:
                    prod_t['ve'] += 1263.0
                    nc.vector.scalar_tensor_tensor(out=h, in0=src, scalar=bias_ap,
                                                   in1=zt, op0=ALU.add, op1=ALU.max,
                                                   accum_out=acc)
                return h

            for p in range(LP3):
                h = h_tile(p, ma0, pa0[:, p:p + 1], hp0[:, _jm(p):_jm(p) + 1])
                nc.tensor.matmul(hm0_ps[:, 0:512], id128, h[:, 0:512],
                                 start=(p == 0), stop=(p == LP3 - 1))
                nc.tensor.matmul(hm0_ps[:, 512:LM3], id128, h[:, 512:LM3],
                                 start=(p == 0), stop=(p == LP3 - 1))

            # p-side gate head start: hp0 is complete before the packed loop
            hp0f = wp.tile([128, JM], BF16, name="hp0f")
            nc.scalar.copy(hp0f, hp0)
            cl0_ps = ps.tile([128, JM], F32, name="cl0_ps", tag="ps")
            nc.tensor.matmul(cl0_ps, wattn[0:128, WCA_A:WCA_A + 128].bitcast(BF16), hp0f,
                             start=True, stop=False, skip_group_check=True)
            cl1_ps = ps.tile([32, JM], F32, name="cl1_ps", tag="ps")
            nc.tensor.matmul(cl1_ps, wattn[0:128, WCA_A + 128:WCA_A + 160].bitcast(BF16), hp0f,
                             start=True, stop=False, skip_group_check=True)

            for g in range(NP4):
                h = h_tile(LP3 + g, ma1p, pa1p[:, g:g + 1], hp1p[:, g:g + 1])
                nc.tensor.matmul(hm1_ps[:, 0:512], idst, h[:, 0:512],
                                 start=(g == 0), stop=(g == NP4 - 1))
                nc.tensor.matmul(hm1_ps[:, 512:LM3], idst, h[:, 512:LM3],
                                 start=(g == 0), stop=(g == NP4 - 1))

            # ================= peptide gate (j-major) =================
            hp1pf = wp.tile([128, NP4], BF16, name="hp1pf")
            nc.scalar.copy(hp1pf, hp1p)
            for j in range(4):
                nc.tensor.matmul(cl0_ps[:, j * NP4:(j + 1) * NP4],
                                 wattn[0:128, WCB0 + j * 128:WCB0 + (j + 1) * 128].bitcast(BF16),
                                 hp1pf, start=False, stop=(j == 3), skip_group_check=True)
                nc.tensor.matmul(cl1_ps[:, j * NP4:(j + 1) * NP4],
                                 wattn[0:128, WCB1 + j * 32:WCB1 + (j + 1) * 32].bitcast(BF16),
                                 hp1pf, start=False, stop=(j == 3), skip_group_check=True)
            catt0 = wp.tile([128, JM], F16, name="catt0")
            nc.scalar.activation(out=catt0, in_=cl0_ps, func=AF.Sigmoid, bias=bias(SB_BA_A))
            catt1 = wp.tile([32, JM], F16, name="catt1")
            nc.scalar.activation(out=catt1, in_=cl1_ps, func=AF.Sigmoid, bias=bias(SB_BA_B, 32))

            pvf = wp.tile([128, 4], F16, name="pvf")   # cols: pv0, pv1, mv0, mv1
            pg0 = wp.tile([128, JM], F16, name="pg0")
            nc.vector.scalar_tensor_tensor(out=pg0, in0=catt0, scalar=0.5, in1=pc0jm,
                                           op0=ALU.add, op1=ALU.mult)
            pg1 = wp.tile([32, JM], F16, name="pg1")
            nc.vector.scalar_tensor_tensor(out=pg1, in0=catt1, scalar=0.5, in1=pc1jm,
                                           op0=ALU.add, op1=ALU.mult)
            with nc.allow_low_precision(reason="fp16 max-pool rounds values only"):
                nc.vector.tensor_reduce(out=pvf[:, 0:1], in_=pg0, op=ALU.max, axis=AX.X)
                nc.vector.tensor_reduce(out=pvf[0:32, 1:2], in_=pg1, op=ALU.max, axis=AX.X)

            # ================= MHC gate (chunk-pipelined) =================
            for lo, hi in ((0, 512), (512, LM3)):
                nc.scalar.activation(out=hm0f[:, lo:hi], in_=hm0_ps[:, lo:hi], func=AF.Copy)
                nc.vector.tensor_scalar(out=hm1f[:, lo:hi], in0=hm1_ps[:, lo:hi],
                                        scalar1=0.0, scalar2=None, op0=ALU.add)
            ml0_ps = ps.tile([128, MP], F32, name="ml0_ps", tag="ps")
            ml1_ps = ps.tile([32, MP], F32, name="ml1_ps", tag="ps")
            matt0 = wp.tile([128, MP], F16, name="matt0")
            matt1 = wp.tile([32, MP], F16, name="matt1")
            for lo, hi in ((0, 512), (512, MP)):
                nc.tensor.matmul(ml0_ps[:, lo:hi], wattn[0:128, WM2_A:WM2_A + 128].bitcast(BF16), hm0f[:, lo:hi], start=True, stop=False)
                nc.tensor.matmul(ml0_ps[:, lo:hi], wattn[0:32, WM2_B:WM2_B + 128].bitcast(BF16), hm1f[:, lo:hi], start=False, stop=True)
                nc.scalar.activation(out=matt0[:, lo:hi], in_=ml0_ps[:, lo:hi], func=AF.Sigmoid, bias=bias(SB_BA_A))
                nc.tensor.matmul(ml1_ps[:, lo:hi], wattn[0:128, WM2_A + 128:WM2_A + 160].bitcast(BF16), hm0f[:, lo:hi], start=True, stop=False)
                nc.tensor.matmul(ml1_ps[:, lo:hi], wattn[0:32, WM2_B + 128:WM2_B + 160].bitcast(BF16), hm1f[:, lo:hi], start=False, stop=True)
                nc.scalar.activation(out=matt1[:, lo:hi], in_=ml1_ps[:, lo:hi], func=AF.Sigmoid, bias=bias(SB_BA_B, 32))

            mg0 = wp.tile([128, MP], F16, name="mg0")
            nc.vector.scalar_tensor_tensor(out=mg0, in0=matt0, scalar=0.5, in1=mc0,
                                           op0=ALU.add, op1=ALU.mult)
            mg1 = wp.tile([32, MP], F16, name="mg1")
            nc.vector.scalar_tensor_tensor(out=mg1, in0=matt1, scalar=0.5, in1=mc1,
                                           op0=ALU.add, op1=ALU.mult)
            with nc.allow_low_precision(reason="fp16 max-pool rounds values only"):
                nc.vector.tensor_reduce(out=pvf[:, 2:3], in_=mg0, op=ALU.max, axis=AX.X)
                nc.vector.tensor_reduce(out=pvf[0:32, 3:4], in_=mg1, op=ALU.max, axis=AX.X)

            # ================= FC head =================
            def lrelu(name, f_ps, bias_lo, ncols):
                fb = wp.tile([128, ncols], F32, name=name + "_b")
                nc.vector.tensor_tensor(out=fb, in0=f_ps, in1=wsmall[:, bias_lo:bias_lo + ncols], op=ALU.add)
                fs = wp.tile([128, ncols], F32, name=name + "_s")
                nc.vector.tensor_scalar(out=fs, in0=fb, scalar1=0.01, scalar2=None, op0=ALU.mult)
                fo = wp.tile([128, ncols], F16, name=name)
                nc.vector.tensor_tensor(out=fo, in0=fb, in1=fs, op=ALU.max)
                return fo

            # f1: per-column accumulation groups (one 2KB region holds all
            # columns, so groups must not interleave); within a column the
            # two 128-row stationaries go first to reduce PE config flips
            f1_ps = ps.tile([128, 8], F32, name="f1_ps", tag="ps")
            for a in range(8):
                nc.tensor.matmul(f1_ps[:, a:a + 1], wfc[0:128, W1A + a * 128:W1A + a * 128 + 128],
                                 pvf[:, 0:1], start=True, stop=False)
                nc.tensor.matmul(f1_ps[:, a:a + 1], wfc[0:128, W1A + 1024 + a * 128:W1A + 1024 + a * 128 + 128],
                                 pvf[:, 2:3], start=False, stop=False)
                nc.tensor.matmul(f1_ps[:, a:a + 1], wfc[0:32, W1B + a * 128:W1B + a * 128 + 128],
                                 pvf[0:32, 1:2], start=False, stop=False)
                nc.tensor.matmul(f1_ps[:, a:a + 1], wfc[0:32, W1B + 1024 + a * 128:W1B + 1024 + a * 128 + 128],
                                 pvf[0:32, 3:4], start=False, stop=True)
            f1 = lrelu("f1", f1_ps, SB_B1, 8)

            f2_ps = ps.tile([128, 8], F32, name="f2_ps", tag="ps")
            for a in range(8):
                for jb in range(8):
                    nc.tensor.matmul(f2_ps[:, a:a + 1],
                                     wfc[0:128, W2C + jb * 1024 + a * 128:W2C + jb * 1024 + a * 128 + 128],
                                     f1[:, jb:jb + 1], start=(jb == 0), stop=(jb == 7))
            f2 = lrelu("f2", f2_ps, SB_B2, 8)

            f3_ps = ps.tile([128, 4], F32, name="f3_ps", tag="ps")
            for a in range(4):
                for jb in range(8):
                    nc.tensor.matmul(f3_ps[:, a:a + 1],
                                     wfc[0:128, W3C + jb * 512 + a * 128:W3C + jb * 512 + a * 128 + 128],
                                     f2[:, jb:jb + 1], start=(jb == 0), stop=(jb == 7))
            f3 = lrelu("f3", f3_ps, SB_B3, 4)

            o_ps = ps.tile([2, 1], F32, name="o_ps", tag="ps")
            for c in range(4):
                nc.tensor.matmul(o_ps, wfc[0:128, WOC + 2 * c:WOC + 2 * c + 2], f3[:, c:c + 1],
                                 start=(c == 0), stop=(c == 3))
            o_sb = wp.tile([2, 1], F32, name="o_sb")
            nc.vector.tensor_tensor(out=o_sb, in0=o_ps, in1=wsmall[0:2, SB_BO:SB_BO + 1], op=ALU.add)
            nc.sync.dma_start(out=out_e[:], in_=o_sb)

    _split_excess_waits(nc, max_waits=1)
    return nc


_PROGRAM = None


def _get_program():
    global _PROGRAM
    if _PROGRAM is None:
        _PROGRAM = _build_program()
    return _PROGRAM


def _prep_weights(inp):
    """Host-side packing shared by all cores."""
    import ml_dtypes
    f16 = np.float16
    bf16 = ml_dtypes.bfloat16
    f32 = lambda x: np.asarray(x, dtype=np.float32)
    as_f16bits = lambda a: np.ascontiguousarray(a).view(np.uint16).view(f16)

    def convw(w):  # [co, ci, k] -> [ci, k*co] fp16
        w = np.asarray(w, dtype=np.float32)
        ci = w.shape[1]
        return w.transpose(1, 2, 0).reshape(ci, -1).astype(f16)

    wboot = np.zeros((128, NBOOT), f16)
    wboot[0:DIM, PW1:PW1 + K1 * CONV] = convw(inp['pw1'])
    wboot[0:DIM, MW1:MW1 + K1 * CONV] = convw(inp['mw1'])
    wboot[0:CONV, PW2:PW2 + K2 * C2] = convw(inp['pw2'])
    wboot[0:CONV, MW2:MW2 + K2 * C2] = convw(inp['mw2'])

    wc3 = np.zeros((128, NC3), f16)
    wc3[0:C2, PW3:PW3 + K3 * C4] = convw(inp['pw3'])
    wc3[0:C2, MW3:MW3 + K3 * C4] = convw(inp['mw3'])

    wpa, wma = f32(inp['Wpa']), f32(inp['Wma'])
    wca = f32(inp['Wa']) / float(LM3)
    wm2 = f32(inp['Wa']) / float(LP3)
    wattn = np.zeros((128, NATTN), f16)
    wattn[0:128, WPA_A:WPA_A + 160] = wpa[0:128].astype(f16)
    wattn[0:32, WPA_B:WPA_B + 160] = wpa[128:160].astype(f16)
    wattn[0:128, WMA_A:WMA_A + 128] = wma[0:128, 0:128].astype(f16)
    wattn[0:128, WMA_A + 128:WMA_A + 256] = np.tile(wma[0:128, 128:160], (1, 4)).astype(f16)
    wattn[0:32, WMA_B:WMA_B + 128] = wma[128:160, 0:128].astype(f16)
    wattn[0:32, WMA_B + 128:WMA_B + 256] = np.tile(wma[128:160, 128:160], (1, 4)).astype(f16)
    wattn[0:128, WCA_A:WCA_A + 160] = as_f16bits(wca[0:128].astype(bf16))
    wattn[0:32, WCA_B:WCA_B + 160] = as_f16bits(wca[128:160].astype(bf16))
    wattn[0:128, WM2_A:WM2_A + 160] = as_f16bits(wm2[0:128].astype(bf16))
    wattn[0:32, WM2_B:WM2_B + 160] = as_f16bits(wm2[128:160].astype(bf16))
    id128 = np.eye(128, dtype=bf16)
    idst = np.tile(np.eye(32, dtype=bf16), (4, 1))
    wattn[0:128, ID128:ID128 + 128] = as_f16bits(id128)
    wattn[0:128, IDST:IDST + 32] = as_f16bits(idst)
    for j in range(4):
        wattn[32 * j:32 * j + 32, WCB0 + j * 128:WCB0 + (j + 1) * 128] = as_f16bits(wca[128:160, 0:128].astype(bf16))
        wattn[32 * j:32 * j + 32, WCB1 + j * 32:WCB1 + (j + 1) * 32] = as_f16bits(wca[128:160, 128:160].astype(bf16))

    w1 = f32(inp['W1'])
    wfc = np.zeros((128, NFC), f16)
    wfc[0:128, W1A:W1A + 2048] = np.concatenate([w1[0:128], w1[160:288]], axis=1).astype(f16)
    wfc[0:32, W1B:W1B + 2048] = np.concatenate([w1[128:160], w1[288:320]], axis=1).astype(f16)

    def fcw(w, nblk):  # [I, J], I = nblk*128 -> [128, nblk*J]
        w = np.asarray(w, dtype=np.float32)
        i, j = w.shape
        return w.reshape(nblk, 128, j).transpose(1, 0, 2).reshape(128, nblk * j).astype(f16)

    wfc[0:128, W2C:W2C + 8192] = fcw(inp['W2'], 8)
    wfc[0:128, W3C:W3C + 4096] = fcw(inp['W3'], 8)
    wfc[0:128, WOC:WOC + 8] = fcw(inp['Wo'], 4)

    wsmall = np.zeros((128, NSMALL), np.float32)
    def bias2(col_a, col_b, b):
        b = f32(b)
        wsmall[0:128, col_a] = b[0:128]
        wsmall[0:32, col_b] = b[128:160]
    wsmall[0:CONV, SB_PB1] = f32(inp['pb1'])
    wsmall[0:C2, SB_PB2] = f32(inp['pb2'])
    bias2(SB_PB3A, SB_PB3B, inp['pb3'])
    wsmall[0:CONV, SB_MB1] = f32(inp['mb1'])
    wsmall[0:C2, SB_MB2] = f32(inp['mb2'])
    bias2(SB_MB3A, SB_MB3B, inp['mb3'])
    bias2(SB_BPA_A, SB_BPA_B, inp['bpa'])
    wsmall[0:128, SB_BMA_A] = f32(inp['bma'])[0:128]
    wsmall[0:128, SB_BMA_R4] = np.tile(f32(inp['bma'])[128:160], 4)
    bias2(SB_BA_A, SB_BA_B, inp['ba'])
    wsmall[0:128, SB_B1:SB_B1 + 8] = f32(inp['b1']).reshape(8, 128).T
    wsmall[0:128, SB_B2:SB_B2 + 8] = f32(inp['b2']).reshape(8, 128).T
    wsmall[0:128, SB_B3:SB_B3 + 4] = f32(inp['b3']).reshape(4, 128).T
    wsmall[0:2, SB_BO] = f32(inp['bo'])

    return {'wboot': wboot, 'wc3': wc3, 'wattn': wattn, 'wfc': wfc, 'wsmall': wsmall}


def _prep_core(inp, b):
    """Per-core embedding gather: [64, 1100] fp16."""
    pep = np.asarray(inp['peptide'])[b]
    mhc = np.asarray(inp['MHC'])[b]
    import ml_dtypes
    pe = np.asarray(inp['pep_emb'], np.float32)[pep].T   # [64, 100]
    me = np.asarray(inp['mhc_emb'], np.float32)[mhc].T   # [64, 1000]
    return np.concatenate([pe, me], axis=1).astype(np.float16)


def kernel(**inputs):
    nc = _get_program()
    shared = _prep_weights(inputs)
    in_maps = []
    for b in range(B):
        m = dict(shared)
        m['emb'] = _prep_core(inputs, b)
        in_maps.append(m)
    res = run_bass_kernel_spmd(nc, in_maps, core_ids=list(range(B)))
    return np.stack([np.asarray(res.results[i]['out']).reshape(2) for i in range(B)]).astype(np.float32)


# revision 19
# speedup vs baseline: 2.6776x; 1.0278x over previous
"""AttentionDTI forward pass on 8 Trainium2 NeuronCores (pure data parallel).

One batch element per core, weights replicated. All matmul operands are
16-bit (f32 PSUM accumulation): fp16 for conv/FC weights and activations,
bf16 for the attention tiles (the Activation engine runs ~1.3x slower on
fp16 than bf16, and the attention path tolerates bf16). Embedding lookup
is done host-side; weights arrive in packed DMAs issued from both the SP
and Activation HWDGE queues so transfers overlap the NEFF prologue.

The 4D additive-attention tensor h[b,p,m,c] = relu(pa + ma) is never
materialized: mean_m(h @ Wa) == mean_m(h) @ Wa, so only hp[c,p] = sum_m h
and hm[c,m] = sum_p h are accumulated on the fly. hm accumulates in PSUM
by streaming h tiles through the PE against a stationary identity; hp
comes from fused per-tile accumulators: the Scalar engine's
relu+bias+accum activation alternates 1:1 with the Vector engine's
scalar_tensor_tensor (relu via max-with-zeros + sum accum) — any DVE op
with an accum output runs at the 1x element rate, so the fused form is
optimal on both engines.

c-channels [128:160] run packed: ma rows replicated 4x vertically (via
host-replicated stationary columns, free) so each tile covers 4 peptide
positions; a 4-stacked [128,32] identity reduces them into hm1. The
peptide gate keeps its column axis in "j-major" order (jm(p) =
(p%4)*22 + p//4) end-to-end — max-pool over p is order-invariant — which
makes the packed hp1p contributions contiguous matmuls and avoids any
unpack DMAs.

Environment constraints discovered empirically (this axon terminal):
  - GPSIMD/Pool compute ops fail codegen; SWDGE DMA hangs: DMAs go
    through SP/Activation HWDGE only.
  - tensor_tensor_reduce fails walrus codegen ("ISA wrong length").
  - tensor_scalar's accum_out hijacks op1 as the reduce op (no fused
    two-op elementwise + sum) — scalar_tensor_tensor does fuse it.
  - walrus allows at most ONE semaphore wait per instruction:
    _split_excess_waits() rewrites the scheduled program.
"""
import sys

_BASS_ROOT = '/opt/trn_rl_repo'
if _BASS_ROOT not in sys.path:
    sys.path.insert(0, _BASS_ROOT)

import numpy as np

import concourse.bass as bass
import concourse.tile as tile
from concourse import mybir
from concourse.bass_utils import run_bass_kernel_spmd

F32 = mybir.dt.float32
F16 = mybir.dt.float16
BF16 = mybir.dt.bfloat16
ALU = mybir.AluOpType
AF = mybir.ActivationFunctionType
AX = mybir.AxisListType

B = 8
LP, LM, DIM, CONV = 100, 1000, 64, 40
C2, C4 = CONV * 2, CONV * 4          # 80, 160
K1, K2, K3 = 4, 6, 8
LP1, LP2, LP3 = 97, 92, 85           # peptide conv output lengths
LM1, LM2, LM3 = 997, 992, 985        # MHC conv output lengths
NP4 = 22                             # ceil(85/4) packed p-groups
JM = 4 * NP4                         # 88 j-major gate columns
MP = 992                             # LM3 padded for 4x-eligible DVE gate ops
NEG = -30000.0
SPLIT_MOD = 2                        # i % SPLIT_MOD == 0 -> scalar h tile

# ---- wboot column map (fp16 [128, 1280]): conv1+conv2 weights ----
PW1, MW1, PW2, MW2 = 0, 160, 320, 800
NBOOT = 1280
# ---- wc3 column map (fp16 [128, 2560]): conv3 weights ----
PW3, MW3 = 0, 1280
NC3 = 2560
# ---- wattn column map (fp16 [128, 1632]) ----
WPA_A, WPA_B = 0, 160        # [128,160], [32,160]
WMA_A, WMA_B = 320, 576      # [128,256], [32,256] (cols 128:256 = rep4 of Wma[:,128:160])
WCA_A, WCA_B = 832, 992      # Wa/LM3: [128,160], [32,160]
WM2_A, WM2_B = 1152, 1312    # Wa/LP3
ID128, IDST = 1472, 1600     # bf16 identities: [128,128], [128,32]
WCB0, WCB1 = 1632, 2144      # j-lifted Wa[128:160]/LM3: 4x[128,128], 4x[128,32]
NATTN = 2272
# ---- wfc column map (fp16 [128, 16392]) ----
W1A, W1B = 0, 2048           # [128, 2048], [32, 2048]
W2C, W3C, WOC = 4096, 12288, 16384
NFC = 16392
# ---- wsmall column map (f32 [128, 35]) ----
SB_PB1, SB_PB2, SB_PB3A, SB_PB3B = 0, 1, 2, 3
SB_MB1, SB_MB2, SB_MB3A, SB_MB3B = 4, 5, 6, 7
SB_BPA_A, SB_BPA_B = 8, 9
SB_BMA_A, SB_BMA_R4 = 10, 11
SB_BA_A, SB_BA_B = 12, 13
SB_B1, SB_B2, SB_B3, SB_BO = 14, 22, 30, 34
NSMALL = 35


def _jm(p):
    return (p % 4) * NP4 + (p // 4)


_ctr = [0]


def _split_excess_waits(nc, max_waits=1):
    n_split = 0
    for f in nc.m.functions:
        for b in f.blocks:
            insts = list(b.instructions)
            out = []
            changed = False
            for inst in insts:
                si = inst.sync_info
                waits = list(si.on_wait) if (si is not None and si.on_wait) else []
                if len(waits) > max_waits:
                    changed = True
                    n_split += 1
                    keep = max(1, max_waits)
                    head, tail = waits[:-keep], waits[-keep:]
                    for i in range(0, len(head), keep):
                        chunk = head[i:i + keep]
                        nop = mybir.InstEventSemaphore(
                            name=f"ant-wait-split-{_ctr[0]}", ins=[], outs=[])
                        _ctr[0] += 1
                        nop.engine = inst.engine
                        nop.sync_info = mybir.SyncInfo(on_wait=chunk, on_update=[])
                        nc.register_instruction(nop)
                        out.append(nop)
                    upd = list(si.on_update) if si.on_update else []
                    inst.sync_info = mybir.SyncInfo(on_wait=tail, on_update=upd)
                out.append(inst)
            if changed:
                b.instructions = out
    return n_split


def _conv_matmuls(nc, psum, wtile, x, k_taps, co_lo, co_hi, m_lo, m_hi, cout_stride):
    """Valid 1-D conv as k shifted matmuls accumulated into `psum`."""
    for k in range(k_taps):
        nc.tensor.matmul(
            psum,
            wtile[:, k * cout_stride + co_lo: k * cout_stride + co_hi],
            x[:, m_lo + k: m_hi + k],
            start=(k == 0), stop=(k == k_taps - 1))


def _build_program():
    nc = bass.Bass("TRN2", target_bir_lowering=False, debug=False)

    emb_e = nc.declare_dram_parameter("emb", [DIM, LP + LM], F16, isOutput=False)
    wsmall_e = nc.declare_dram_parameter("wsmall", [128, NSMALL], F32, isOutput=False)
    wboot_e = nc.declare_dram_parameter("wboot", [128, NBOOT], F16, isOutput=False)
    wc3_e = nc.declare_dram_parameter("wc3", [128, NC3], F16, isOutput=False)
    wattn_e = nc.declare_dram_parameter("wattn", [128, NATTN], F16, isOutput=False)
    wfc_e = nc.declare_dram_parameter("wfc", [128, NFC], F16, isOutput=False)
    out_e = nc.declare_dram_parameter("out", [2, 1], F32, isOutput=True)

    with tile.TileContext(nc) as tc:
        with tc.tile_pool(name="consts", bufs=1) as cp, \
             tc.tile_pool(name="work", bufs=1) as wp, \
             tc.tile_pool(name="hpool", bufs=8) as hpool, \
             tc.tile_pool(name="ps_hm", bufs=1, space="PSUM") as ps_hm, \
             tc.tile_pool(name="ps_work", bufs=2, space="PSUM") as ps:

            # loads: small/boot/emb from SP; conv3+attn from Activation HWDGE;
            # the big FC pack last on SP.
            emb = cp.tile([DIM, LP + LM], F16, name="emb")
            nc.sync.dma_start(out=emb, in_=emb_e[:])
            wboot = cp.tile([128, NBOOT], F16, name="wboot")
            nc.scalar.dma_start(out=wboot, in_=wboot_e[:])
            wsmall = cp.tile([128, NSMALL], F32, name="wsmall")
            nc.sync.dma_start(out=wsmall, in_=wsmall_e[:])
            wc3 = cp.tile([128, NC3], F16, name="wc3")
            nc.scalar.dma_start(out=wc3, in_=wc3_e[:])
            wattn = cp.tile([128, NATTN], F16, name="wattn")
            nc.sync.dma_start(out=wattn, in_=wattn_e[:])
            wfc = cp.tile([128, NFC], F16, name="wfc")
            nc.sync.dma_start(out=wfc, in_=wfc_e[:])

            pe = emb[:, 0:LP]
            me = emb[:, LP:LP + LM]
            bias = lambda col, rows=128: wsmall[0:rows, col:col + 1]

            # early zero/NEG fills on the (idle) Vector engine
            zt = wp.tile([128, LM3], BF16, name="zt")
            nc.vector.memset(zt, 0.0)
            # warm the activation table (one-time ~1.3us load) off the
            # critical path, before any DMA lands
            atl = wp.tile([1, 1], F16, name="atl")
            nc.scalar.activation(out=atl, in_=zt[0:1, 0:1], func=AF.Relu, bias=0.0)
            hp0 = wp.tile([128, JM], F32, name="hp0")
            nc.vector.memset(hp0, 0.0)
            pa1p = wp.tile([128, NP4], F32, name="pa1p")
            nc.vector.memset(pa1p, NEG)
            pc0jm = wp.tile([128, JM], F16, name="pc0jm")
            nc.vector.memset(pc0jm, 0.0)
            pc1jm = wp.tile([32, JM], F16, name="pc1jm")
            nc.vector.memset(pc1jm, 0.0)
            hm0f = wp.tile([128, MP], BF16, name="hm0f")
            nc.vector.memset(hm0f[:, LM3:MP], 0.0)
            hm1f = wp.tile([32, MP], BF16, name="hm1f")
            nc.vector.memset(hm1f[:, LM3:MP], 0.0)
            mc0 = wp.tile([128, MP], F16, name="mc0")
            nc.vector.memset(mc0[:, LM3:MP], 0.0)
            mc1 = wp.tile([32, MP], F16, name="mc1")
            nc.vector.memset(mc1[:, LM3:MP], 0.0)

            # ================= conv stacks (fp16, f32 psum) =================
            # MHC conv1: [64,1000] -> [40,997]; relu chunked so conv2 starts early
            mx1_ps = ps.tile([CONV, LM1], F32, name="mx1_ps", tag="ps")
            _conv_matmuls(nc, mx1_ps[:, 0:512], wboot[0:DIM, MW1:MW1 + K1 * CONV], me, K1, 0, CONV, 0, 512, CONV)
            _conv_matmuls(nc, mx1_ps[:, 512:LM1], wboot[0:DIM, MW1:MW1 + K1 * CONV], me, K1, 0, CONV, 512, LM1, CONV)
            # peptide conv1 fills the PE while relu1 runs
            px1_ps = ps.tile([CONV, LP1], F32, name="px1_ps", tag="ps")
            _conv_matmuls(nc, px1_ps, wboot[0:DIM, PW1:PW1 + K1 * CONV], pe, K1, 0, CONV, 0, LP1, CONV)
            mx1 = wp.tile([CONV, LM1], F16, name="mx1")
            nc.scalar.activation(out=mx1[:, 0:520], in_=mx1_ps[:, 0:520], func=AF.Relu, bias=bias(SB_MB1, CONV))
            nc.vector.tensor_scalar(out=mx1[:, 520:LM1], in0=mx1_ps[:, 520:LM1], scalar1=bias(SB_MB1, CONV),
                                    scalar2=0.0, op0=ALU.add, op1=ALU.max)
            px1 = wp.tile([CONV, LP1], F16, name="px1")
            nc.scalar.activation(out=px1, in_=px1_ps, func=AF.Relu, bias=bias(SB_PB1, CONV))

            # MHC conv2 -> [80, 992]
            mx2_ps = ps.tile([C2, LM2], F32, name="mx2_ps", tag="ps")
            _conv_matmuls(nc, mx2_ps[:, 0:512], wboot[0:CONV, MW2:MW2 + K2 * C2], mx1, K2, 0, C2, 0, 512, C2)
            _conv_matmuls(nc, mx2_ps[:, 512:LM2], wboot[0:CONV, MW2:MW2 + K2 * C2], mx1, K2, 0, C2, 512, LM2, C2)
            px2_ps = ps.tile([C2, LP2], F32, name="px2_ps", tag="ps")
            _conv_matmuls(nc, px2_ps, wboot[0:CONV, PW2:PW2 + K2 * C2], px1, K2, 0, C2, 0, LP2, C2)
            mx2 = wp.tile([C2, LM2], F16, name="mx2")
            nc.scalar.activation(out=mx2[:, 0:520], in_=mx2_ps[:, 0:520], func=AF.Relu, bias=bias(SB_MB2, C2))
            nc.vector.tensor_scalar(out=mx2[:, 520:LM2], in0=mx2_ps[:, 520:LM2], scalar1=bias(SB_MB2, C2),
                                    scalar2=0.0, op0=ALU.add, op1=ALU.max)
            px2 = wp.tile([C2, LP2], F16, name="px2")
            nc.scalar.activation(out=px2, in_=px2_ps, func=AF.Relu, bias=bias(SB_PB2, C2))

            # MHC conv3 -> [160, 985] as [128,985]+[32,985] (into MP-padded tiles)
            mc0_ps = ps.tile([128, LM3], F32, name="mc0_ps", tag="ps")
            _conv_matmuls(nc, mc0_ps[:, 0:512], wc3[0:C2, MW3:MW3 + K3 * C4], mx2, K3, 0, 128, 0, 512, C4)
            _conv_matmuls(nc, mc0_ps[:, 512:LM3], wc3[0:C2, MW3:MW3 + K3 * C4], mx2, K3, 0, 128, 512, LM3, C4)
            nc.scalar.activation(out=mc0[:, 0:512], in_=mc0_ps[:, 0:512], func=AF.Relu, bias=bias(SB_MB3A))
            nc.vector.tensor_scalar(out=mc0[:, 512:LM3], in0=mc0_ps[:, 512:LM3], scalar1=bias(SB_MB3A),
                                    scalar2=0.0, op0=ALU.add, op1=ALU.max)
            mc1_ps = ps.tile([32, LM3], F32, name="mc1_ps", tag="ps")
            _conv_matmuls(nc, mc1_ps[:, 0:512], wc3[0:C2, MW3:MW3 + K3 * C4], mx2, K3, 128, C4, 0, 512, C4)
            _conv_matmuls(nc, mc1_ps[:, 512:LM3], wc3[0:C2, MW3:MW3 + K3 * C4], mx2, K3, 128, C4, 512, LM3, C4)
            nc.scalar.activation(out=mc1[:, 0:512], in_=mc1_ps[:, 0:512], func=AF.Relu, bias=bias(SB_MB3B, 32))
            nc.vector.tensor_scalar(out=mc1[:, 512:LM3], in0=mc1_ps[:, 512:LM3], scalar1=bias(SB_MB3B, 32),
                                    scalar2=0.0, op0=ALU.add, op1=ALU.max)

            # peptide conv3 (tiles padded to 88 cols for the 4-strided views)
            pc0_ps = ps.tile([128, LP3], F32, name="pc0_ps", tag="ps")
            _conv_matmuls(nc, pc0_ps, wc3[0:C2, PW3:PW3 + K3 * C4], px2, K3, 0, 128, 0, LP3, C4)
            pc0 = wp.tile([128, 88], F16, name="pc0")
            nc.scalar.activation(out=pc0[:, 0:LP3], in_=pc0_ps, func=AF.Relu, bias=bias(SB_PB3A))
            pc1_ps = ps.tile([32, LP3], F32, name="pc1_ps", tag="ps")
            _conv_matmuls(nc, pc1_ps, wc3[0:C2, PW3:PW3 + K3 * C4], px2, K3, 128, C4, 0, LP3, C4)
            pc1 = wp.tile([32, 88], F16, name="pc1")
            nc.scalar.activation(out=pc1[:, 0:LP3], in_=pc1_ps, func=AF.Relu, bias=bias(SB_PB3B, 32))

            # j-major copies of pc for the gate (vector, strided reads)
            pc0_g = pc0.rearrange("c (g f) -> c g f", f=4)
            pc1_g = pc1.rearrange("c (g f) -> c g f", f=4)
            for j in range(4):
                ncol = NP4 if j == 0 else NP4 - 1
                nc.vector.tensor_scalar(out=pc0jm[:, j * NP4:j * NP4 + ncol],
                                        in0=pc0_g[:, 0:ncol, j], scalar1=0.0,
                                        scalar2=None, op0=ALU.add)
                nc.vector.tensor_scalar(out=pc1jm[:, j * NP4:j * NP4 + ncol],
                                        in0=pc1_g[:, 0:ncol, j], scalar1=0.0,
                                        scalar2=None, op0=ALU.add)

            # ================= attention projections =================
            # ma0[c,m] c in 0:128 (bf16 for the Activation-engine h producer)
            ma0_ps = ps.tile([128, LM3], F32, name="ma0_ps", tag="ps")
            for lo, hi in ((0, 512), (512, LM3)):
                nc.tensor.matmul(ma0_ps[:, lo:hi], wattn[0:128, WMA_A:WMA_A + 128], mc0[:, lo:hi], start=True, stop=False)
                nc.tensor.matmul(ma0_ps[:, lo:hi], wattn[0:32, WMA_B:WMA_B + 128], mc1[:, lo:hi], start=False, stop=True)
            ma0 = wp.tile([128, LM3], BF16, name="ma0")
            nc.scalar.activation(out=ma0, in_=ma0_ps, func=AF.Identity, bias=bias(SB_BMA_A))

            # ma1p: c in 128:160 replicated 4x vertically (stationary pre-replicated)
            ma1p_ps = ps.tile([128, LM3], F32, name="ma1p_ps", tag="ps")
            for lo, hi in ((0, 512), (512, LM3)):
                nc.tensor.matmul(ma1p_ps[:, lo:hi], wattn[0:128, WMA_A + 128:WMA_A + 256], mc0[:, lo:hi], start=True, stop=False)
                nc.tensor.matmul(ma1p_ps[:, lo:hi], wattn[0:32, WMA_B + 128:WMA_B + 256], mc1[:, lo:hi], start=False, stop=True)
            ma1p = wp.tile([128, LM3], BF16, name="ma1p")
            nc.vector.tensor_scalar(out=ma1p, in0=ma1p_ps, scalar1=bias(SB_BMA_R4), scalar2=None, op0=ALU.add)

            # pa0[c,p] c in 0:128 (f32, used as per-partition bias)
            pa0_ps = ps.tile([128, LP3], F32, name="pa0_ps", tag="ps")
            nc.tensor.matmul(pa0_ps, wattn[0:128, WPA_A:WPA_A + 128], pc0[:, 0:LP3], start=True, stop=False)
            nc.tensor.matmul(pa0_ps, wattn[0:32, WPA_B:WPA_B + 128], pc1[0:32, 0:LP3], start=False, stop=True)
            pa0 = wp.tile([128, LP3], F32, name="pa0")
            nc.scalar.add(pa0, pa0_ps, bias(SB_BPA_A))

            # pa1p[32j+d, g] = pa[128+d, 4g+j]: partition-offset matmuls over
            # 4-strided moving views of pc
            pa1p_ps = ps.tile([128, NP4], F32, name="pa1p_ps", tag="ps")
            for j in range(4):
                ncol = NP4 if j == 0 else NP4 - 1
                nc.tensor.matmul(pa1p_ps[32 * j:32 * j + 32, 0:ncol],
                                 wattn[0:128, WPA_A + 128:WPA_A + 160],
                                 pc0_g[:, 0:ncol, j],
                                 start=True, stop=False, skip_group_check=True,
                                 tile_position=(0, 32 * j))
                nc.tensor.matmul(pa1p_ps[32 * j:32 * j + 32, 0:ncol],
                                 wattn[0:32, WPA_B + 128:WPA_B + 160],
                                 pc1_g[:, 0:ncol, j],
                                 start=False, stop=True, skip_group_check=True,
                                 tile_position=(0, 32 * j))
                nc.scalar.add(pa1p[32 * j:32 * j + 32, 0:ncol],
                              pa1p_ps[32 * j:32 * j + 32, 0:ncol], bias(SB_BPA_B, 32))

            # ================= 4D attention reductions =================
            # hp0 columns are written in j-major order (gate is order-free)
            hp1p = wp.tile([128, NP4], F32, name="hp1p")
            hm0_ps = ps_hm.tile([128, LM3], F32, name="hm0_ps")
            hm1_ps = ps_hm.tile([32, LM3], F32, name="hm1_ps")

            # identities and gate weights stored as bf16 bit patterns in the
            # fp16 pack; bitcast views pair them with bf16 moving operands
            id128 = wattn[0:128, ID128:ID128 + 128].bitcast(BF16)
            idst = wattn[0:128, IDST:IDST + 32].bitcast(BF16)

            # greedy producer balance: assign each tile to whichever engine
            # would finish it first (measured fused costs: ACT 1199, DVE 1263)
            prod_t = {'sc': 0.0, 've': 0.0}

            def h_tile(i, src, bias_ap, acc):
                h = hpool.tile([128, LM3], BF16, tag="h", name="h")
                if prod_t['sc'] + 1199.0 <= prod_t['ve'] + 1263.0:
                    prod_t['sc'] += 1199.0
                    nc.scalar.activation(out=h, in_=src, func=AF.Relu,
                                         bias=bias_ap, accum_out=acc)
                else:
                    prod_t['ve'] += 1263.0
                    nc.vector.scalar_tensor_tensor(out=h, in0=src, scalar=bias_ap,
                                                   in1=zt, op0=ALU.add, op1=ALU.max,
                                                   accum_out=acc)
                return h

            for p in range(LP3):
                h = h_tile(p, ma0, pa0[:, p:p + 1], hp0[:, _jm(p):_jm(p) + 1])
                nc.tensor.matmul(hm0_ps[:, 0:512], id128, h[:, 0:512],
                                 start=(p == 0), stop=(p == LP3 - 1))
                nc.tensor.matmul(hm0_ps[:, 512:LM3], id128, h[:, 512:LM3],
                                 start=(p == 0), stop=(p == LP3 - 1))

            # p-side gate head start: hp0 is complete before the packed loop
            hp0f = wp.tile([128, JM], BF16, name="hp0f")
            nc.scalar.copy(hp0f, hp0)
            cl0_ps = ps.tile([128, JM], F32, name="cl0_ps", tag="ps")
            nc.tensor.matmul(cl0_ps, wattn[0:128, WCA_A:WCA_A + 128].bitcast(BF16), hp0f,
                             start=True, stop=False, skip_group_check=True)
            cl1_ps = ps.tile([32, JM], F32, name="cl1_ps", tag="ps")
            nc.tensor.matmul(cl1_ps, wattn[0:128, WCA_A + 128:WCA_A + 160].bitcast(BF16), hp0f,
                             start=True, stop=False, skip_group_check=True)

            for g in range(NP4):
                h = h_tile(LP3 + g, ma1p, pa1p[:, g:g + 1], hp1p[:, g:g + 1])
                nc.tensor.matmul(hm1_ps[:, 0:512], idst, h[:, 0:512],
                                 start=(g == 0), stop=(g == NP4 - 1))
                nc.tensor.matmul(hm1_ps[:, 512:LM3], idst, h[:, 512:LM3],
                                 start=(g == 0), stop=(g == NP4 - 1))

            # ================= peptide gate (j-major) =================
            hp1pf = wp.tile([128, NP4], BF16, name="hp1pf")
            nc.scalar.copy(hp1pf, hp1p)
            for j in range(4):
                nc.tensor.matmul(cl0_ps[:, j * NP4:(j + 1) * NP4],
                                 wattn[0:128, WCB0 + j * 128:WCB0 + (j + 1) * 128].bitcast(BF16),
                                 hp1pf, start=False, stop=(j == 3), skip_group_check=True)
                nc.tensor.matmul(cl1_ps[:, j * NP4:(j + 1) * NP4],
                                 wattn[0:128, WCB1 + j * 32:WCB1 + (j + 1) * 32].bitcast(BF16),
                                 hp1pf, start=False, stop=(j == 3), skip_group_check=True)
            catt0 = wp.tile([128, JM], F16, name="catt0")
            nc.scalar.activation(out=catt0, in_=cl0_ps, func=AF.Sigmoid, bias=bias(SB_BA_A))
            catt1 = wp.tile([32, JM], F16, name="catt1")
            nc.scalar.activation(out=catt1, in_=cl1_ps, func=AF.Sigmoid, bias=bias(SB_BA_B, 32))

            pvf = wp.tile([128, 4], F16, name="pvf")   # cols: pv0, pv1, mv0, mv1
            pg0 = wp.tile([128, JM], F16, name="pg0")
            nc.vector.scalar_tensor_tensor(out=pg0, in0=catt0, scalar=0.5, in1=pc0jm,
                                           op0=ALU.add, op1=ALU.mult)
            pg1 = wp.tile([32, JM], F16, name="pg1")
            nc.vector.scalar_tensor_tensor(out=pg1, in0=catt1, scalar=0.5, in1=pc1jm,
                                           op0=ALU.add, op1=ALU.mult)
            with nc.allow_low_precision(reason="fp16 max-pool rounds values only"):
                nc.vector.tensor_reduce(out=pvf[:, 0:1], in_=pg0, op=ALU.max, axis=AX.X)
                nc.vector.tensor_reduce(out=pvf[0:32, 1:2], in_=pg1, op=ALU.max, axis=AX.X)

            # ================= MHC gate (chunk-pipelined) =================
            for lo, hi in ((0, 512), (512, LM3)):
                nc.scalar.activation(out=hm0f[:, lo:hi], in_=hm0_ps[:, lo:hi], func=AF.Copy)
                nc.vector.tensor_scalar(out=hm1f[:, lo:hi], in0=hm1_ps[:, lo:hi],
                                        scalar1=0.0, scalar2=None, op0=ALU.add)
            ml0_ps = ps.tile([128, MP], F32, name="ml0_ps", tag="ps")
            ml1_ps = ps.tile([32, MP], F32, name="ml1_ps", tag="ps")
            matt0 = wp.tile([128, MP], F16, name="matt0")
            matt1 = wp.tile([32, MP], F16, name="matt1")
            for lo, hi in ((0, 512), (512, MP)):
                nc.tensor.matmul(ml0_ps[:, lo:hi], wattn[0:128, WM2_A:WM2_A + 128].bitcast(BF16), hm0f[:, lo:hi], start=True, stop=False)
                nc.tensor.matmul(ml0_ps[:, lo:hi], wattn[0:32, WM2_B:WM2_B + 128].bitcast(BF16), hm1f[:, lo:hi], start=False, stop=True)
                nc.scalar.activation(out=matt0[:, lo:hi], in_=ml0_ps[:, lo:hi], func=AF.Sigmoid, bias=bias(SB_BA_A))
                nc.tensor.matmul(ml1_ps[:, lo:hi], wattn[0:128, WM2_A + 128:WM2_A + 160].bitcast(BF16), hm0f[:, lo:hi], start=True, stop=False)
                nc.tensor.matmul(ml1_ps[:, lo:hi], wattn[0:32, WM2_B + 128:WM2_B + 160].bitcast(BF16), hm1f[:, lo:hi], start=False, stop=True)
                nc.scalar.activation(out=matt1[:, lo:hi], in_=ml1_ps[:, lo:hi], func=AF.Sigmoid, bias=bias(SB_BA_B, 32))

            mg0 = wp.tile([128, MP], F16, name="mg0")
            mg1 = wp.tile([32, MP], F16, name="mg1")
            mvp = wp.tile([128, 4], F16, name="mvp")   # partial maxes per chunk
            with nc.allow_low_precision(reason="fp16 max-pool rounds values only"):
                for ci, (lo, hi) in enumerate(((0, 512), (512, MP))):
                    nc.vector.scalar_tensor_tensor(out=mg0[:, lo:hi], in0=matt0[:, lo:hi],
                                                   scalar=0.5, in1=mc0[:, lo:hi],
                                                   op0=ALU.add, op1=ALU.mult)
                    nc.vector.tensor_reduce(out=mvp[:, ci:ci + 1], in_=mg0[:, lo:hi],
                                            op=ALU.max, axis=AX.X)
                    nc.vector.scalar_tensor_tensor(out=mg1[:, lo:hi], in0=matt1[:, lo:hi],
                                                   scalar=0.5, in1=mc1[:, lo:hi],
                                                   op0=ALU.add, op1=ALU.mult)
                    nc.vector.tensor_reduce(out=mvp[0:32, 2 + ci:3 + ci], in_=mg1[:, lo:hi],
                                            op=ALU.max, axis=AX.X)
                nc.vector.tensor_reduce(out=pvf[:, 2:3], in_=mvp[:, 0:2], op=ALU.max, axis=AX.X)
                nc.vector.tensor_reduce(out=pvf[0:32, 3:4], in_=mvp[0:32, 2:4], op=ALU.max, axis=AX.X)

            # ================= FC head =================
            def lrelu(name, f_ps, bias_lo, ncols):
                fb = wp.tile([128, ncols], F32, name=name + "_b")
                nc.vector.tensor_tensor(out=fb, in0=f_ps, in1=wsmall[:, bias_lo:bias_lo + ncols], op=ALU.add)
                fs = wp.tile([128, ncols], F32, name=name + "_s")
                nc.vector.tensor_scalar(out=fs, in0=fb, scalar1=0.01, scalar2=None, op0=ALU.mult)
                fo = wp.tile([128, ncols], F16, name=name)
                nc.vector.tensor_tensor(out=fo, in0=fb, in1=fs, op=ALU.max)
                return fo

            # f1: per-column accumulation groups (one 2KB region holds all
            # columns, so groups must not interleave); within a column the
            # two 128-row stationaries go first to reduce PE config flips
            f1_ps = ps.tile([128, 8], F32, name="f1_ps", tag="ps")
            for a in range(8):
                nc.tensor.matmul(f1_ps[:, a:a + 1], wfc[0:128, W1A + a * 128:W1A + a * 128 + 128],
                                 pvf[:, 0:1], start=True, stop=False)
                nc.tensor.matmul(f1_ps[:, a:a + 1], wfc[0:128, W1A + 1024 + a * 128:W1A + 1024 + a * 128 + 128],
                                 pvf[:, 2:3], start=False, stop=False)
                nc.tensor.matmul(f1_ps[:, a:a + 1], wfc[0:32, W1B + a * 128:W1B + a * 128 + 128],
                                 pvf[0:32, 1:2], start=False, stop=False)
                nc.tensor.matmul(f1_ps[:, a:a + 1], wfc[0:32, W1B + 1024 + a * 128:W1B + 1024 + a * 128 + 128],
                                 pvf[0:32, 3:4], start=False, stop=True)
            f1 = lrelu("f1", f1_ps, SB_B1, 8)

            f2_ps = ps.tile([128, 8], F32, name="f2_ps", tag="ps")
            for a in range(8):
                for jb in range(8):
                    nc.tensor.matmul(f2_ps[:, a:a + 1],
                                     wfc[0:128, W2C + jb * 1024 + a * 128:W2C + jb * 1024 + a * 128 + 128],
                                     f1[:, jb:jb + 1], start=(jb == 0), stop=(jb == 7))
            f2 = lrelu("f2", f2_ps, SB_B2, 8)

            f3_ps = ps.tile([128, 4], F32, name="f3_ps", tag="ps")
            for a in range(4):
                for jb in range(8):
                    nc.tensor.matmul(f3_ps[:, a:a + 1],
                                     wfc[0:128, W3C + jb * 512 + a * 128:W3C + jb * 512 + a * 128 + 128],
                                     f2[:, jb:jb + 1], start=(jb == 0), stop=(jb == 7))
            f3 = lrelu("f3", f3_ps, SB_B3, 4)

            o_ps = ps.tile([2, 1], F32, name="o_ps", tag="ps")
            for c in range(4):
                nc.tensor.matmul(o_ps, wfc[0:128, WOC + 2 * c:WOC + 2 * c + 2], f3[:, c:c + 1],
                                 start=(c == 0), stop=(c == 3))
            o_sb = wp.tile([2, 1], F32, name="o_sb")
            nc.vector.tensor_tensor(out=o_sb, in0=o_ps, in1=wsmall[0:2, SB_BO:SB_BO + 1], op=ALU.add)
            nc.sync.dma_start(out=out_e[:], in_=o_sb)

    _split_excess_waits(nc, max_waits=1)
    return nc


_PROGRAM = None


def _get_program():
    global _PROGRAM
    if _PROGRAM is None:
        _PROGRAM = _build_program()
    return _PROGRAM


def _prep_weights(inp):
    """Host-side packing shared by all cores."""
    import ml_dtypes
    f16 = np.float16
    bf16 = ml_dtypes.bfloat16
    f32 = lambda x: np.asarray(x, dtype=np.float32)
    as_f16bits = lambda a: np.ascontiguousarray(a).view(np.uint16).view(f16)

    def convw(w):  # [co, ci, k] -> [ci, k*co] fp16
        w = np.asarray(w, dtype=np.float32)
        ci = w.shape[1]
        return w.transpose(1, 2, 0).reshape(ci, -1).astype(f16)

    wboot = np.zeros((128, NBOOT), f16)
    wboot[0:DIM, PW1:PW1 + K1 * CONV] = convw(inp['pw1'])
    wboot[0:DIM, MW1:MW1 + K1 * CONV] = convw(inp['mw1'])
    wboot[0:CONV, PW2:PW2 + K2 * C2] = convw(inp['pw2'])
    wboot[0:CONV, MW2:MW2 + K2 * C2] = convw(inp['mw2'])

    wc3 = np.zeros((128, NC3), f16)
    wc3[0:C2, PW3:PW3 + K3 * C4] = convw(inp['pw3'])
    wc3[0:C2, MW3:MW3 + K3 * C4] = convw(inp['mw3'])

    wpa, wma = f32(inp['Wpa']), f32(inp['Wma'])
    wca = f32(inp['Wa']) / float(LM3)
    wm2 = f32(inp['Wa']) / float(LP3)
    wattn = np.zeros((128, NATTN), f16)
    wattn[0:128, WPA_A:WPA_A + 160] = wpa[0:128].astype(f16)
    wattn[0:32, WPA_B:WPA_B + 160] = wpa[128:160].astype(f16)
    wattn[0:128, WMA_A:WMA_A + 128] = wma[0:128, 0:128].astype(f16)
    wattn[0:128, WMA_A + 128:WMA_A + 256] = np.tile(wma[0:128, 128:160], (1, 4)).astype(f16)
    wattn[0:32, WMA_B:WMA_B + 128] = wma[128:160, 0:128].astype(f16)
    wattn[0:32, WMA_B + 128:WMA_B + 256] = np.tile(wma[128:160, 128:160], (1, 4)).astype(f16)
    wattn[0:128, WCA_A:WCA_A + 160] = as_f16bits(wca[0:128].astype(bf16))
    wattn[0:32, WCA_B:WCA_B + 160] = as_f16bits(wca[128:160].astype(bf16))
    wattn[0:128, WM2_A:WM2_A + 160] = as_f16bits(wm2[0:128].astype(bf16))
    wattn[0:32, WM2_B:WM2_B + 160] = as_f16bits(wm2[128:160].astype(bf16))
    id128 = np.eye(128, dtype=bf16)
    idst = np.tile(np.eye(32, dtype=bf16), (4, 1))
    wattn[0:128, ID128:ID128 + 128] = as_f16bits(id128)
    wattn[0:128, IDST:IDST + 32] = as_f16bits(idst)
    for j in range(4):
        wattn[32 * j:32 * j + 32, WCB0 + j * 128:WCB0 + (j + 1) * 128] = as_f16bits(wca[128:160, 0:128].astype(bf16))
        wattn[32 * j:32 * j + 32, WCB1 + j * 32:WCB1 + (j + 1) * 32] = as_f16bits(wca[128:160, 128:160].astype(bf16))

    w1 = f32(inp['W1'])
    wfc = np.zeros((128, NFC), f16)
    wfc[0:128, W1A:W1A + 2048] = np.concatenate([w1[0:128], w1[160:288]], axis=1).astype(f16)
    wfc[0:32, W1B:W1B + 2048] = np.concatenate([w1[128:160], w1[288:320]], axis=1).astype(f16)

    def fcw(w, nblk):  # [I, J], I = nblk*128 -> [128, nblk*J]
        w = np.asarray(w, dtype=np.float32)
        i, j = w.shape
        return w.reshape(nblk, 128, j).transpose(1, 0, 2).reshape(128, nblk * j).astype(f16)

    wfc[0:128, W2C:W2C + 8192] = fcw(inp['W2'], 8)
    wfc[0:128, W3C:W3C + 4096] = fcw(inp['W3'], 8)
    wfc[0:128, WOC:WOC + 8] = fcw(inp['Wo'], 4)

    wsmall = np.zeros((128, NSMALL), np.float32)
    def bias2(col_a, col_b, b):
        b = f32(b)
        wsmall[0:128, col_a] = b[0:128]
        wsmall[0:32, col_b] = b[128:160]
    wsmall[0:CONV, SB_PB1] = f32(inp['pb1'])
    wsmall[0:C2, SB_PB2] = f32(inp['pb2'])
    bias2(SB_PB3A, SB_PB3B, inp['pb3'])
    wsmall[0:CONV, SB_MB1] = f32(inp['mb1'])
    wsmall[0:C2, SB_MB2] = f32(inp['mb2'])
    bias2(SB_MB3A, SB_MB3B, inp['mb3'])
    bias2(SB_BPA_A, SB_BPA_B, inp['bpa'])
    wsmall[0:128, SB_BMA_A] = f32(inp['bma'])[0:128]
    wsmall[0:128, SB_BMA_R4] = np.tile(f32(inp['bma'])[128:160], 4)
    bias2(SB_BA_A, SB_BA_B, inp['ba'])
    wsmall[0:128, SB_B1:SB_B1 + 8] = f32(inp['b1']).reshape(8, 128).T
    wsmall[0:128, SB_B2:SB_B2 + 8] = f32(inp['b2']).reshape(8, 128).T
    wsmall[0:128, SB_B3:SB_B3 + 4] = f32(inp['b3']).reshape(4, 128).T
    wsmall[0:2, SB_BO] = f32(inp['bo'])

    return {'wboot': wboot, 'wc3': wc3, 'wattn': wattn, 'wfc': wfc, 'wsmall': wsmall}


def _prep_core(inp, b):
    """Per-core embedding gather: [64, 1100] fp16."""
    pep = np.asarray(inp['peptide'])[b]
    mhc = np.asarray(inp['MHC'])[b]
    import ml_dtypes
    pe = np.asarray(inp['pep_emb'], np.float32)[pep].T   # [64, 100]
    me = np.asarray(inp['mhc_emb'], np.float32)[mhc].T   # [64, 1000]
    return np.concatenate([pe, me], axis=1).astype(np.float16)


def kernel(**inputs):
    nc = _get_program()
    shared = _prep_weights(inputs)
    in_maps = []
    for b in range(B):
        m = dict(shared)
        m['emb'] = _prep_core(inputs, b)
        in_maps.append(m)
    res = run_bass_kernel_spmd(nc, in_maps, core_ids=list(range(B)))
    return np.stack([np.asarray(res.results[i]['out']).reshape(2) for i in range(B)]).astype(np.float32)


# revision 20
# speedup vs baseline: 2.7689x; 1.0341x over previous
"""AttentionDTI forward pass on 8 Trainium2 NeuronCores (pure data parallel).

One batch element per core, weights replicated. All matmul operands are
16-bit (f32 PSUM accumulation): fp16 for conv/FC weights and activations,
bf16 for the attention tiles (the Activation engine runs ~1.3x slower on
fp16 than bf16, and the attention path tolerates bf16). Embedding lookup
is done host-side; weights arrive in packed DMAs issued from both the SP
and Activation HWDGE queues so transfers overlap the NEFF prologue.

The 4D additive-attention tensor h[b,p,m,c] = relu(pa + ma) is never
materialized: mean_m(h @ Wa) == mean_m(h) @ Wa, so only hp[c,p] = sum_m h
and hm[c,m] = sum_p h are accumulated on the fly. hm accumulates in PSUM
by streaming h tiles through the PE against a stationary identity; hp
comes from fused per-tile accumulators: the Scalar engine's
relu+bias+accum activation alternates 1:1 with the Vector engine's
scalar_tensor_tensor (relu via max-with-zeros + sum accum) — any DVE op
with an accum output runs at the 1x element rate, so the fused form is
optimal on both engines.

c-channels [128:160] run packed: ma rows replicated 4x vertically (via
host-replicated stationary columns, free) so each tile covers 4 peptide
positions; a 4-stacked [128,32] identity reduces them into hm1. The
peptide gate keeps its column axis in "j-major" order (jm(p) =
(p%4)*22 + p//4) end-to-end — max-pool over p is order-invariant — which
makes the packed hp1p contributions contiguous matmuls and avoids any
unpack DMAs.

Environment constraints discovered empirically (this axon terminal):
  - GPSIMD/Pool compute ops fail codegen; SWDGE DMA hangs: DMAs go
    through SP/Activation HWDGE only.
  - tensor_tensor_reduce fails walrus codegen ("ISA wrong length").
  - tensor_scalar's accum_out hijacks op1 as the reduce op (no fused
    two-op elementwise + sum) — scalar_tensor_tensor does fuse it.
  - walrus allows at most ONE semaphore wait per instruction:
    _split_excess_waits() rewrites the scheduled program.
"""
import sys

_BASS_ROOT = '/opt/trn_rl_repo'
if _BASS_ROOT not in sys.path:
    sys.path.insert(0, _BASS_ROOT)

import numpy as np

import concourse.bass as bass
import concourse.tile as tile
from concourse import mybir
from concourse.bass_utils import run_bass_kernel_spmd

F32 = mybir.dt.float32
F16 = mybir.dt.float16
BF16 = mybir.dt.bfloat16
ALU = mybir.AluOpType
AF = mybir.ActivationFunctionType
AX = mybir.AxisListType

B = 8
LP, LM, DIM, CONV = 100, 1000, 64, 40
C2, C4 = CONV * 2, CONV * 4          # 80, 160
K1, K2, K3 = 4, 6, 8
LP1, LP2, LP3 = 97, 92, 85           # peptide conv output lengths
LM1, LM2, LM3 = 997, 992, 985        # MHC conv output lengths
NP4 = 22                             # ceil(85/4) packed p-groups
JM = 4 * NP4                         # 88 j-major gate columns
MP = 992                             # LM3 padded for 4x-eligible DVE gate ops
NEG = -30000.0
SPLIT_MOD = 2                        # i % SPLIT_MOD == 0 -> scalar h tile

# ---- wboot column map (fp16 [128, 1120]): conv1 (pair-folded) + conv2 ----
PW1, MW1, PW2, MW2 = 0, 80, 160, 640
NBOOT = 1120
# ---- wc3 column map (fp16 [128, 2560]): conv3 weights ----
PW3, MW3 = 0, 1280
NC3 = 2560
# ---- wattn column map (fp16 [128, 1632]) ----
WPA_A, WPA_B = 0, 160        # [128,160], [32,160]
WMA_A, WMA_B = 320, 576      # [128,256], [32,256] (cols 128:256 = rep4 of Wma[:,128:160])
WCA_A, WCA_B = 832, 992      # Wa/LM3: [128,160], [32,160]
WM2_A, WM2_B = 1152, 1312    # Wa/LP3
ID128, IDST = 1472, 1600     # bf16 identities: [128,128], [128,32]
WCB0, WCB1 = 1632, 2144      # j-lifted Wa[128:160]/LM3: 4x[128,128], 4x[128,32]
NATTN = 2272
# ---- wfc column map (fp16 [128, 16392]) ----
W1A, W1B = 0, 2048           # [128, 2048], [32, 2048]
W2C, W3C, WOC = 4096, 12288, 16384
NFC = 16392
# ---- wsmall column map (f32 [128, 35]) ----
SB_PB1, SB_PB2, SB_PB3A, SB_PB3B = 0, 1, 2, 3
SB_MB1, SB_MB2, SB_MB3A, SB_MB3B = 4, 5, 6, 7
SB_BPA_A, SB_BPA_B = 8, 9
SB_BMA_A, SB_BMA_R4 = 10, 11
SB_BA_A, SB_BA_B = 12, 13
SB_B1, SB_B2, SB_B3, SB_BO = 14, 22, 30, 34
NSMALL = 35


def _jm(p):
    return (p % 4) * NP4 + (p // 4)


_ctr = [0]


def _split_excess_waits(nc, max_waits=1):
    n_split = 0
    for f in nc.m.functions:
        for b in f.blocks:
            insts = list(b.instructions)
            out = []
            changed = False
            for inst in insts:
                si = inst.sync_info
                waits = list(si.on_wait) if (si is not None and si.on_wait) else []
                if len(waits) > max_waits:
                    changed = True
                    n_split += 1
                    keep = max(1, max_waits)
                    head, tail = waits[:-keep], waits[-keep:]
                    for i in range(0, len(head), keep):
                        chunk = head[i:i + keep]
                        nop = mybir.InstEventSemaphore(
                            name=f"ant-wait-split-{_ctr[0]}", ins=[], outs=[])
                        _ctr[0] += 1
                        nop.engine = inst.engine
                        nop.sync_info = mybir.SyncInfo(on_wait=chunk, on_update=[])
                        nc.register_instruction(nop)
                        out.append(nop)
                    upd = list(si.on_update) if si.on_update else []
                    inst.sync_info = mybir.SyncInfo(on_wait=tail, on_update=upd)
                out.append(inst)
            if changed:
                b.instructions = out
    return n_split


def _conv_matmuls(nc, psum, wtile, x, k_taps, co_lo, co_hi, m_lo, m_hi, cout_stride):
    """Valid 1-D conv as k shifted matmuls accumulated into `psum`."""
    for k in range(k_taps):
        nc.tensor.matmul(
            psum,
            wtile[:, k * cout_stride + co_lo: k * cout_stride + co_hi],
            x[:, m_lo + k: m_hi + k],
            start=(k == 0), stop=(k == k_taps - 1))


def _build_program():
    nc = bass.Bass("TRN2", target_bir_lowering=False, debug=False)

    emb_e = nc.declare_dram_parameter("emb", [128, LP + LM], F16, isOutput=False)
    wsmall_e = nc.declare_dram_parameter("wsmall", [128, NSMALL], F32, isOutput=False)
    wboot_e = nc.declare_dram_parameter("wboot", [128, NBOOT], F16, isOutput=False)
    wc3_e = nc.declare_dram_parameter("wc3", [128, NC3], F16, isOutput=False)
    wattn_e = nc.declare_dram_parameter("wattn", [128, NATTN], F16, isOutput=False)
    wfc_e = nc.declare_dram_parameter("wfc", [128, NFC], F16, isOutput=False)
    out_e = nc.declare_dram_parameter("out", [2, 1], F32, isOutput=True)

    with tile.TileContext(nc) as tc:
        with tc.tile_pool(name="consts", bufs=1) as cp, \
             tc.tile_pool(name="work", bufs=1) as wp, \
             tc.tile_pool(name="hpool", bufs=8) as hpool, \
             tc.tile_pool(name="ps_hm", bufs=1, space="PSUM") as ps_hm, \
             tc.tile_pool(name="ps_work", bufs=2, space="PSUM") as ps:

            # loads: small/boot/emb from SP; conv3+attn from Activation HWDGE;
            # the big FC pack last on SP.
            emb = cp.tile([128, LP + LM], F16, name="emb")
            nc.sync.dma_start(out=emb, in_=emb_e[:])
            wboot = cp.tile([128, NBOOT], F16, name="wboot")
            nc.scalar.dma_start(out=wboot, in_=wboot_e[:])
            wsmall = cp.tile([128, NSMALL], F32, name="wsmall")
            nc.sync.dma_start(out=wsmall, in_=wsmall_e[:])
            wc3 = cp.tile([128, NC3], F16, name="wc3")
            nc.scalar.dma_start(out=wc3, in_=wc3_e[:])
            wattn = cp.tile([128, NATTN], F16, name="wattn")
            nc.sync.dma_start(out=wattn, in_=wattn_e[:])
            wfc = cp.tile([128, NFC], F16, name="wfc")
            nc.sync.dma_start(out=wfc, in_=wfc_e[:])

            pe = emb[:, 0:LP]
            me = emb[:, LP:LP + LM]

            def conv1_pair(psum, w_base, x, m_lo, m_hi):
                # taps (0,1) and (2,3) folded via the stacked-shifted embedding
                for tp in range(2):
                    nc.tensor.matmul(
                        psum,
                        wboot[0:128, w_base + tp * CONV:w_base + (tp + 1) * CONV],
                        x[:, m_lo + 2 * tp:m_hi + 2 * tp],
                        start=(tp == 0), stop=(tp == 1))
            bias = lambda col, rows=128: wsmall[0:rows, col:col + 1]

            # early zero/NEG fills on the (idle) Vector engine
            zt = wp.tile([128, LM3], BF16, name="zt")
            nc.vector.memset(zt, 0.0)
            # warm the activation table (one-time ~1.3us load) off the
            # critical path, before any DMA lands
            atl = wp.tile([1, 1], F16, name="atl")
            nc.scalar.activation(out=atl, in_=zt[0:1, 0:1], func=AF.Relu, bias=0.0)
            hp0 = wp.tile([128, JM], F32, name="hp0")
            nc.vector.memset(hp0, 0.0)
            pa1p = wp.tile([128, NP4], F32, name="pa1p")
            nc.vector.memset(pa1p, NEG)
            pc0jm = wp.tile([128, JM], F16, name="pc0jm")
            nc.vector.memset(pc0jm, 0.0)
            pc1jm = wp.tile([32, JM], F16, name="pc1jm")
            nc.vector.memset(pc1jm, 0.0)
            hm0f = wp.tile([128, MP], BF16, name="hm0f")
            nc.vector.memset(hm0f[:, LM3:MP], 0.0)
            hm1f = wp.tile([32, MP], BF16, name="hm1f")
            nc.vector.memset(hm1f[:, LM3:MP], 0.0)
            mc0 = wp.tile([128, MP], F16, name="mc0")
            nc.vector.memset(mc0[:, LM3:MP], 0.0)
            mc1 = wp.tile([32, MP], F16, name="mc1")
            nc.vector.memset(mc1[:, LM3:MP], 0.0)

            # ================= conv stacks (fp16, f32 psum) =================
            # MHC conv1: [64,1000] -> [40,997]; relu chunked so conv2 starts early
            mx1_ps = ps.tile([CONV, LM1], F32, name="mx1_ps", tag="ps")
            conv1_pair(mx1_ps[:, 0:512], MW1, me, 0, 512)
            conv1_pair(mx1_ps[:, 512:LM1], MW1, me, 512, LM1)
            # peptide conv1 fills the PE while relu1 runs
            px1_ps = ps.tile([CONV, LP1], F32, name="px1_ps", tag="ps")
            conv1_pair(px1_ps, PW1, pe, 0, LP1)
            mx1 = wp.tile([CONV, LM1], F16, name="mx1")
            nc.scalar.activation(out=mx1[:, 0:520], in_=mx1_ps[:, 0:520], func=AF.Relu, bias=bias(SB_MB1, CONV))
            nc.vector.tensor_scalar(out=mx1[:, 520:LM1], in0=mx1_ps[:, 520:LM1], scalar1=bias(SB_MB1, CONV),
                                    scalar2=0.0, op0=ALU.add, op1=ALU.max)
            px1 = wp.tile([CONV, LP1], F16, name="px1")
            nc.scalar.activation(out=px1, in_=px1_ps, func=AF.Relu, bias=bias(SB_PB1, CONV))

            # MHC conv2 -> [80, 992]
            mx2_ps = ps.tile([C2, LM2], F32, name="mx2_ps", tag="ps")
            _conv_matmuls(nc, mx2_ps[:, 0:512], wboot[0:CONV, MW2:MW2 + K2 * C2], mx1, K2, 0, C2, 0, 512, C2)
            _conv_matmuls(nc, mx2_ps[:, 512:LM2], wboot[0:CONV, MW2:MW2 + K2 * C2], mx1, K2, 0, C2, 512, LM2, C2)
            px2_ps = ps.tile([C2, LP2], F32, name="px2_ps", tag="ps")
            _conv_matmuls(nc, px2_ps, wboot[0:CONV, PW2:PW2 + K2 * C2], px1, K2, 0, C2, 0, LP2, C2)
            mx2 = wp.tile([C2, LM2], F16, name="mx2")
            nc.scalar.activation(out=mx2[:, 0:520], in_=mx2_ps[:, 0:520], func=AF.Relu, bias=bias(SB_MB2, C2))
            nc.vector.tensor_scalar(out=mx2[:, 520:LM2], in0=mx2_ps[:, 520:LM2], scalar1=bias(SB_MB2, C2),
                                    scalar2=0.0, op0=ALU.add, op1=ALU.max)
            px2 = wp.tile([C2, LP2], F16, name="px2")
            nc.scalar.activation(out=px2, in_=px2_ps, func=AF.Relu, bias=bias(SB_PB2, C2))

            # MHC conv3 -> [160, 985] as [128,985]+[32,985] (into MP-padded tiles)
            mc0_ps = ps.tile([128, LM3], F32, name="mc0_ps", tag="ps")
            _conv_matmuls(nc, mc0_ps[:, 0:512], wc3[0:C2, MW3:MW3 + K3 * C4], mx2, K3, 0, 128, 0, 512, C4)
            _conv_matmuls(nc, mc0_ps[:, 512:LM3], wc3[0:C2, MW3:MW3 + K3 * C4], mx2, K3, 0, 128, 512, LM3, C4)
            nc.scalar.activation(out=mc0[:, 0:512], in_=mc0_ps[:, 0:512], func=AF.Relu, bias=bias(SB_MB3A))
            nc.vector.tensor_scalar(out=mc0[:, 512:LM3], in0=mc0_ps[:, 512:LM3], scalar1=bias(SB_MB3A),
                                    scalar2=0.0, op0=ALU.add, op1=ALU.max)
            mc1_ps = ps.tile([32, LM3], F32, name="mc1_ps", tag="ps")
            _conv_matmuls(nc, mc1_ps[:, 0:512], wc3[0:C2, MW3:MW3 + K3 * C4], mx2, K3, 128, C4, 0, 512, C4)
            _conv_matmuls(nc, mc1_ps[:, 512:LM3], wc3[0:C2, MW3:MW3 + K3 * C4], mx2, K3, 128, C4, 512, LM3, C4)
            nc.scalar.activation(out=mc1[:, 0:512], in_=mc1_ps[:, 0:512], func=AF.Relu, bias=bias(SB_MB3B, 32))
            nc.vector.tensor_scalar(out=mc1[:, 512:LM3], in0=mc1_ps[:, 512:LM3], scalar1=bias(SB_MB3B, 32),
                                    scalar2=0.0, op0=ALU.add, op1=ALU.max)

            # peptide conv3 (tiles padded to 88 cols for the 4-strided views)
            pc0_ps = ps.tile([128, LP3], F32, name="pc0_ps", tag="ps")
            _conv_matmuls(nc, pc0_ps, wc3[0:C2, PW3:PW3 + K3 * C4], px2, K3, 0, 128, 0, LP3, C4)
            pc0 = wp.tile([128, 88], F16, name="pc0")
            nc.scalar.activation(out=pc0[:, 0:LP3], in_=pc0_ps, func=AF.Relu, bias=bias(SB_PB3A))
            pc1_ps = ps.tile([32, LP3], F32, name="pc1_ps", tag="ps")
            _conv_matmuls(nc, pc1_ps, wc3[0:C2, PW3:PW3 + K3 * C4], px2, K3, 128, C4, 0, LP3, C4)
            pc1 = wp.tile([32, 88], F16, name="pc1")
            nc.scalar.activation(out=pc1[:, 0:LP3], in_=pc1_ps, func=AF.Relu, bias=bias(SB_PB3B, 32))

            # j-major copies of pc for the gate (vector, strided reads)
            pc0_g = pc0.rearrange("c (g f) -> c g f", f=4)
            pc1_g = pc1.rearrange("c (g f) -> c g f", f=4)
            for j in range(4):
                ncol = NP4 if j == 0 else NP4 - 1
                nc.vector.tensor_scalar(out=pc0jm[:, j * NP4:j * NP4 + ncol],
                                        in0=pc0_g[:, 0:ncol, j], scalar1=0.0,
                                        scalar2=None, op0=ALU.add)
                nc.vector.tensor_scalar(out=pc1jm[:, j * NP4:j * NP4 + ncol],
                                        in0=pc1_g[:, 0:ncol, j], scalar1=0.0,
                                        scalar2=None, op0=ALU.add)

            # ================= attention projections =================
            # ma0[c,m] c in 0:128 (bf16 for the Activation-engine h producer)
            ma0_ps = ps.tile([128, LM3], F32, name="ma0_ps", tag="ps")
            for lo, hi in ((0, 512), (512, LM3)):
                nc.tensor.matmul(ma0_ps[:, lo:hi], wattn[0:128, WMA_A:WMA_A + 128], mc0[:, lo:hi], start=True, stop=False)
                nc.tensor.matmul(ma0_ps[:, lo:hi], wattn[0:32, WMA_B:WMA_B + 128], mc1[:, lo:hi], start=False, stop=True)
            ma0 = wp.tile([128, LM3], BF16, name="ma0")
            nc.scalar.activation(out=ma0, in_=ma0_ps, func=AF.Identity, bias=bias(SB_BMA_A))

            # ma1p: c in 128:160 replicated 4x vertically (stationary pre-replicated)
            ma1p_ps = ps.tile([128, LM3], F32, name="ma1p_ps", tag="ps")
            for lo, hi in ((0, 512), (512, LM3)):
                nc.tensor.matmul(ma1p_ps[:, lo:hi], wattn[0:128, WMA_A + 128:WMA_A + 256], mc0[:, lo:hi], start=True, stop=False)
                nc.tensor.matmul(ma1p_ps[:, lo:hi], wattn[0:32, WMA_B + 128:WMA_B + 256], mc1[:, lo:hi], start=False, stop=True)
            ma1p = wp.tile([128, LM3], BF16, name="ma1p")
            nc.vector.tensor_scalar(out=ma1p, in0=ma1p_ps, scalar1=bias(SB_BMA_R4), scalar2=None, op0=ALU.add)

            # pa0[c,p] c in 0:128 (f32, used as per-partition bias)
            pa0_ps = ps.tile([128, LP3], F32, name="pa0_ps", tag="ps")
            nc.tensor.matmul(pa0_ps, wattn[0:128, WPA_A:WPA_A + 128], pc0[:, 0:LP3], start=True, stop=False)
            nc.tensor.matmul(pa0_ps, wattn[0:32, WPA_B:WPA_B + 128], pc1[0:32, 0:LP3], start=False, stop=True)
            pa0 = wp.tile([128, LP3], F32, name="pa0")
            nc.scalar.add(pa0, pa0_ps, bias(SB_BPA_A))

            # pa1p[32j+d, g] = pa[128+d, 4g+j]: partition-offset matmuls over
            # 4-strided moving views of pc
            pa1p_ps = ps.tile([128, NP4], F32, name="pa1p_ps", tag="ps")
            for j in range(4):
                ncol = NP4 if j == 0 else NP4 - 1
                nc.tensor.matmul(pa1p_ps[32 * j:32 * j + 32, 0:ncol],
                                 wattn[0:128, WPA_A + 128:WPA_A + 160],
                                 pc0_g[:, 0:ncol, j],
                                 start=True, stop=False, skip_group_check=True,
                                 tile_position=(0, 32 * j))
                nc.tensor.matmul(pa1p_ps[32 * j:32 * j + 32, 0:ncol],
                                 wattn[0:32, WPA_B + 128:WPA_B + 160],
                                 pc1_g[:, 0:ncol, j],
                                 start=False, stop=True, skip_group_check=True,
                                 tile_position=(0, 32 * j))
                nc.scalar.add(pa1p[32 * j:32 * j + 32, 0:ncol],
                              pa1p_ps[32 * j:32 * j + 32, 0:ncol], bias(SB_BPA_B, 32))

            # ================= 4D attention reductions =================
            # hp0 columns are written in j-major order (gate is order-free)
            hp1p = wp.tile([128, NP4], F32, name="hp1p")
            hm0_ps = ps_hm.tile([128, LM3], F32, name="hm0_ps")
            hm1_ps = ps_hm.tile([32, LM3], F32, name="hm1_ps")

            # identities and gate weights stored as bf16 bit patterns in the
            # fp16 pack; bitcast views pair them with bf16 moving operands
            id128 = wattn[0:128, ID128:ID128 + 128].bitcast(BF16)
            idst = wattn[0:128, IDST:IDST + 32].bitcast(BF16)

            # greedy producer balance: assign each tile to whichever engine
            # would finish it first (measured fused costs: ACT 1199, DVE 1263)
            prod_t = {'sc': 0.0, 've': 0.0}

            def h_tile(i, src, bias_ap, acc):
                h = hpool.tile([128, LM3], BF16, tag="h", name="h")
                if prod_t['sc'] + 1200.0 <= prod_t['ve'] + 1187.0:
                    prod_t['sc'] += 1200.0
                    nc.scalar.activation(out=h, in_=src, func=AF.Relu,
                                         bias=bias_ap, accum_out=acc)
                else:
                    prod_t['ve'] += 1187.0
                    nc.vector.scalar_tensor_tensor(out=h, in0=src, scalar=bias_ap,
                                                   in1=zt, op0=ALU.add, op1=ALU.max,
                                                   accum_out=acc)
                return h

            for p in range(LP3):
                h = h_tile(p, ma0, pa0[:, p:p + 1], hp0[:, _jm(p):_jm(p) + 1])
                nc.tensor.matmul(hm0_ps[:, 0:512], id128, h[:, 0:512],
                                 start=(p == 0), stop=(p == LP3 - 1))
                nc.tensor.matmul(hm0_ps[:, 512:LM3], id128, h[:, 512:LM3],
                                 start=(p == 0), stop=(p == LP3 - 1))

            # p-side gate head start: hp0 is complete before the packed loop
            hp0f = wp.tile([128, JM], BF16, name="hp0f")
            nc.scalar.copy(hp0f, hp0)
            cl0_ps = ps.tile([128, JM], F32, name="cl0_ps", tag="ps")
            nc.tensor.matmul(cl0_ps, wattn[0:128, WCA_A:WCA_A + 128].bitcast(BF16), hp0f,
                             start=True, stop=False, skip_group_check=True)
            cl1_ps = ps.tile([32, JM], F32, name="cl1_ps", tag="ps")
            nc.tensor.matmul(cl1_ps, wattn[0:128, WCA_A + 128:WCA_A + 160].bitcast(BF16), hp0f,
                             start=True, stop=False, skip_group_check=True)

            for g in range(NP4):
                h = h_tile(LP3 + g, ma1p, pa1p[:, g:g + 1], hp1p[:, g:g + 1])
                nc.tensor.matmul(hm1_ps[:, 0:512], idst, h[:, 0:512],
                                 start=(g == 0), stop=(g == NP4 - 1))
                nc.tensor.matmul(hm1_ps[:, 512:LM3], idst, h[:, 512:LM3],
                                 start=(g == 0), stop=(g == NP4 - 1))

            # ================= peptide gate (j-major) =================
            hp1pf = wp.tile([128, NP4], BF16, name="hp1pf")
            nc.scalar.copy(hp1pf, hp1p)
            for j in range(4):
                nc.tensor.matmul(cl0_ps[:, j * NP4:(j + 1) * NP4],
                                 wattn[0:128, WCB0 + j * 128:WCB0 + (j + 1) * 128].bitcast(BF16),
                                 hp1pf, start=False, stop=(j == 3), skip_group_check=True)
                nc.tensor.matmul(cl1_ps[:, j * NP4:(j + 1) * NP4],
                                 wattn[0:128, WCB1 + j * 32:WCB1 + (j + 1) * 32].bitcast(BF16),
                                 hp1pf, start=False, stop=(j == 3), skip_group_check=True)
            catt0 = wp.tile([128, JM], F16, name="catt0")
            nc.scalar.activation(out=catt0, in_=cl0_ps, func=AF.Sigmoid, bias=bias(SB_BA_A))
            catt1 = wp.tile([32, JM], F16, name="catt1")
            nc.scalar.activation(out=catt1, in_=cl1_ps, func=AF.Sigmoid, bias=bias(SB_BA_B, 32))

            pvf = wp.tile([128, 4], F16, name="pvf")   # cols: pv0, pv1, mv0, mv1
            nc.vector.memset(pvf, 0.0)
            pg0 = wp.tile([128, JM], F16, name="pg0")
            nc.vector.scalar_tensor_tensor(out=pg0, in0=catt0, scalar=0.5, in1=pc0jm,
                                           op0=ALU.add, op1=ALU.mult)
            pg1 = wp.tile([32, JM], F16, name="pg1")
            nc.vector.scalar_tensor_tensor(out=pg1, in0=catt1, scalar=0.5, in1=pc1jm,
                                           op0=ALU.add, op1=ALU.mult)
            with nc.allow_low_precision(reason="fp16 max-pool rounds values only"):
                nc.vector.tensor_reduce(out=pvf[:, 0:1], in_=pg0, op=ALU.max, axis=AX.X)
                nc.vector.tensor_reduce(out=pvf[0:32, 1:2], in_=pg1, op=ALU.max, axis=AX.X)

            # ================= MHC gate (chunk-pipelined) =================
            for lo, hi in ((0, 512), (512, LM3)):
                nc.scalar.activation(out=hm0f[:, lo:hi], in_=hm0_ps[:, lo:hi], func=AF.Copy)
                nc.vector.tensor_scalar(out=hm1f[:, lo:hi], in0=hm1_ps[:, lo:hi],
                                        scalar1=0.0, scalar2=None, op0=ALU.add)
            ml0_ps = ps.tile([128, MP], F32, name="ml0_ps", tag="ps")
            ml1_ps = ps.tile([32, MP], F32, name="ml1_ps", tag="ps")
            matt0 = wp.tile([128, MP], F16, name="matt0")
            matt1 = wp.tile([32, MP], F16, name="matt1")
            for lo, hi in ((0, 512), (512, MP)):
                nc.tensor.matmul(ml0_ps[:, lo:hi], wattn[0:128, WM2_A:WM2_A + 128].bitcast(BF16), hm0f[:, lo:hi], start=True, stop=False)
                nc.tensor.matmul(ml0_ps[:, lo:hi], wattn[0:32, WM2_B:WM2_B + 128].bitcast(BF16), hm1f[:, lo:hi], start=False, stop=True)
                nc.scalar.activation(out=matt0[:, lo:hi], in_=ml0_ps[:, lo:hi], func=AF.Sigmoid, bias=bias(SB_BA_A))
                nc.tensor.matmul(ml1_ps[:, lo:hi], wattn[0:128, WM2_A + 128:WM2_A + 160].bitcast(BF16), hm0f[:, lo:hi], start=True, stop=False)
                nc.tensor.matmul(ml1_ps[:, lo:hi], wattn[0:32, WM2_B + 128:WM2_B + 160].bitcast(BF16), hm1f[:, lo:hi], start=False, stop=True)
                nc.scalar.activation(out=matt1[:, lo:hi], in_=ml1_ps[:, lo:hi], func=AF.Sigmoid, bias=bias(SB_BA_B, 32))

            mg0 = wp.tile([128, MP], F16, name="mg0")
            mg1 = wp.tile([32, MP], F16, name="mg1")
            mvp = wp.tile([128, 4], F16, name="mvp")   # partial maxes per chunk
            with nc.allow_low_precision(reason="fp16 max-pool rounds values only"):
                for ci, (lo, hi) in enumerate(((0, 512), (512, MP))):
                    nc.vector.scalar_tensor_tensor(out=mg0[:, lo:hi], in0=matt0[:, lo:hi],
                                                   scalar=0.5, in1=mc0[:, lo:hi],
                                                   op0=ALU.add, op1=ALU.mult)
                    nc.vector.tensor_reduce(out=mvp[:, ci:ci + 1], in_=mg0[:, lo:hi],
                                            op=ALU.max, axis=AX.X)
                    nc.vector.scalar_tensor_tensor(out=mg1[:, lo:hi], in0=matt1[:, lo:hi],
                                                   scalar=0.5, in1=mc1[:, lo:hi],
                                                   op0=ALU.add, op1=ALU.mult)
                    nc.vector.tensor_reduce(out=mvp[0:32, 2 + ci:3 + ci], in_=mg1[:, lo:hi],
                                            op=ALU.max, axis=AX.X)
                nc.vector.tensor_reduce(out=pvf[:, 2:3], in_=mvp[:, 0:2], op=ALU.max, axis=AX.X)
                nc.vector.tensor_reduce(out=pvf[0:32, 3:4], in_=mvp[0:32, 2:4], op=ALU.max, axis=AX.X)

            # ================= FC head =================
            def lrelu(name, f_ps, bias_lo, ncols):
                fb = wp.tile([128, ncols], F32, name=name + "_b")
                nc.vector.tensor_tensor(out=fb, in0=f_ps, in1=wsmall[:, bias_lo:bias_lo + ncols], op=ALU.add)
                fs = wp.tile([128, ncols], F32, name=name + "_s")
                nc.vector.tensor_scalar(out=fs, in0=fb, scalar1=0.01, scalar2=None, op0=ALU.mult)
                fo = wp.tile([128, ncols], F16, name=name)
                nc.vector.tensor_tensor(out=fo, in0=fb, in1=fs, op=ALU.max)
                return fo

            # f1: per-column accumulation groups (one 2KB region holds all
            # columns, so groups must not interleave); within a column the
            # two 128-row stationaries go first to reduce PE config flips
            # all K=128 (W1B rows 32:128 and pvf rows 32:128 are zero) so the
            # PE streams without stationary-shape reconfigs
            f1_ps = ps.tile([128, 8], F32, name="f1_ps", tag="ps")
            for a in range(8):
                nc.tensor.matmul(f1_ps[:, a:a + 1], wfc[0:128, W1A + a * 128:W1A + a * 128 + 128],
                                 pvf[:, 0:1], start=True, stop=False)
                nc.tensor.matmul(f1_ps[:, a:a + 1], wfc[0:128, W1A + 1024 + a * 128:W1A + 1024 + a * 128 + 128],
                                 pvf[:, 2:3], start=False, stop=False)
                nc.tensor.matmul(f1_ps[:, a:a + 1], wfc[0:128, W1B + a * 128:W1B + a * 128 + 128],
                                 pvf[:, 1:2], start=False, stop=False)
                nc.tensor.matmul(f1_ps[:, a:a + 1], wfc[0:128, W1B + 1024 + a * 128:W1B + 1024 + a * 128 + 128],
                                 pvf[:, 3:4], start=False, stop=True)
            f1 = lrelu("f1", f1_ps, SB_B1, 8)

            f2_ps = ps.tile([128, 8], F32, name="f2_ps", tag="ps")
            for a in range(8):
                for jb in range(8):
                    nc.tensor.matmul(f2_ps[:, a:a + 1],
                                     wfc[0:128, W2C + jb * 1024 + a * 128:W2C + jb * 1024 + a * 128 + 128],
                                     f1[:, jb:jb + 1], start=(jb == 0), stop=(jb == 7))
            f2 = lrelu("f2", f2_ps, SB_B2, 8)

            f3_ps = ps.tile([128, 4], F32, name="f3_ps", tag="ps")
            for a in range(4):
                for jb in range(8):
                    nc.tensor.matmul(f3_ps[:, a:a + 1],
                                     wfc[0:128, W3C + jb * 512 + a * 128:W3C + jb * 512 + a * 128 + 128],
                                     f2[:, jb:jb + 1], start=(jb == 0), stop=(jb == 7))
            f3 = lrelu("f3", f3_ps, SB_B3, 4)

            o_ps = ps.tile([2, 1], F32, name="o_ps", tag="ps")
            for c in range(4):
                nc.tensor.matmul(o_ps, wfc[0:128, WOC + 2 * c:WOC + 2 * c + 2], f3[:, c:c + 1],
                                 start=(c == 0), stop=(c == 3))
            o_sb = wp.tile([2, 1], F32, name="o_sb")
            nc.vector.tensor_tensor(out=o_sb, in0=o_ps, in1=wsmall[0:2, SB_BO:SB_BO + 1], op=ALU.add)
            nc.sync.dma_start(out=out_e[:], in_=o_sb)

    _split_excess_waits(nc, max_waits=1)
    return nc


_PROGRAM = None


def _get_program():
    global _PROGRAM
    if _PROGRAM is None:
        _PROGRAM = _build_program()
    return _PROGRAM


def _prep_weights(inp):
    """Host-side packing shared by all cores."""
    import ml_dtypes
    f16 = np.float16
    bf16 = ml_dtypes.bfloat16
    f32 = lambda x: np.asarray(x, dtype=np.float32)
    as_f16bits = lambda a: np.ascontiguousarray(a).view(np.uint16).view(f16)

    def convw(w):  # [co, ci, k] -> [ci, k*co] fp16
        w = np.asarray(w, dtype=np.float32)
        ci = w.shape[1]
        return w.transpose(1, 2, 0).reshape(ci, -1).astype(f16)

    wboot = np.zeros((128, NBOOT), f16)
    def conv1_pairs(w):  # [40, 64, 4] -> two [128, 40] pair stationaries
        w = np.asarray(w, dtype=np.float32)
        out = np.zeros((128, 2 * CONV), np.float32)
        for tp in range(2):
            out[0:DIM, tp * CONV:(tp + 1) * CONV] = w[:, :, 2 * tp].T
            out[DIM:128, tp * CONV:(tp + 1) * CONV] = w[:, :, 2 * tp + 1].T
        return out.astype(f16)
    wboot[:, PW1:PW1 + 2 * CONV] = conv1_pairs(inp['pw1'])
    wboot[:, MW1:MW1 + 2 * CONV] = conv1_pairs(inp['mw1'])
    wboot[0:CONV, PW2:PW2 + K2 * C2] = convw(inp['pw2'])
    wboot[0:CONV, MW2:MW2 + K2 * C2] = convw(inp['mw2'])

    wc3 = np.zeros((128, NC3), f16)
    wc3[0:C2, PW3:PW3 + K3 * C4] = convw(inp['pw3'])
    wc3[0:C2, MW3:MW3 + K3 * C4] = convw(inp['mw3'])

    wpa, wma = f32(inp['Wpa']), f32(inp['Wma'])
    wca = f32(inp['Wa']) / float(LM3)
    wm2 = f32(inp['Wa']) / float(LP3)
    wattn = np.zeros((128, NATTN), f16)
    wattn[0:128, WPA_A:WPA_A + 160] = wpa[0:128].astype(f16)
    wattn[0:32, WPA_B:WPA_B + 160] = wpa[128:160].astype(f16)
    wattn[0:128, WMA_A:WMA_A + 128] = wma[0:128, 0:128].astype(f16)
    wattn[0:128, WMA_A + 128:WMA_A + 256] = np.tile(wma[0:128, 128:160], (1, 4)).astype(f16)
    wattn[0:32, WMA_B:WMA_B + 128] = wma[128:160, 0:128].astype(f16)
    wattn[0:32, WMA_B + 128:WMA_B + 256] = np.tile(wma[128:160, 128:160], (1, 4)).astype(f16)
    wattn[0:128, WCA_A:WCA_A + 160] = as_f16bits(wca[0:128].astype(bf16))
    wattn[0:32, WCA_B:WCA_B + 160] = as_f16bits(wca[128:160].astype(bf16))
    wattn[0:128, WM2_A:WM2_A + 160] = as_f16bits(wm2[0:128].astype(bf16))
    wattn[0:32, WM2_B:WM2_B + 160] = as_f16bits(wm2[128:160].astype(bf16))
    id128 = np.eye(128, dtype=bf16)
    idst = np.tile(np.eye(32, dtype=bf16), (4, 1))
    wattn[0:128, ID128:ID128 + 128] = as_f16bits(id128)
    wattn[0:128, IDST:IDST + 32] = as_f16bits(idst)
    for j in range(4):
        wattn[32 * j:32 * j + 32, WCB0 + j * 128:WCB0 + (j + 1) * 128] = as_f16bits(wca[128:160, 0:128].astype(bf16))
        wattn[32 * j:32 * j + 32, WCB1 + j * 32:WCB1 + (j + 1) * 32] = as_f16bits(wca[128:160, 128:160].astype(bf16))

    w1 = f32(inp['W1'])
    wfc = np.zeros((128, NFC), f16)
    wfc[0:128, W1A:W1A + 2048] = np.concatenate([w1[0:128], w1[160:288]], axis=1).astype(f16)
    wfc[0:32, W1B:W1B + 2048] = np.concatenate([w1[128:160], w1[288:320]], axis=1).astype(f16)

    def fcw(w, nblk):  # [I, J], I = nblk*128 -> [128, nblk*J]
        w = np.asarray(w, dtype=np.float32)
        i, j = w.shape
        return w.reshape(nblk, 128, j).transpose(1, 0, 2).reshape(128, nblk * j).astype(f16)

    wfc[0:128, W2C:W2C + 8192] = fcw(inp['W2'], 8)
    wfc[0:128, W3C:W3C + 4096] = fcw(inp['W3'], 8)
    wfc[0:128, WOC:WOC + 8] = fcw(inp['Wo'], 4)

    wsmall = np.zeros((128, NSMALL), np.float32)
    def bias2(col_a, col_b, b):
        b = f32(b)
        wsmall[0:128, col_a] = b[0:128]
        wsmall[0:32, col_b] = b[128:160]
    wsmall[0:CONV, SB_PB1] = f32(inp['pb1'])
    wsmall[0:C2, SB_PB2] = f32(inp['pb2'])
    bias2(SB_PB3A, SB_PB3B, inp['pb3'])
    wsmall[0:CONV, SB_MB1] = f32(inp['mb1'])
    wsmall[0:C2, SB_MB2] = f32(inp['mb2'])
    bias2(SB_MB3A, SB_MB3B, inp['mb3'])
    bias2(SB_BPA_A, SB_BPA_B, inp['bpa'])
    wsmall[0:128, SB_BMA_A] = f32(inp['bma'])[0:128]
    wsmall[0:128, SB_BMA_R4] = np.tile(f32(inp['bma'])[128:160], 4)
    bias2(SB_BA_A, SB_BA_B, inp['ba'])
    wsmall[0:128, SB_B1:SB_B1 + 8] = f32(inp['b1']).reshape(8, 128).T
    wsmall[0:128, SB_B2:SB_B2 + 8] = f32(inp['b2']).reshape(8, 128).T
    wsmall[0:128, SB_B3:SB_B3 + 4] = f32(inp['b3']).reshape(4, 128).T
    wsmall[0:2, SB_BO] = f32(inp['bo'])

    return {'wboot': wboot, 'wc3': wc3, 'wattn': wattn, 'wfc': wfc, 'wsmall': wsmall}


def _prep_core(inp, b):
    """Per-core embedding gather: [64, 1100] fp16."""
    pep = np.asarray(inp['peptide'])[b]
    mhc = np.asarray(inp['MHC'])[b]
    pe = np.asarray(inp['pep_emb'], np.float32)[pep].T   # [64, 100]
    me = np.asarray(inp['mhc_emb'], np.float32)[mhc].T   # [64, 1000]
    emb = np.concatenate([pe, me], axis=1)
    out = np.zeros((128, emb.shape[1]), np.float32)
    out[0:64] = emb
    out[64:128, 0:-1] = emb[:, 1:]          # shifted-left copy for tap pairs
    return out.astype(np.float16)


def kernel(**inputs):
    nc = _get_program()
    shared = _prep_weights(inputs)
    in_maps = []
    for b in range(B):
        m = dict(shared)
        m['emb'] = _prep_core(inputs, b)
        in_maps.append(m)
    res = run_bass_kernel_spmd(nc, in_maps, core_ids=list(range(B)))
    return np.stack([np.asarray(res.results[i]['out']).reshape(2) for i in range(B)]).astype(np.float32)


# revision 22
# speedup vs baseline: 2.7861x; 1.0062x over previous
"""AttentionDTI forward pass on 8 Trainium2 NeuronCores (pure data parallel).

One batch element per core, weights replicated. All matmul operands are
16-bit (f32 PSUM accumulation): fp16 for conv/FC weights and activations,
bf16 for the attention tiles (the Activation engine runs ~1.3x slower on
fp16 than bf16, and the attention path tolerates bf16). Embedding lookup
is done host-side; weights arrive in packed DMAs issued from both the SP
and Activation HWDGE queues so transfers overlap the NEFF prologue.

The 4D additive-attention tensor h[b,p,m,c] = relu(pa + ma) is never
materialized: mean_m(h @ Wa) == mean_m(h) @ Wa, so only hp[c,p] = sum_m h
and hm[c,m] = sum_p h are accumulated on the fly. hm accumulates in PSUM
by streaming h tiles through the PE against a stationary identity; hp
comes from fused per-tile accumulators: the Scalar engine's
relu+bias+accum activation alternates 1:1 with the Vector engine's
scalar_tensor_tensor (relu via max-with-zeros + sum accum) — any DVE op
with an accum output runs at the 1x element rate, so the fused form is
optimal on both engines.

c-channels [128:160] run packed: ma rows replicated 4x vertically (via
host-replicated stationary columns, free) so each tile covers 4 peptide
positions; a 4-stacked [128,32] identity reduces them into hm1. The
peptide gate keeps its column axis in "j-major" order (jm(p) =
(p%4)*22 + p//4) end-to-end — max-pool over p is order-invariant — which
makes the packed hp1p contributions contiguous matmuls and avoids any
unpack DMAs.

Environment constraints discovered empirically (this axon terminal):
  - GPSIMD/Pool compute ops fail codegen; SWDGE DMA hangs: DMAs go
    through SP/Activation HWDGE only.
  - tensor_tensor_reduce fails walrus codegen ("ISA wrong length").
  - tensor_scalar's accum_out hijacks op1 as the reduce op (no fused
    two-op elementwise + sum) — scalar_tensor_tensor does fuse it.
  - walrus allows at most ONE semaphore wait per instruction:
    _split_excess_waits() rewrites the scheduled program.
"""
import sys

_BASS_ROOT = '/opt/trn_rl_repo'
if _BASS_ROOT not in sys.path:
    sys.path.insert(0, _BASS_ROOT)

import numpy as np

import concourse.bass as bass
import concourse.tile as tile
from concourse import mybir
from concourse.bass_utils import run_bass_kernel_spmd

F32 = mybir.dt.float32
F16 = mybir.dt.float16
BF16 = mybir.dt.bfloat16
ALU = mybir.AluOpType
AF = mybir.ActivationFunctionType
AX = mybir.AxisListType

B = 8
LP, LM, DIM, CONV = 100, 1000, 64, 40
C2, C4 = CONV * 2, CONV * 4          # 80, 160
K1, K2, K3 = 4, 6, 8
LP1, LP2, LP3 = 97, 92, 85           # peptide conv output lengths
LM1, LM2, LM3 = 997, 992, 985        # MHC conv output lengths
NP4 = 22                             # ceil(85/4) packed p-groups
JM = 4 * NP4                         # 88 j-major gate columns
MP = 992                             # LM3 padded for 4x-eligible DVE gate ops
NEG = -30000.0
SPLIT_MOD = 2                        # i % SPLIT_MOD == 0 -> scalar h tile

# ---- wboot column map (fp16 [128, 1120]): conv1 (pair-folded) + conv2 ----
PW1, MW1, PW2, MW2 = 0, 80, 160, 640
NBOOT = 1120
# ---- wc3 column map (fp16 [128, 2560]): conv3 weights ----
PW3, MW3 = 0, 1280
NC3 = 2560
# ---- wattn column map (fp16 [128, 1632]) ----
WPA_A, WPA_B = 0, 160        # [128,160], [32,160]
WMA_A, WMA_B = 320, 576      # [128,256], [32,256] (cols 128:256 = rep4 of Wma[:,128:160])
WCA_A, WCA_B = 832, 992      # Wa/LM3: [128,160], [32,160]
WM2_A, WM2_B = 1152, 1312    # Wa/LP3
ID128, IDST = 1472, 1600     # bf16 identities: [128,128], [128,32]
WCB0, WCB1 = 1632, 2144      # j-lifted Wa[128:160]/LM3: 4x[128,128], 4x[128,32]
NATTN = 2272
# ---- wfc column map (fp16 [128, 16392]) ----
W1A, W1B = 0, 2048           # [128, 2048], [32, 2048]
W2C, W3C, WOC = 4096, 12288, 16384
NFC = 16392
# ---- wsmall column map (f32 [128, 35]) ----
SB_PB1, SB_PB2, SB_PB3A, SB_PB3B = 0, 1, 2, 3
SB_MB1, SB_MB2, SB_MB3A, SB_MB3B = 4, 5, 6, 7
SB_BPA_A, SB_BPA_B = 8, 9
SB_BMA_A, SB_BMA_R4 = 10, 11
SB_BA_A, SB_BA_B = 12, 13
SB_B1, SB_B2, SB_B3, SB_BO = 14, 22, 30, 34
NSMALL = 35


def _jm(p):
    return (p % 4) * NP4 + (p // 4)


_ctr = [0]


def _split_excess_waits(nc, max_waits=1):
    n_split = 0
    for f in nc.m.functions:
        for b in f.blocks:
            insts = list(b.instructions)
            out = []
            changed = False
            for inst in insts:
                si = inst.sync_info
                waits = list(si.on_wait) if (si is not None and si.on_wait) else []
                if len(waits) > max_waits:
                    changed = True
                    n_split += 1
                    keep = max(1, max_waits)
                    head, tail = waits[:-keep], waits[-keep:]
                    for i in range(0, len(head), keep):
                        chunk = head[i:i + keep]
                        nop = mybir.InstEventSemaphore(
                            name=f"ant-wait-split-{_ctr[0]}", ins=[], outs=[])
                        _ctr[0] += 1
                        nop.engine = inst.engine
                        nop.sync_info = mybir.SyncInfo(on_wait=chunk, on_update=[])
                        nc.register_instruction(nop)
                        out.append(nop)
                    upd = list(si.on_update) if si.on_update else []
                    inst.sync_info = mybir.SyncInfo(on_wait=tail, on_update=upd)
                out.append(inst)
            if changed:
                b.instructions = out
    return n_split


def _conv_matmuls(nc, psum, wtile, x, k_taps, co_lo, co_hi, m_lo, m_hi, cout_stride):
    """Valid 1-D conv as k shifted matmuls accumulated into `psum`."""
    for k in range(k_taps):
        nc.tensor.matmul(
            psum,
            wtile[:, k * cout_stride + co_lo: k * cout_stride + co_hi],
            x[:, m_lo + k: m_hi + k],
            start=(k == 0), stop=(k == k_taps - 1))


def _build_program():
    nc = bass.Bass("TRN2", target_bir_lowering=False, debug=False)

    emb_e = nc.declare_dram_parameter("emb", [128, LP + LM], F16, isOutput=False)
    wsmall_e = nc.declare_dram_parameter("wsmall", [128, NSMALL], F32, isOutput=False)
    wboot_e = nc.declare_dram_parameter("wboot", [128, NBOOT], F16, isOutput=False)
    wc3_e = nc.declare_dram_parameter("wc3", [128, NC3], F16, isOutput=False)
    wattn_e = nc.declare_dram_parameter("wattn", [128, NATTN], F16, isOutput=False)
    wfc_e = nc.declare_dram_parameter("wfc", [128, NFC], F16, isOutput=False)
    out_e = nc.declare_dram_parameter("out", [2, 1], F32, isOutput=True)

    with tile.TileContext(nc) as tc:
        with tc.tile_pool(name="consts", bufs=1) as cp, \
             tc.tile_pool(name="work", bufs=1) as wp, \
             tc.tile_pool(name="hpool", bufs=8) as hpool, \
             tc.tile_pool(name="ps_hm", bufs=1, space="PSUM") as ps_hm, \
             tc.tile_pool(name="ps_work", bufs=2, space="PSUM") as ps:

            # loads: small/boot/emb from SP; conv3+attn from Activation HWDGE;
            # the big FC pack last on SP.
            emb = cp.tile([128, LP + LM], F16, name="emb")
            nc.sync.dma_start(out=emb, in_=emb_e[:])
            wboot = cp.tile([128, NBOOT], F16, name="wboot")
            nc.scalar.dma_start(out=wboot, in_=wboot_e[:])
            wsmall = cp.tile([128, NSMALL], F32, name="wsmall")
            nc.sync.dma_start(out=wsmall, in_=wsmall_e[:])
            wc3 = cp.tile([128, NC3], F16, name="wc3")
            nc.scalar.dma_start(out=wc3, in_=wc3_e[:])
            wattn = cp.tile([128, NATTN], F16, name="wattn")
            nc.sync.dma_start(out=wattn, in_=wattn_e[:])
            wfc = cp.tile([128, NFC], F16, name="wfc")
            nc.sync.dma_start(out=wfc, in_=wfc_e[:])

            pe = emb[:, 0:LP]
            me = emb[:, LP:LP + LM]

            def conv1_pair(psum, w_base, x, m_lo, m_hi):
                # taps (0,1) and (2,3) folded via the stacked-shifted embedding
                for tp in range(2):
                    nc.tensor.matmul(
                        psum,
                        wboot[0:128, w_base + tp * CONV:w_base + (tp + 1) * CONV],
                        x[:, m_lo + 2 * tp:m_hi + 2 * tp],
                        start=(tp == 0), stop=(tp == 1))
            bias = lambda col, rows=128: wsmall[0:rows, col:col + 1]

            # early zero/NEG fills on the (idle) Vector engine
            zt = wp.tile([128, LM3], BF16, name="zt")
            nc.vector.memset(zt, 0.0)
            # warm the activation table (one-time ~1.3us load) off the
            # critical path, before any DMA lands
            atl = wp.tile([1, 1], F16, name="atl")
            nc.scalar.activation(out=atl, in_=zt[0:1, 0:1], func=AF.Relu, bias=0.0)
            hp0 = wp.tile([128, JM], F32, name="hp0")
            nc.vector.memset(hp0, 0.0)
            pa1p = wp.tile([128, NP4], F32, name="pa1p")
            nc.vector.memset(pa1p, NEG)
            pc0jm = wp.tile([128, JM], F16, name="pc0jm")
            nc.vector.memset(pc0jm, 0.0)
            pc1jm = wp.tile([32, JM], F16, name="pc1jm")
            nc.vector.memset(pc1jm, 0.0)
            hm0f = wp.tile([128, MP], BF16, name="hm0f")
            nc.vector.memset(hm0f[:, LM3:MP], 0.0)
            hm1f = wp.tile([32, MP], BF16, name="hm1f")
            nc.vector.memset(hm1f[:, LM3:MP], 0.0)
            mc0 = wp.tile([128, MP], F16, name="mc0")
            nc.vector.memset(mc0[:, LM3:MP], 0.0)
            mc1 = wp.tile([32, MP], F16, name="mc1")
            nc.vector.memset(mc1[:, LM3:MP], 0.0)

            # ================= conv stacks (fp16, f32 psum) =================
            # MHC conv1: [64,1000] -> [40,997]; relu chunked so conv2 starts early
            mx1_ps = ps.tile([CONV, LM1], F32, name="mx1_ps", tag="ps")
            conv1_pair(mx1_ps[:, 0:512], MW1, me, 0, 512)
            conv1_pair(mx1_ps[:, 512:LM1], MW1, me, 512, LM1)
            # peptide conv1 fills the PE while relu1 runs
            px1_ps = ps.tile([CONV, LP1], F32, name="px1_ps", tag="ps")
            conv1_pair(px1_ps, PW1, pe, 0, LP1)
            mx1 = wp.tile([CONV, LM1], F16, name="mx1")
            nc.scalar.activation(out=mx1[:, 0:520], in_=mx1_ps[:, 0:520], func=AF.Relu, bias=bias(SB_MB1, CONV))
            nc.vector.tensor_scalar(out=mx1[:, 520:LM1], in0=mx1_ps[:, 520:LM1], scalar1=bias(SB_MB1, CONV),
                                    scalar2=0.0, op0=ALU.add, op1=ALU.max)
            px1 = wp.tile([CONV, LP1], F16, name="px1")
            nc.scalar.activation(out=px1, in_=px1_ps, func=AF.Relu, bias=bias(SB_PB1, CONV))

            # MHC conv2 -> [80, 992]
            mx2_ps = ps.tile([C2, LM2], F32, name="mx2_ps", tag="ps")
            _conv_matmuls(nc, mx2_ps[:, 0:512], wboot[0:CONV, MW2:MW2 + K2 * C2], mx1, K2, 0, C2, 0, 512, C2)
            _conv_matmuls(nc, mx2_ps[:, 512:LM2], wboot[0:CONV, MW2:MW2 + K2 * C2], mx1, K2, 0, C2, 512, LM2, C2)
            px2_ps = ps.tile([C2, LP2], F32, name="px2_ps", tag="ps")
            _conv_matmuls(nc, px2_ps, wboot[0:CONV, PW2:PW2 + K2 * C2], px1, K2, 0, C2, 0, LP2, C2)
            mx2 = wp.tile([C2, LM2], F16, name="mx2")
            nc.scalar.activation(out=mx2[:, 0:520], in_=mx2_ps[:, 0:520], func=AF.Relu, bias=bias(SB_MB2, C2))
            nc.vector.tensor_scalar(out=mx2[:, 520:LM2], in0=mx2_ps[:, 520:LM2], scalar1=bias(SB_MB2, C2),
                                    scalar2=0.0, op0=ALU.add, op1=ALU.max)
            px2 = wp.tile([C2, LP2], F16, name="px2")
            nc.scalar.activation(out=px2, in_=px2_ps, func=AF.Relu, bias=bias(SB_PB2, C2))

            # MHC conv3 -> [160, 985] as [128,985]+[32,985] (into MP-padded tiles)
            mc0_ps = ps.tile([128, LM3], F32, name="mc0_ps", tag="ps")
            _conv_matmuls(nc, mc0_ps[:, 0:512], wc3[0:C2, MW3:MW3 + K3 * C4], mx2, K3, 0, 128, 0, 512, C4)
            _conv_matmuls(nc, mc0_ps[:, 512:LM3], wc3[0:C2, MW3:MW3 + K3 * C4], mx2, K3, 0, 128, 512, LM3, C4)
            nc.scalar.activation(out=mc0[:, 0:512], in_=mc0_ps[:, 0:512], func=AF.Relu, bias=bias(SB_MB3A))
            nc.vector.tensor_scalar(out=mc0[:, 512:LM3], in0=mc0_ps[:, 512:LM3], scalar1=bias(SB_MB3A),
                                    scalar2=0.0, op0=ALU.add, op1=ALU.max)
            mc1_ps = ps.tile([32, LM3], F32, name="mc1_ps", tag="ps")
            _conv_matmuls(nc, mc1_ps[:, 0:512], wc3[0:C2, MW3:MW3 + K3 * C4], mx2, K3, 128, C4, 0, 512, C4)
            _conv_matmuls(nc, mc1_ps[:, 512:LM3], wc3[0:C2, MW3:MW3 + K3 * C4], mx2, K3, 128, C4, 512, LM3, C4)
            nc.scalar.activation(out=mc1[:, 0:512], in_=mc1_ps[:, 0:512], func=AF.Relu, bias=bias(SB_MB3B, 32))
            nc.vector.tensor_scalar(out=mc1[:, 512:LM3], in0=mc1_ps[:, 512:LM3], scalar1=bias(SB_MB3B, 32),
                                    scalar2=0.0, op0=ALU.add, op1=ALU.max)

            # peptide conv3 (tiles padded to 88 cols for the 4-strided views)
            pc0_ps = ps.tile([128, LP3], F32, name="pc0_ps", tag="ps")
            _conv_matmuls(nc, pc0_ps, wc3[0:C2, PW3:PW3 + K3 * C4], px2, K3, 0, 128, 0, LP3, C4)
            pc0 = wp.tile([128, 88], F16, name="pc0")
            nc.scalar.activation(out=pc0[:, 0:LP3], in_=pc0_ps, func=AF.Relu, bias=bias(SB_PB3A))
            pc1_ps = ps.tile([32, LP3], F32, name="pc1_ps", tag="ps")
            _conv_matmuls(nc, pc1_ps, wc3[0:C2, PW3:PW3 + K3 * C4], px2, K3, 128, C4, 0, LP3, C4)
            pc1 = wp.tile([32, 88], F16, name="pc1")
            nc.scalar.activation(out=pc1[:, 0:LP3], in_=pc1_ps, func=AF.Relu, bias=bias(SB_PB3B, 32))

            # j-major copies of pc for the gate (vector, strided reads)
            pc0_g = pc0.rearrange("c (g f) -> c g f", f=4)
            pc1_g = pc1.rearrange("c (g f) -> c g f", f=4)
            for j in range(4):
                ncol = NP4 if j == 0 else NP4 - 1
                nc.vector.tensor_scalar(out=pc0jm[:, j * NP4:j * NP4 + ncol],
                                        in0=pc0_g[:, 0:ncol, j], scalar1=0.0,
                                        scalar2=None, op0=ALU.add)
                nc.vector.tensor_scalar(out=pc1jm[:, j * NP4:j * NP4 + ncol],
                                        in0=pc1_g[:, 0:ncol, j], scalar1=0.0,
                                        scalar2=None, op0=ALU.add)

            # ================= attention projections =================
            # ma0[c,m] c in 0:128 (bf16 for the Activation-engine h producer)
            ma0_ps = ps.tile([128, LM3], F32, name="ma0_ps", tag="ps")
            for lo, hi in ((0, 512), (512, LM3)):
                nc.tensor.matmul(ma0_ps[:, lo:hi], wattn[0:128, WMA_A:WMA_A + 128], mc0[:, lo:hi], start=True, stop=False)
                nc.tensor.matmul(ma0_ps[:, lo:hi], wattn[0:32, WMA_B:WMA_B + 128], mc1[:, lo:hi], start=False, stop=True)
            ma0 = wp.tile([128, LM3], BF16, name="ma0")
            nc.scalar.activation(out=ma0, in_=ma0_ps, func=AF.Identity, bias=bias(SB_BMA_A))

            # ma1p: c in 128:160 replicated 4x vertically (stationary pre-replicated)
            ma1p_ps = ps.tile([128, LM3], F32, name="ma1p_ps", tag="ps")
            for lo, hi in ((0, 512), (512, LM3)):
                nc.tensor.matmul(ma1p_ps[:, lo:hi], wattn[0:128, WMA_A + 128:WMA_A + 256], mc0[:, lo:hi], start=True, stop=False)
                nc.tensor.matmul(ma1p_ps[:, lo:hi], wattn[0:32, WMA_B + 128:WMA_B + 256], mc1[:, lo:hi], start=False, stop=True)
            ma1p = wp.tile([128, LM3], BF16, name="ma1p")
            nc.vector.tensor_scalar(out=ma1p, in0=ma1p_ps, scalar1=bias(SB_BMA_R4), scalar2=None, op0=ALU.add)

            # pa0[c,p] c in 0:128 (f32, used as per-partition bias)
            pa0_ps = ps.tile([128, LP3], F32, name="pa0_ps", tag="ps")
            nc.tensor.matmul(pa0_ps, wattn[0:128, WPA_A:WPA_A + 128], pc0[:, 0:LP3], start=True, stop=False)
            nc.tensor.matmul(pa0_ps, wattn[0:32, WPA_B:WPA_B + 128], pc1[0:32, 0:LP3], start=False, stop=True)
            pa0 = wp.tile([128, LP3], F32, name="pa0")
            nc.scalar.add(pa0, pa0_ps, bias(SB_BPA_A))

            # pa1p[32j+d, g] = pa[128+d, 4g+j]: partition-offset matmuls over
            # 4-strided moving views of pc
            pa1p_ps = ps.tile([128, NP4], F32, name="pa1p_ps", tag="ps")
            for j in range(4):
                ncol = NP4 if j == 0 else NP4 - 1
                nc.tensor.matmul(pa1p_ps[32 * j:32 * j + 32, 0:ncol],
                                 wattn[0:128, WPA_A + 128:WPA_A + 160],
                                 pc0_g[:, 0:ncol, j],
                                 start=True, stop=False, skip_group_check=True,
                                 tile_position=(0, 32 * j))
                nc.tensor.matmul(pa1p_ps[32 * j:32 * j + 32, 0:ncol],
                                 wattn[0:32, WPA_B + 128:WPA_B + 160],
                                 pc1_g[:, 0:ncol, j],
                                 start=False, stop=True, skip_group_check=True,
                                 tile_position=(0, 32 * j))
                nc.scalar.add(pa1p[32 * j:32 * j + 32, 0:ncol],
                              pa1p_ps[32 * j:32 * j + 32, 0:ncol], bias(SB_BPA_B, 32))

            # ================= 4D attention reductions =================
            # hp0 columns are written in j-major order (gate is order-free)
            hp1p = wp.tile([128, NP4], F32, name="hp1p")
            hm0_ps = ps_hm.tile([128, LM3], F32, name="hm0_ps")
            hm1_ps = ps_hm.tile([32, LM3], F32, name="hm1_ps")

            # identities and gate weights stored as bf16 bit patterns in the
            # fp16 pack; bitcast views pair them with bf16 moving operands
            id128 = wattn[0:128, ID128:ID128 + 128].bitcast(BF16)
            idst = wattn[0:128, IDST:IDST + 32].bitcast(BF16)

            # greedy producer balance: assign each tile to whichever engine
            # would finish it first (measured fused costs: ACT 1199, DVE 1263)
            prod_t = {'sc': 0.0, 've': 0.0}

            def h_tile(i, src, bias_ap, acc):
                h = hpool.tile([128, LM3], BF16, tag="h", name="h")
                if prod_t['sc'] + 1200.0 <= prod_t['ve'] + 1187.0:
                    prod_t['sc'] += 1200.0
                    nc.scalar.activation(out=h, in_=src, func=AF.Relu,
                                         bias=bias_ap, accum_out=acc)
                else:
                    prod_t['ve'] += 1187.0
                    nc.vector.scalar_tensor_tensor(out=h, in0=src, scalar=bias_ap,
                                                   in1=zt, op0=ALU.add, op1=ALU.max,
                                                   accum_out=acc)
                return h

            for p in range(LP3):
                h = h_tile(p, ma0, pa0[:, p:p + 1], hp0[:, _jm(p):_jm(p) + 1])
                nc.tensor.matmul(hm0_ps[:, 0:512], id128, h[:, 0:512],
                                 start=(p == 0), stop=(p == LP3 - 1))
                nc.tensor.matmul(hm0_ps[:, 512:LM3], id128, h[:, 512:LM3],
                                 start=(p == 0), stop=(p == LP3 - 1))

            # p-side gate head start: hp0 is complete before the packed loop
            hp0f = wp.tile([128, JM], BF16, name="hp0f")
            nc.scalar.copy(hp0f, hp0)
            cl0_ps = ps.tile([128, JM], F32, name="cl0_ps", tag="ps")
            nc.tensor.matmul(cl0_ps, wattn[0:128, WCA_A:WCA_A + 128].bitcast(BF16), hp0f,
                             start=True, stop=False, skip_group_check=True)
            cl1_ps = ps.tile([32, JM], F32, name="cl1_ps", tag="ps")
            nc.tensor.matmul(cl1_ps, wattn[0:128, WCA_A + 128:WCA_A + 160].bitcast(BF16), hp0f,
                             start=True, stop=False, skip_group_check=True)

            for g in range(NP4):
                h = h_tile(LP3 + g, ma1p, pa1p[:, g:g + 1], hp1p[:, g:g + 1])
                nc.tensor.matmul(hm1_ps[:, 0:512], idst, h[:, 0:512],
                                 start=(g == 0), stop=(g == NP4 - 1))
                nc.tensor.matmul(hm1_ps[:, 512:LM3], idst, h[:, 512:LM3],
                                 start=(g == 0), stop=(g == NP4 - 1))

            # ================= peptide gate (j-major) =================
            hp1pf = wp.tile([128, NP4], BF16, name="hp1pf")
            nc.scalar.copy(hp1pf, hp1p)
            for j in range(4):
                nc.tensor.matmul(cl0_ps[:, j * NP4:(j + 1) * NP4],
                                 wattn[0:128, WCB0 + j * 128:WCB0 + (j + 1) * 128].bitcast(BF16),
                                 hp1pf, start=False, stop=(j == 3), skip_group_check=True)
                nc.tensor.matmul(cl1_ps[:, j * NP4:(j + 1) * NP4],
                                 wattn[0:128, WCB1 + j * 32:WCB1 + (j + 1) * 32].bitcast(BF16),
                                 hp1pf, start=False, stop=(j == 3), skip_group_check=True)
            catt0 = wp.tile([128, JM], F16, name="catt0")
            nc.scalar.activation(out=catt0, in_=cl0_ps, func=AF.Sigmoid, bias=bias(SB_BA_A))
            catt1 = wp.tile([32, JM], F16, name="catt1")
            nc.scalar.activation(out=catt1, in_=cl1_ps, func=AF.Sigmoid, bias=bias(SB_BA_B, 32))

            pvf = wp.tile([128, 4], F16, name="pvf")   # cols: pv0, pv1, mv0, mv1
            nc.vector.memset(pvf, 0.0)
            pg0 = wp.tile([128, JM], F16, name="pg0")
            nc.vector.scalar_tensor_tensor(out=pg0, in0=catt0, scalar=0.5, in1=pc0jm,
                                           op0=ALU.add, op1=ALU.mult)
            pg1 = wp.tile([32, JM], F16, name="pg1")
            nc.vector.scalar_tensor_tensor(out=pg1, in0=catt1, scalar=0.5, in1=pc1jm,
                                           op0=ALU.add, op1=ALU.mult)
            with nc.allow_low_precision(reason="fp16 max-pool rounds values only"):
                nc.vector.tensor_reduce(out=pvf[:, 0:1], in_=pg0, op=ALU.max, axis=AX.X)
                nc.vector.tensor_reduce(out=pvf[0:32, 1:2], in_=pg1, op=ALU.max, axis=AX.X)

            # ================= MHC gate (chunk-pipelined) =================
            for lo, hi in ((0, 512), (512, LM3)):
                nc.scalar.activation(out=hm0f[:, lo:hi], in_=hm0_ps[:, lo:hi], func=AF.Copy)
                nc.vector.tensor_scalar(out=hm1f[:, lo:hi], in0=hm1_ps[:, lo:hi],
                                        scalar1=0.0, scalar2=None, op0=ALU.add)
            ml0_ps = ps.tile([128, MP], F32, name="ml0_ps", tag="ps")
            ml1_ps = ps.tile([32, MP], F32, name="ml1_ps", tag="ps")
            matt0 = wp.tile([128, MP], F16, name="matt0")
            matt1 = wp.tile([32, MP], F16, name="matt1")
            for lo, hi in ((0, 512), (512, MP)):
                nc.tensor.matmul(ml0_ps[:, lo:hi], wattn[0:128, WM2_A:WM2_A + 128].bitcast(BF16), hm0f[:, lo:hi], start=True, stop=False)
                nc.tensor.matmul(ml0_ps[:, lo:hi], wattn[0:32, WM2_B:WM2_B + 128].bitcast(BF16), hm1f[:, lo:hi], start=False, stop=True)
                nc.scalar.activation(out=matt0[:, lo:hi], in_=ml0_ps[:, lo:hi], func=AF.Sigmoid, bias=bias(SB_BA_A))
                nc.tensor.matmul(ml1_ps[:, lo:hi], wattn[0:128, WM2_A + 128:WM2_A + 160].bitcast(BF16), hm0f[:, lo:hi], start=True, stop=False)
                nc.tensor.matmul(ml1_ps[:, lo:hi], wattn[0:32, WM2_B + 128:WM2_B + 160].bitcast(BF16), hm1f[:, lo:hi], start=False, stop=True)
                nc.scalar.activation(out=matt1[:, lo:hi], in_=ml1_ps[:, lo:hi], func=AF.Sigmoid, bias=bias(SB_BA_B, 32))

            mg0 = wp.tile([128, MP], F16, name="mg0")
            mg1 = wp.tile([32, MP], F16, name="mg1")
            mvp = wp.tile([128, 4], F16, name="mvp")   # partial maxes per chunk
            with nc.allow_low_precision(reason="fp16 max-pool rounds values only"):
                for ci, (lo, hi) in enumerate(((0, 512), (512, MP))):
                    nc.vector.scalar_tensor_tensor(out=mg0[:, lo:hi], in0=matt0[:, lo:hi],
                                                   scalar=0.5, in1=mc0[:, lo:hi],
                                                   op0=ALU.add, op1=ALU.mult)
                    nc.vector.tensor_reduce(out=mvp[:, ci:ci + 1], in_=mg0[:, lo:hi],
                                            op=ALU.max, axis=AX.X)
                    nc.vector.scalar_tensor_tensor(out=mg1[:, lo:hi], in0=matt1[:, lo:hi],
                                                   scalar=0.5, in1=mc1[:, lo:hi],
                                                   op0=ALU.add, op1=ALU.mult)
                    nc.vector.tensor_reduce(out=mvp[0:32, 2 + ci:3 + ci], in_=mg1[:, lo:hi],
                                            op=ALU.max, axis=AX.X)
                nc.vector.tensor_reduce(out=pvf[:, 2:3], in_=mvp[:, 0:2], op=ALU.max, axis=AX.X)
                nc.vector.tensor_reduce(out=pvf[0:32, 3:4], in_=mvp[0:32, 2:4], op=ALU.max, axis=AX.X)

            # ================= FC head =================
            def lrelu(name, f_ps, bias_lo, ncols):
                fb = wp.tile([128, ncols], F32, name=name + "_b")
                nc.vector.tensor_tensor(out=fb, in0=f_ps, in1=wsmall[:, bias_lo:bias_lo + ncols], op=ALU.add)
                fs = wp.tile([128, ncols], F32, name=name + "_s")
                nc.vector.tensor_scalar(out=fs, in0=fb, scalar1=0.01, scalar2=None, op0=ALU.mult)
                fo = wp.tile([128, ncols], F16, name=name)
                nc.vector.tensor_tensor(out=fo, in0=fb, in1=fs, op=ALU.max)
                return fo

            # f1: per-column accumulation groups (one 2KB region holds all
            # columns, so groups must not interleave); within a column the
            # two 128-row stationaries go first to reduce PE config flips
            # all K=128 (W1B rows 32:128 and pvf rows 32:128 are zero) so the
            # PE streams without stationary-shape reconfigs; the peptide-side
            # half accumulates in its own psum bank as soon as pv is ready
            # (the MHC max-pool is the critical tail)
            f1p_ps = ps.tile([128, 8], F32, name="f1p_ps", tag="ps")
            for a in range(8):
                nc.tensor.matmul(f1p_ps[:, a:a + 1], wfc[0:128, W1A + a * 128:W1A + a * 128 + 128],
                                 pvf[:, 0:1], start=True, stop=False)
                nc.tensor.matmul(f1p_ps[:, a:a + 1], wfc[0:128, W1B + a * 128:W1B + a * 128 + 128],
                                 pvf[:, 1:2], start=False, stop=True)
            f1_ps = ps.tile([128, 8], F32, name="f1_ps", tag="ps")
            for a in range(8):
                nc.tensor.matmul(f1_ps[:, a:a + 1], wfc[0:128, W1A + 1024 + a * 128:W1A + 1024 + a * 128 + 128],
                                 pvf[:, 2:3], start=True, stop=False)
                nc.tensor.matmul(f1_ps[:, a:a + 1], wfc[0:128, W1B + 1024 + a * 128:W1B + 1024 + a * 128 + 128],
                                 pvf[:, 3:4], start=False, stop=True)
            fbp = wp.tile([128, 8], F32, name="fbp")
            nc.vector.tensor_tensor(out=fbp, in0=f1p_ps, in1=wsmall[:, SB_B1:SB_B1 + 8], op=ALU.add)
            fb1 = wp.tile([128, 8], F32, name="fb1")
            nc.vector.tensor_tensor(out=fb1, in0=f1_ps, in1=fbp, op=ALU.add)
            fs1 = wp.tile([128, 8], F32, name="fs1")
            nc.vector.tensor_scalar(out=fs1, in0=fb1, scalar1=0.01, scalar2=None, op0=ALU.mult)
            f1 = wp.tile([128, 8], F16, name="f1")
            nc.vector.tensor_tensor(out=f1, in0=fb1, in1=fs1, op=ALU.max)

            f2_ps = ps.tile([128, 8], F32, name="f2_ps", tag="ps")
            for a in range(8):
                for jb in range(8):
                    nc.tensor.matmul(f2_ps[:, a:a + 1],
                                     wfc[0:128, W2C + jb * 1024 + a * 128:W2C + jb * 1024 + a * 128 + 128],
                                     f1[:, jb:jb + 1], start=(jb == 0), stop=(jb == 7))
            f2 = lrelu("f2", f2_ps, SB_B2, 8)

            f3_ps = ps.tile([128, 4], F32, name="f3_ps", tag="ps")
            for a in range(4):
                for jb in range(8):
                    nc.tensor.matmul(f3_ps[:, a:a + 1],
                                     wfc[0:128, W3C + jb * 512 + a * 128:W3C + jb * 512 + a * 128 + 128],
                                     f2[:, jb:jb + 1], start=(jb == 0), stop=(jb == 7))
            f3 = lrelu("f3", f3_ps, SB_B3, 4)

            o_ps = ps.tile([2, 1], F32, name="o_ps", tag="ps")
            for c in range(4):
                nc.tensor.matmul(o_ps, wfc[0:128, WOC + 2 * c:WOC + 2 * c + 2], f3[:, c:c + 1],
                                 start=(c == 0), stop=(c == 3))
            o_sb = wp.tile([2, 1], F32, name="o_sb")
            nc.vector.tensor_tensor(out=o_sb, in0=o_ps, in1=wsmall[0:2, SB_BO:SB_BO + 1], op=ALU.add)
            nc.sync.dma_start(out=out_e[:], in_=o_sb)

    _split_excess_waits(nc, max_waits=1)
    return nc


_PROGRAM = None


def _get_program():
    global _PROGRAM
    if _PROGRAM is None:
        _PROGRAM = _build_program()
    return _PROGRAM


def _prep_weights(inp):
    """Host-side packing shared by all cores."""
    import ml_dtypes
    f16 = np.float16
    bf16 = ml_dtypes.bfloat16
    f32 = lambda x: np.asarray(x, dtype=np.float32)
    as_f16bits = lambda a: np.ascontiguousarray(a).view(np.uint16).view(f16)

    def convw(w):  # [co, ci, k] -> [ci, k*co] fp16
        w = np.asarray(w, dtype=np.float32)
        ci = w.shape[1]
        return w.transpose(1, 2, 0).reshape(ci, -1).astype(f16)

    wboot = np.zeros((128, NBOOT), f16)
    def conv1_pairs(w):  # [40, 64, 4] -> two [128, 40] pair stationaries
        w = np.asarray(w, dtype=np.float32)
        out = np.zeros((128, 2 * CONV), np.float32)
        for tp in range(2):
            out[0:DIM, tp * CONV:(tp + 1) * CONV] = w[:, :, 2 * tp].T
            out[DIM:128, tp * CONV:(tp + 1) * CONV] = w[:, :, 2 * tp + 1].T
        return out.astype(f16)
    wboot[:, PW1:PW1 + 2 * CONV] = conv1_pairs(inp['pw1'])
    wboot[:, MW1:MW1 + 2 * CONV] = conv1_pairs(inp['mw1'])
    wboot[0:CONV, PW2:PW2 + K2 * C2] = convw(inp['pw2'])
    wboot[0:CONV, MW2:MW2 + K2 * C2] = convw(inp['mw2'])

    wc3 = np.zeros((128, NC3), f16)
    wc3[0:C2, PW3:PW3 + K3 * C4] = convw(inp['pw3'])
    wc3[0:C2, MW3:MW3 + K3 * C4] = convw(inp['mw3'])

    wpa, wma = f32(inp['Wpa']), f32(inp['Wma'])
    wca = f32(inp['Wa']) / float(LM3)
    wm2 = f32(inp['Wa']) / float(LP3)
    wattn = np.zeros((128, NATTN), f16)
    wattn[0:128, WPA_A:WPA_A + 160] = wpa[0:128].astype(f16)
    wattn[0:32, WPA_B:WPA_B + 160] = wpa[128:160].astype(f16)
    wattn[0:128, WMA_A:WMA_A + 128] = wma[0:128, 0:128].astype(f16)
    wattn[0:128, WMA_A + 128:WMA_A + 256] = np.tile(wma[0:128, 128:160], (1, 4)).astype(f16)
    wattn[0:32, WMA_B:WMA_B + 128] = wma[128:160, 0:128].astype(f16)
    wattn[0:32, WMA_B + 128:WMA_B + 256] = np.tile(wma[128:160, 128:160], (1, 4)).astype(f16)
    wattn[0:128, WCA_A:WCA_A + 160] = as_f16bits(wca[0:128].astype(bf16))
    wattn[0:32, WCA_B:WCA_B + 160] = as_f16bits(wca[128:160].astype(bf16))
    wattn[0:128, WM2_A:WM2_A + 160] = as_f16bits(wm2[0:128].astype(bf16))
    wattn[0:32, WM2_B:WM2_B + 160] = as_f16bits(wm2[128:160].astype(bf16))
    id128 = np.eye(128, dtype=bf16)
    idst = np.tile(np.eye(32, dtype=bf16), (4, 1))
    wattn[0:128, ID128:ID128 + 128] = as_f16bits(id128)
    wattn[0:128, IDST:IDST + 32] = as_f16bits(idst)
    for j in range(4):
        wattn[32 * j:32 * j + 32, WCB0 + j * 128:WCB0 + (j + 1) * 128] = as_f16bits(wca[128:160, 0:128].astype(bf16))
        wattn[32 * j:32 * j + 32, WCB1 + j * 32:WCB1 + (j + 1) * 32] = as_f16bits(wca[128:160, 128:160].astype(bf16))

    w1 = f32(inp['W1'])
    wfc = np.zeros((128, NFC), f16)
    wfc[0:128, W1A:W1A + 2048] = np.concatenate([w1[0:128], w1[160:288]], axis=1).astype(f16)
    wfc[0:32, W1B:W1B + 2048] = np.concatenate([w1[128:160], w1[288:320]], axis=1).astype(f16)

    def fcw(w, nblk):  # [I, J], I = nblk*128 -> [128, nblk*J]
        w = np.asarray(w, dtype=np.float32)
        i, j = w.shape
        return w.reshape(nblk, 128, j).transpose(1, 0, 2).reshape(128, nblk * j).astype(f16)

    wfc[0:128, W2C:W2C + 8192] = fcw(inp['W2'], 8)
    wfc[0:128, W3C:W3C + 4096] = fcw(inp['W3'], 8)
    wfc[0:128, WOC:WOC + 8] = fcw(inp['Wo'], 4)

    wsmall = np.zeros((128, NSMALL), np.float32)
    def bias2(col_a, col_b, b):
        b = f32(b)
        wsmall[0:128, col_a] = b[0:128]
        wsmall[0:32, col_b] = b[128:160]
    wsmall[0:CONV, SB_PB1] = f32(inp['pb1'])
    wsmall[0:C2, SB_PB2] = f32(inp['pb2'])
    bias2(SB_PB3A, SB_PB3B, inp['pb3'])
    wsmall[0:CONV, SB_MB1] = f32(inp['mb1'])
    wsmall[0:C2, SB_MB2] = f32(inp['mb2'])
    bias2(SB_MB3A, SB_MB3B, inp['mb3'])
    bias2(SB_BPA_A, SB_BPA_B, inp['bpa'])
    wsmall[0:128, SB_BMA_A] = f32(inp['bma'])[0:128]
    wsmall[0:128, SB_BMA_R4] = np.tile(f32(inp['bma'])[128:160], 4)
    bias2(SB_BA_A, SB_BA_B, inp['ba'])
    wsmall[0:128, SB_B1:SB_B1 + 8] = f32(inp['b1']).reshape(8, 128).T
    wsmall[0:128, SB_B2:SB_B2 + 8] = f32(inp['b2']).reshape(8, 128).T
    wsmall[0:128, SB_B3:SB_B3 + 4] = f32(inp['b3']).reshape(4, 128).T
    wsmall[0:2, SB_BO] = f32(inp['bo'])

    return {'wboot': wboot, 'wc3': wc3, 'wattn': wattn, 'wfc': wfc, 'wsmall': wsmall}


def _prep_core(inp, b):
    """Per-core embedding gather: [64, 1100] fp16."""
    pep = np.asarray(inp['peptide'])[b]
    mhc = np.asarray(inp['MHC'])[b]
    pe = np.asarray(inp['pep_emb'], np.float32)[pep].T   # [64, 100]
    me = np.asarray(inp['mhc_emb'], np.float32)[mhc].T   # [64, 1000]
    emb = np.concatenate([pe, me], axis=1)
    out = np.zeros((128, emb.shape[1]), np.float32)
    out[0:64] = emb
    out[64:128, 0:-1] = emb[:, 1:]          # shifted-left copy for tap pairs
    return out.astype(np.float16)


def kernel(**inputs):
    nc = _get_program()
    shared = _prep_weights(inputs)
    in_maps = []
    for b in range(B):
        m = dict(shared)
        m['emb'] = _prep_core(inputs, b)
        in_maps.append(m)
    res = run_bass_kernel_spmd(nc, in_maps, core_ids=list(range(B)))
    return np.stack([np.asarray(res.results[i]['out']).reshape(2) for i in range(B)]).astype(np.float32)
